# revision 44
# baseline (speedup 1.0000x reference)
"""Distributed Trainium2 Bass kernel for nn_AMK_Block (kernelized-attention + ConvSwiGLU).

Sharding: sequence-parallel. Each of the 8 cores owns (batch b, query-row block q):
core = b*4 + q, rows q*512..q*512+511 of batch b, ALL heads. Each core computes
Q/K/V projections for its rows, AllGathers PhiK^T and V(+ones) across the 4 cores
of its batch group (fp8), then computes its 512 rows of attention, Wo, LN2 and
the full FFN locally. The depthwise-conv halo rows of Q_interact come from a tiny
third AllGather of boundary rows, extracted rank-agnostically with a mask-matrix
matmul. Weight matmuls run in bf16 (fp32 PSUM accumulation); the attention
kernel-matrix matmuls run in fp8 (PhiQ/PhiK/V/W^2 evicted as e4m3, Attr uses
DoubleRow packed k-pairs); norm/statistics in fp32. Wdown's first column-half is
interleaved into the FFN chunk stream so only half remains as a tail.
"""

import sys

sys.path.insert(0, "/opt/trn_rl_repo")

from contextlib import ExitStack

import ml_dtypes
import numpy as np

import concourse.bass as bass
import concourse.tile as tile
from concourse import bacc, mybir
from concourse.bass_utils import run_bass_kernel_spmd
from concourse.masks import make_identity

F32 = mybir.dt.float32
F32R = mybir.dt.float32r
BF16 = mybir.dt.bfloat16
F8 = mybir.dt.float8e4
ALU = mybir.AluOpType
AF = mybir.ActivationFunctionType
DR = mybir.MatmulPerfMode.DoubleRow

B, N, D = 2, 2048, 1024
H, DH = 16, 64
INNER = 4096
LN_EPS = 1e-5
WSC = 1.0 / 32.0          # W^2 is evicted as (W/32)^2 = W^2/1024 in fp8
WSC2 = WSC * WSC
UPS = 32.0                # Wup fp8 host scale (values ~N(0,1/32) -> ~N(0,1))
UPSC = 1.0 / UPS
WDS = 64.0                # Wdown fp8 host scale
WDINV = 1.0 / WDS

RO = 512          # owned rows per core
NQ = 4            # cores per batch group
GROUPS = [[0, 1, 2, 3], [4, 5, 6, 7]]
CHUNKS = [(0, 128), (128, 128), (256, 128), (384, 128)]
HALVES = [(0, 258), (258, 256)]  # even halves of 514; halo cols 512/513 in 2nd

_cache: dict[float, object] = {}
_last_in_maps: list | None = None


def _build(dt_safe: float):
    nc = bacc.Bacc("TRN2", target_bir_lowering=False, debug=False, num_devices=8)

    # ---------------- DRAM parameters (per-core shapes) ----------------
    p_qin = nc.declare_dram_parameter("q_in", [RO, D], F32, isOutput=False)
    p_xb1 = nc.declare_dram_parameter("x_b1", [RO, D], F32, isOutput=False)
    p_wq = nc.declare_dram_parameter("wq", [D, D], BF16, isOutput=False)
    p_wk = nc.declare_dram_parameter("wk", [D, D], BF16, isOutput=False)
    p_wv = nc.declare_dram_parameter("wv", [D, D], BF16, isOutput=False)
    p_wo = nc.declare_dram_parameter("wo", [D, D], BF16, isOutput=False)
    p_wup = nc.declare_dram_parameter("wup", [D, 2 * INNER], BF16, isOutput=False)
    p_bgu = nc.declare_dram_parameter("bias_gu", [2 * INNER], F32, isOutput=False)
    p_wd08 = nc.declare_dram_parameter("wd08", [INNER // 2, D], F8, isOutput=False)
    p_wd18 = nc.declare_dram_parameter("wd18", [INNER // 2, D], F8, isOutput=False)
    p_cw = nc.declare_dram_parameter("cw3", [INNER, 3], F32, isOutput=False)
    p_g1 = nc.declare_dram_parameter("g1", [D], F32, isOutput=False)
    p_mask = nc.declare_dram_parameter("masks", [2], F32, isOutput=False)
    p_mm = nc.declare_dram_parameter("maskmat", [2 * NQ, 2], F32R, isOutput=False)
    p_out = nc.declare_dram_parameter("out", [RO, D], F32, isOutput=True)

    with tile.TileContext(nc) as tc:
        build_ctx = ExitStack()
        with build_ctx:
            _emit(nc, tc, build_ctx, dt_safe, p_qin, p_xb1, p_wq, p_wk, p_wv,
                  p_wo, p_wup, p_bgu, p_wd08, p_wd18, p_cw, p_g1,
                  p_mask, p_mm, p_out)
    nc.finalize()
    return nc


def _emit(nc, tc, bctx, dt_safe, p_qin, p_xb1, p_wq, p_wk, p_wv, p_wo, p_wup,
          p_bgu, p_wd08, p_wd18, p_cw, p_g1, p_mask, p_mm, p_out):
    # ---------------- constant tiles ----------------
    consts = bctx.enter_context(tc.tile_pool(name="consts", bufs=1))
    g1b = consts.tile([128, D], F32, name="g1b")
    nc.sync.dma_start(
        out=g1b[:],
        in_=bass.AP(tensor=p_g1, offset=0, ap=[[0, 128], [1, D]]),
    )
    ident_f = consts.tile([128, 128], F32, name="ident_f")
    make_identity(nc, ident_f[:])
    ident_r = consts.tile([128, 128], F32R, name="ident_r")
    nc.gpsimd.dma_start(out=ident_r[:], in_=ident_f[:])
    mask_p = consts.tile([128, 1], F32, name="mask_p")
    nc.sync.dma_start(out=mask_p[:], in_=bass.AP(tensor=p_mask, offset=0, ap=[[0, 128], [1, 1]]))
    mask_n = consts.tile([128, 1], F32, name="mask_n")
    nc.sync.dma_start(out=mask_n[:], in_=bass.AP(tensor=p_mask, offset=1, ap=[[0, 128], [1, 1]]))
    ones_col = consts.tile([128, 1], F8, name="ones_col")
    nc.vector.memset(ones_col[:], 1.0)
    eps_t = consts.tile([128, 1], F32, name="eps_t")
    nc.vector.memset(eps_t[:], LN_EPS)
    maskmat = consts.tile([2 * NQ, 2], F32R, name="maskmat")
    nc.sync.dma_start(out=maskmat[:], in_=p_mm[:, :])

    # DRAM scratch for the collectives (fp8 payloads)
    dram = bctx.enter_context(tc.tile_pool(name="dram", bufs=1, space="DRAM"))
    kag_in = dram.tile([H * DH, RO], F8, name="kag_in")        # PhiK^T local slice
    kag1 = dram.tile([NQ * 512, RO], F8, name="kag1")          # gathered heads 0-7
    kag2 = dram.tile([NQ * 512, RO], F8, name="kag2")          # gathered heads 8-15
    vag_in = dram.tile([RO, H * 66], F8, name="vag_in")        # V(+ones) local rows
    vag1 = dram.tile([NQ * 256, H * 66], F8, name="vag1")      # gathered rows 0-255
    vag2 = dram.tile([NQ * 256, H * 66], F8, name="vag2")      # gathered rows 256-511
    hag_in = dram.tile([2, D], F32R, name="hag_in")            # my boundary Qint rows
    hag = dram.tile([2 * NQ, D], F32R, name="hag")             # gathered boundaries

    ev_state = [0]

    def evict_copy(dst_ap, src_ap):
        ev_state[0] += 1
        if ev_state[0] % 2 == 0:
            nc.vector.tensor_copy(dst_ap, src_ap)
        else:
            nc.scalar.activation(dst_ap, src_ap, AF.Copy)

    ln_pool = bctx.enter_context(tc.tile_pool(name="ln", bufs=3))

    def layernorm_to(x_ap, p):
        """Returns (mv, rstd) tiles: mean in mv[:,0:1], rstd [p,1], for x_ap [p, D]."""
        st = ln_pool.tile([128, 2, 6], F32, tag="bn_st")
        xr = x_ap.rearrange("p (s f) -> p s f", s=2)
        for s in range(2):
            nc.vector.bn_stats(st[:p, s, :], xr[:, s, :])
        mv = ln_pool.tile([128, 2], F32, tag="bn_mv")
        nc.vector.bn_aggr(mv[:p], st[:p])
        rstd = ln_pool.tile([128, 1], F32, tag="bn_rstd")
        nc.scalar.activation(rstd[:p], mv[:p, 1:2], AF.Sqrt, bias=eps_t[:p, 0:1])
        nc.vector.reciprocal(rstd[:p], rstd[:p])
        return mv, rstd

    # ---- lifetime stacks (must nest LIFO): f34 > av2 > av > phase stacks ----
    f34_stack = ExitStack()   # hfc + qint: from Wo until the end
    av2_stack = ExitStack()   # mTc: until end of Wo
    av_stack = ExitStack()    # vT, phiQ: until end of head loop
    hfc_pool = f34_stack.enter_context(tc.tile_pool(name="hfc", bufs=1))
    qint_pool = f34_stack.enter_context(tc.tile_pool(name="qint", bufs=1))
    mTc_pool = av2_stack.enter_context(tc.tile_pool(name="mTc", bufs=1))
    vT_pool = av_stack.enter_context(tc.tile_pool(name="vT", bufs=1))
    phiQ_pool = av_stack.enter_context(tc.tile_pool(name="phiQ", bufs=1))
    mTc = [mTc_pool.tile([128, RO], BF16, name=f"mTc{j}") for j in range(8)]
    vT = [vT_pool.tile([128, RO], F32R, name=f"vT{j}") for j in range(8)]
    phiQT = [phiQ_pool.tile([128, RO], F8, name=f"phiQT{j}") for j in range(8)]

    # ---------------- Phase P: LN1 + Hc + transposes ----------------
    p_stack = ExitStack()
    hcT_pool = p_stack.enter_context(tc.tile_pool(name="hcT", bufs=1))
    hcT = [hcT_pool.tile([128, RO], BF16, name=f"hcT{j}") for j in range(8)]
    io_pool = p_stack.enter_context(tc.tile_pool(name="io", bufs=4))
    hc_pool = p_stack.enter_context(tc.tile_pool(name="hc", bufs=2))
    # the three PSUM phases of P (hc transposes / projections / V transposes)
    # are disjoint in time, so each gets its own short-lived 4-buffer pool
    t_stack = ExitStack()
    pp_t = t_stack.enter_context(tc.tile_pool(name="pp_t", bufs=4, space="PSUM"))

    # stream all qin/xb1 chunks up front on two queues
    qin_ts, xb1_ts = [], []
    for i, (r0, p) in enumerate(CHUNKS):
        qin_t = io_pool.tile([p, D], F32, tag="qin")
        nc.sync.dma_start(out=qin_t[:], in_=p_qin[r0:r0 + p, :])
        qin_ts.append(qin_t)
        xb1_t = io_pool.tile([p, D], F32, tag="xb1")
        nc.scalar.dma_start(out=xb1_t[:], in_=p_xb1[r0:r0 + p, :])
        xb1_ts.append(xb1_t)

    for i, (r0, p) in enumerate(CHUNKS):
        qin_t, xb1_t = qin_ts[i], xb1_ts[i]
        mv, rstd = layernorm_to(qin_t[:p, :], p)
        hc_t = hc_pool.tile([p, D], F32, tag="hc")
        nc.vector.tensor_scalar(
            out=hc_t[:p, :], in0=qin_t[:p, :], scalar1=mv[:p, 0:1],
            scalar2=rstd[:p, 0:1], op0=ALU.subtract, op1=ALU.mult,
        )
        nc.vector.tensor_mul(hc_t[:p, :], hc_t[:p, :], g1b[:p, :])
        nc.vector.tensor_add(hc_t[:p, :], hc_t[:p, :], xb1_t[:p, :])

        # transpose this row-chunk into the 8 hcT column tiles
        for j in range(8):
            tp = pp_t.tile([128, 128], F32, tag="tp")
            nc.tensor.transpose(tp[:128, :p], hc_t[:p, j * 128:(j + 1) * 128], ident_f[:p, :p])
            evict_copy(hcT[j][:, r0:r0 + p], tp[:128, :p])
    t_stack.close()

    # ---------------- Phase P: projections ----------------
    # Order: K -> K-AllGather (smallest latency to first collective), then Q
    # (needed with K for the W matmuls), then V -> V-AllGather. The rings
    # serialize on the collective lane, so K's goes first.
    wstream = p_stack.enter_context(tc.tile_pool(name="wstream", bufs=12))
    pr_stack = ExitStack()
    pp_a = pr_stack.enter_context(tc.tile_pool(name="pp_a", bufs=4, space="PSUM"))
    elu_pool = p_stack.enter_context(tc.tile_pool(name="elu", bufs=3))

    def elu1_evict(dst_ap, src_psum_ap, p, w):
        """dst = elu(src)+1 = relu(src) + exp(min(src,0)) (fp8 out)"""
        tmin = elu_pool.tile([128, 512], F32, tag="tmin")
        nc.vector.tensor_scalar_min(tmin[:p, :w], src_psum_ap, 0.0)
        texp = elu_pool.tile([128, 512], F32, tag="texp")
        nc.scalar.activation(texp[:p, :w], tmin[:p, :w], AF.Exp)
        nc.vector.scalar_tensor_tensor(
            out=dst_ap, in0=src_psum_ap, scalar=0.0, in1=texp[:p, :w],
            op0=ALU.max, op1=ALU.add,
        )

    # K^T -> PhiK^T (fp8) -> kag_in; two chunked AllGathers (heads 0-7, 8-15)
    wk_sb = []
    for k in range(8):
        w_t = wstream.tile([128, D], BF16, tag="wproj")
        nc.sync.dma_start(out=w_t[:], in_=p_wk[k * 128:(k + 1) * 128, :])
        wk_sb.append(w_t)
    phiK_pool = p_stack.enter_context(tc.tile_pool(name="phiK", bufs=4))
    for j in range(8):
        ps = pp_a.tile([128, 512], F32, tag="proj")
        for k in range(8):
            nc.tensor.matmul(
                ps[:], wk_sb[k][:, j * 128:(j + 1) * 128],
                hcT[k][:, 0:RO], start=(k == 0), stop=(k == 7),
            )
        phiK_t = phiK_pool.tile([128, RO], F8, tag="phiK")
        elu1_evict(phiK_t[:, :], ps[:], 128, RO)
        nc.sync.dma_start(out=kag_in[j * 128:(j + 1) * 128, :], in_=phiK_t[:])
        if j == 3:
            nc.gpsimd.collective_compute(
                "AllGather", ALU.bypass, replica_groups=GROUPS,
                ins=[kag_in[0:512, :].opt()], outs=[kag1[:].opt()],
            )
    nc.gpsimd.collective_compute(
        "AllGather", ALU.bypass, replica_groups=GROUPS,
        ins=[kag_in[512:1024, :].opt()], outs=[kag2[:].opt()],
    )

    # Q^T -> PhiQ^T (fp8, kept in SBUF)
    wq_sb = []
    for k in range(8):
        w_t = wstream.tile([128, D], BF16, tag="wproj")
        nc.scalar.dma_start(out=w_t[:], in_=p_wq[k * 128:(k + 1) * 128, :])
        wq_sb.append(w_t)
    for j in range(8):
        ps = pp_a.tile([128, 512], F32, tag="proj")
        for k in range(8):
            nc.tensor.matmul(
                ps[:], wq_sb[k][:, j * 128:(j + 1) * 128],
                hcT[k][:, 0:RO], start=(k == 0), stop=(k == 7),
            )
        elu1_evict(phiQT[j][:, :], ps[:], 128, RO)

    # V^T, then transpose back to row-major (+ones cols, fp8) and stage for AGs
    wv_sb = []
    for k in range(8):
        w_t = wstream.tile([128, D], BF16, tag="wproj")
        nc.sync.dma_start(out=w_t[:], in_=p_wv[k * 128:(k + 1) * 128, :])
        wv_sb.append(w_t)
    for j in range(8):
        ps = pp_a.tile([128, 512], F32, tag="proj")
        for k in range(8):
            nc.tensor.matmul(
                ps[:], wv_sb[k][:, j * 128:(j + 1) * 128],
                hcT[k][:, 0:RO], start=(k == 0), stop=(k == 7),
            )
        evict_copy(vT[j][:, :], ps[:])
    pr_stack.close()

    tv_stack = ExitStack()
    pp_tv = tv_stack.enter_context(tc.tile_pool(name="pp_tv", bufs=4, space="PSUM"))
    vs_pool = p_stack.enter_context(tc.tile_pool(name="vs", bufs=3))
    for i in range(4):
        r0 = i * 128
        vstage = vs_pool.tile([128, H * 66], F8, tag="vstage")
        for j in range(8):
            tpv = pp_tv.tile([128, 128], F32R, tag="tpv")
            nc.tensor.transpose(tpv[:], vT[j][:, r0:r0 + 128], ident_r[:])
            h0, h1 = 2 * j, 2 * j + 1
            evict_copy(vstage[:, h0 * 66:h0 * 66 + 64], tpv[:, 0:64])
            evict_copy(vstage[:, h1 * 66:h1 * 66 + 64], tpv[:, 64:128])
        # per-head ones column (64) + zero pad column (65), strided memsets
        vsr = vstage[:].rearrange("p (h d) -> p h d", h=H)
        nc.vector.memset(vsr[:, :, 64:65], 1.0)
        nc.vector.memset(vsr[:, :, 65:66], 0.0)
        nc.gpsimd.dma_start(out=vag_in[r0:r0 + 128, :], in_=vstage[:])
        if i == 1:
            nc.gpsimd.collective_compute(
                "AllGather", ALU.bypass, replica_groups=GROUPS,
                ins=[vag_in[0:256, :].opt()], outs=[vag1[:].opt()],
            )
    nc.gpsimd.collective_compute(
        "AllGather", ALU.bypass, replica_groups=GROUPS,
        ins=[vag_in[256:512, :].opt()], outs=[vag2[:].opt()],
    )
    tv_stack.close()

    p_stack.close()

    # ---------------- Phase A: attention ----------------
    # Per head-group g (4 heads): W(g) = 64 fp8 matmuls (K=64) evicted as
    # (W/32)^2 fp8 into DoubleRow pair tiles; Attr(g) = per head 8 fp8-DR
    # matmuls over (m-block pair, key) tiles. Emission order W0 W1 A0 W2 A1
    # W3 A2 A3 keeps the PE busy while the V AllGathers land.
    # Wo weights: pool created first (released after attention pools), loads
    # issued now so the Wo phase starts instantly
    wo_stack = ExitStack()
    wo_pool = wo_stack.enter_context(tc.tile_pool(name="wo", bufs=8))
    wo_sb = []
    for k in range(8):
        w_t = wo_pool.tile([128, D], BF16, tag="wo")
        nc.scalar.dma_start(out=w_t[:], in_=p_wo[k * 128:(k + 1) * 128, :])
        wo_sb.append(w_t)

    a_stack = ExitStack()
    kq_pool = a_stack.enter_context(tc.tile_pool(name="kq", bufs=3))
    vhd_pool = a_stack.enter_context(tc.tile_pool(name="vhd", bufs=16))
    # dual-fp8 LDWEIGHTS needs stationary width % 32 == 0: vhd is 96 wide
    # (V 0-63, ones 64, pad 65-95). DMA writes cols 0-65; zero the pad cols
    # once per pool buffer (round-robin reuse keeps them zero).
    for _ in range(16):
        vz = vhd_pool.tile([128, 8, 96], F8, tag="vhd")
        nc.vector.memset(vz[:, :, 66:96], 0.0)
    wt_pool = a_stack.enter_context(tc.tile_pool(name="wt", bufs=64))
    asm_pool = a_stack.enter_context(tc.tile_pool(name="asm", bufs=3))
    pp_w = a_stack.enter_context(tc.tile_pool(name="pp_w", bufs=3, space="PSUM"))
    pp_at = a_stack.enter_context(tc.tile_pool(name="pp_at", bufs=2, space="PSUM"))

    sq_state = [0]

    def square_evict(dst_ap, src_psum_ap):
        """dst = src^2 fp8 over a 2-bank-wide [128, 1024] PSUM region; src is
        already W/32 (W > 0, relu is a no-op). Rotated 3:1 across ACT/DVE:
        ACT streams ~1 col/ns single-pass; the DVE two-pass path costs ~2.2x
        that, so it only takes the overflow share."""
        sq_state[0] = (sq_state[0] + 1) % 4
        if sq_state[0] < 3:
            nc.scalar.activation(dst_ap, src_psum_ap, AF.Square, scale=WSC)
        else:
            tr = asm_pool.tile([128, 1024], BF16, tag="r2tmpv")
            nc.vector.tensor_scalar_mul(tr[:, :], src_psum_ap, WSC)
            nc.vector.tensor_mul(dst_ap, tr[:, :], tr[:, :])

    def emit_w(hg):
        """W^T for 4 heads of group hg -> wtp fp8 DoubleRow pair tiles.
        The two heads sharing a kq/phiQT tile (PE row halves 0-63 / 64-127)
        are interleaved: consecutive matmuls hit disjoint row groups, so
        LDWEIGHTS overlaps the in-flight matmul and the two matmuls of a
        pair execute concurrently. Both m-blocks of a wt pair tile fill one
        [128, 1024] 2-bank PSUM tile, evicted by a single wide op."""
        kag_t = kag1 if hg < 2 else kag2
        kq_sb = {}
        for j2 in (2 * hg, 2 * hg + 1):
            hrow = (j2 % 4) * 128  # row offset of head-pair j2 within kag_t
            kt = kq_pool.tile([128, NQ, RO], F8, tag="kq")
            ksrc = kag_t[:, :]
            nc.sync.dma_start(
                out=kt[:, :, :],
                in_=bass.AP(tensor=ksrc.tensor, offset=ksrc.offset + hrow * RO,
                            ap=[[RO, 128], [512 * RO, NQ], [1, RO]]),
            )
            kq_sb[j2] = kt
        wtp = {}
        for hh in range(4):
            h = hg * 4 + hh
            wtp[h] = [wt_pool.tile([128, 2, RO], F8, tag="wt", name=f"wt{h}_{t}")
                      for t in range(8)]
        for j2 in (2 * hg, 2 * hg + 1):
            hA, hB = 2 * j2, 2 * j2 + 1
            for t in range(8):
                qq = t // 2
                psA = pp_w.tile([128, 1024], F32, tag="psw")
                psB = pp_w.tile([128, 1024], F32, tag="psw")
                ps = {0: psA, 64: psB}
                for half in range(2):
                    m = 2 * t + half
                    lc = m % 4
                    for off in (0, 64):
                        nc.tensor.matmul(
                            ps[off][:, half * 512:(half + 1) * 512],
                            kq_sb[j2][off:off + 64, qq, lc * 128:(lc + 1) * 128],
                            phiQT[j2][off:off + 64, :], start=True, stop=True,
                        )
                square_evict(wtp[hA][t][:, :, :], ps[0][:, :])
                square_evict(wtp[hB][t][:, :, :], ps[64][:, :])
        return wtp

    # pair order follows the chunked V gathers: vag1 pairs (lc 0,1) first
    T_ORDER = [qq * 2 for qq in range(NQ)] + [qq * 2 + 1 for qq in range(NQ)]

    def emit_attr(hg, wtp):
        pats = []
        for hh in range(4):
            h = hg * 4 + hh
            pat = pp_at.tile([96, 512], F32, tag="pat", name=f"pat{h}")
            vh = {}
            for half in range(2):
                vsrc = vag1 if half == 0 else vag2
                vt = vhd_pool.tile([128, 8, 96], F8, tag="vhd")
                vap = vsrc[:, :]
                W16 = H * 66
                nc.sync.dma_start(
                    out=vt[:, :, 0:66],
                    in_=bass.AP(tensor=vap.tensor, offset=vap.offset + h * 66,
                                ap=[[W16, 128], [128 * W16, 8], [1, 66]]))
                vh[half] = vt
            for ti, t in enumerate(T_ORDER):
                qq, half = t // 2, t % 2
                nc.tensor.matmul(
                    pat[:], vh[half][:, 2 * qq:2 * qq + 2, :], wtp[h][t][:, :, :],
                    start=(ti == 0), stop=(ti == 7), perf_mode=DR,
                )
            pats.append(pat)
        for hh in range(4):
            h = hg * 4 + hh
            j2, off = h // 2, (h % 2) * 64
            nrm = asm_pool.tile([1, RO], F32, tag="nrm")
            nc.vector.tensor_scalar_add(nrm[0:1, :], pats[hh][64:65, :], WSC2)
            nc.vector.reciprocal_approx_fast(out=nrm[:], in_=nrm[:])
            rcb = asm_pool.tile([64, RO], F32, tag="rcb")
            nc.gpsimd.partition_broadcast(rcb[:], nrm[:])
            tm = asm_pool.tile([128, RO], F32, tag="tm")
            nc.vector.tensor_mul(tm[off:off + 64, :], pats[hh][0:64, :], rcb[:, :])
            nc.gpsimd.tensor_sub(
                mTc[j2][off:off + 64, :], tm[off:off + 64, :],
                vT[j2][off:off + 64, :],
            )

    wtp_q = [emit_w(0), emit_w(1)]
    for hg in range(4):
        emit_attr(hg, wtp_q[hg])
        if hg + 2 < 4:
            wtp_q.append(emit_w(hg + 2))
    a_stack.close()

    # ---------------- Phase A5: Wo + Q_interact ----------------
    # Chunk order 0,3,1,2 so the conv-halo boundary rows exist after two
    # chunks and their AllGather overlaps the rest of Wo + LN2.
    a5_stack = ExitStack()
    qi_pool = a5_stack.enter_context(tc.tile_pool(name="qi", bufs=3))
    pp_o = a5_stack.enter_context(tc.tile_pool(name="pp_o", bufs=4, space="PSUM"))
    qint = [None] * 4
    for oi, i in enumerate((0, 3, 1, 2)):
        r0, p = CHUNKS[i]
        qin_t = qi_pool.tile([p, D], F32, tag="qin2")
        nc.sync.dma_start(out=qin_t[:], in_=p_qin[r0:r0 + p, :])
        qi = qint_pool.tile([p, D], F32, name=f"qint{i}")
        for half in range(2):
            pso = pp_o.tile([128, 512], F32, tag="pso")
            for k in range(8):
                nc.tensor.matmul(
                    pso[:p, :], mTc[k][:, r0:r0 + p],
                    wo_sb[k][:, half * 512:(half + 1) * 512],
                    start=(k == 0), stop=(k == 7),
                )
            nc.vector.scalar_tensor_tensor(
                out=qi[:p, half * 512:(half + 1) * 512], in0=pso[:p, :],
                scalar=dt_safe, in1=qin_t[:p, half * 512:(half + 1) * 512],
                op0=ALU.mult, op1=ALU.add,
            )
        qint[i] = qi
        if oi == 1:
            # boundary rows ready: stage + AllGather (conv halo exchange)
            nc.gpsimd.dma_start(out=hag_in[0:1, :], in_=qint[0][0:1, :])
            nc.gpsimd.dma_start(out=hag_in[1:2, :], in_=qint[3][127:128, :])
            nc.gpsimd.collective_compute(
                "AllGather", ALU.bypass, replica_groups=GROUPS,
                ins=[hag_in[:].opt()], outs=[hag[:].opt()],
            )
    a5_stack.close()
    wo_stack.close()
    av_stack.close()   # frees vT, phiQ
    av2_stack.close()  # frees mTc

    # ---------------- Phase F: LN2 + transpose + FFN ----------------
    hfc = []

    qn2T_pool = f34_stack.enter_context(tc.tile_pool(name="qn2T", bufs=1))
    f_stack = ExitStack()
    qn2_pool = f_stack.enter_context(tc.tile_pool(name="qn2", bufs=2))
    pp_f = f_stack.enter_context(tc.tile_pool(name="pp_f", bufs=4, space="PSUM"))
    # qn2T cols: 0..511 owned rows, 512 = prev-halo row, 513 = next-halo row
    qn2T = [qn2T_pool.tile([128, RO + 2], BF16, name=f"qn2T{j}") for j in range(8)]
    for i, (r0, p) in enumerate(CHUNKS):
        mv, rstd = layernorm_to(qint[i][:p, :], p)
        qn2_t = qn2_pool.tile([p, D], F32, tag="qn2")
        nc.vector.tensor_scalar(
            out=qn2_t[:p, :], in0=qint[i][:p, :], scalar1=mv[:p, 0:1],
            scalar2=rstd[:p, 0:1], op0=ALU.subtract, op1=ALU.mult,
        )
        for j in range(8):
            tp = pp_f.tile([128, 128], F32, tag="tpf")
            nc.tensor.transpose(tp[:128, :p], qn2_t[:p, j * 128:(j + 1) * 128], ident_f[:p, :p])
            evict_copy(qn2T[j][:, r0:r0 + p], tp[:128, :p])

    # halo rows: extract prev/next boundary rows via maskmat.T @ gathered,
    # then LN2 + transpose into qn2T cols 512/513
    pp_h = f_stack.enter_context(tc.tile_pool(name="pp_h", bufs=1, space="PSUM"))
    hg_sb = qn2_pool.tile([2 * NQ, D], F32R, name="hg_sb")
    nc.sync.dma_start(out=hg_sb[:], in_=hag[:, :])
    qih = qn2_pool.tile([2, D], F32, name="qih")
    for half in range(2):
        ph = pp_h.tile([2, 512], F32, tag="psh", name=f"ph{half}")
        nc.tensor.matmul(
            ph[:], maskmat[:], hg_sb[:, half * 512:(half + 1) * 512],
            start=True, stop=True,
        )
        nc.vector.tensor_copy(qih[:, half * 512:(half + 1) * 512], ph[:])
    mv, rstd = layernorm_to(qih[:2, :], 2)
    qn2h = qn2_pool.tile([2, D], F32, name="qn2h")
    nc.vector.tensor_scalar(
        out=qn2h[:2, :], in0=qih[:2, :], scalar1=mv[:2, 0:1],
        scalar2=rstd[:2, 0:1], op0=ALU.subtract, op1=ALU.mult,
    )
    for j in range(8):
        tp = pp_f.tile([128, 128], F32, tag="tpf")
        nc.tensor.transpose(tp[:128, :2], qn2h[:2, j * 128:(j + 1) * 128], ident_f[:2, :2])
        evict_copy(qn2T[j][:, RO:RO + 2], tp[:128, :2])
    f_stack.close()

    # Wup (fp8 DoubleRow, K=256 per matmul) + SwiGLU + depthwise conv, in
    # 512-col superchunks; Wdown's first column-half rides along, one inner
    # pair behind the conv. Scales: wup carries x32, wdown x64 (host side);
    # the 1/32 descale folds into the Silu input scale / U bias / conv taps,
    # the 1/64 into the output eviction.
    f2_stack = ExitStack()
    pp_d = f2_stack.enter_context(tc.tile_pool(name="pp_d", bufs=1, space="PSUM"))
    ffn_stack = ExitStack()
    pp_g = ffn_stack.enter_context(tc.tile_pool(name="pp_g", bufs=2, space="PSUM"))
    pp_u = ffn_stack.enter_context(tc.tile_pool(name="pp_u", bufs=2, space="PSUM"))
    wup_pool = ffn_stack.enter_context(tc.tile_pool(name="wup", bufs=12))
    wupu_pool = ffn_stack.enter_context(tc.tile_pool(name="wupu", bufs=12))
    fsm_pool = ffn_stack.enter_context(tc.tile_pool(name="fsm", bufs=3))
    bias_pool = ffn_stack.enter_context(tc.tile_pool(name="bias", bufs=6))
    wd0_pool = ffn_stack.enter_context(tc.tile_pool(name="wd0", bufs=4))

    psd0 = [pp_d.tile([128, 512], F32, name=f"psd0_{i}") for i in range(4)]
    # hfc: fp8 DoubleRow pair tiles; pair t holds inner blocks (2t, 2t+1)
    for t in range(16):
        hfc.append(hfc_pool.tile([128, 2, RO], F8, name=f"hfc{t}"))
    wd0_sb = {}

    def emit_wdown_pair(t):
        wd_t = wd0_sb.pop(t)
        for i in range(4):
            nc.tensor.matmul(
                psd0[i][:], hfc[t][:, :, i * 128:(i + 1) * 128],
                wd_t[:, :, :], start=(t == 0), stop=(t == 15), perf_mode=DR,
            )

    for sc in range(8):
        wupg_sb, wupu_sb = [], []
        for k in range(8):
            wg = wup_pool.tile([128, 512], BF16, tag="wupg")
            nc.sync.dma_start(
                out=wg[:], in_=p_wup[k * 128:(k + 1) * 128, sc * 512:(sc + 1) * 512]
            )
            wupg_sb.append(wg)
            wu = wupu_pool.tile([128, 512], BF16, tag="wupu")
            nc.scalar.dma_start(
                out=wu[:], in_=p_wup[k * 128:(k + 1) * 128, INNER + sc * 512:INNER + (sc + 1) * 512]
            )
            wupu_sb.append(wu)
        for c in range(4):
            cc = sc * 4 + c
            if cc % 2 == 0:
                tn = cc // 2
                wd_t = wd0_pool.tile([128, 2, 512], F8, tag="wd0")
                nc.sync.dma_start(out=wd_t[:, :, :], in_=p_wd08[tn * 128:(tn + 1) * 128, :])
                wd0_sb[tn] = wd_t
            bg = bias_pool.tile([128, 1], F32, tag="bg")
            nc.sync.dma_start(out=bg[:], in_=bass.AP(tensor=p_bgu, offset=cc * 128, ap=[[1, 128], [1, 1]]))
            bu = bias_pool.tile([128, 1], F32, tag="bu")
            nc.sync.dma_start(out=bu[:], in_=bass.AP(tensor=p_bgu, offset=INNER + cc * 128, ap=[[1, 128], [1, 1]]))
            cw = bias_pool.tile([128, 3], F32, tag="cw")
            nc.sync.dma_start(out=cw[:], in_=p_cw[cc * 128:(cc + 1) * 128, :])

            gact = fsm_pool.tile([128, RO + 2], F32, tag="gact")
            hf = fsm_pool.tile([128, RO + 2], F32, tag="hf")
            for h0, w in HALVES:
                psg = pp_g.tile([128, 258], F32, tag="psg")
                for k in range(8):
                    nc.tensor.matmul(
                        psg[:, :w], wupg_sb[k][:, c * 128:(c + 1) * 128],
                        qn2T[k][:, h0:h0 + w], start=(k == 0), stop=(k == 7),
                    )
                nc.scalar.activation(gact[:, h0:h0 + w], psg[:, :w], AF.Silu, bias=bg[:, 0:1])
                psu = pp_u.tile([128, 258], F32, tag="psu")
                for k in range(8):
                    nc.tensor.matmul(
                        psu[:, :w], wupu_sb[k][:, c * 128:(c + 1) * 128],
                        qn2T[k][:, h0:h0 + w], start=(k == 0), stop=(k == 7),
                    )
                nc.vector.scalar_tensor_tensor(
                    out=hf[:, h0:h0 + w], in0=psu[:, :w], scalar=bu[:, 0:1],
                    in1=gact[:, h0:h0 + w], op0=ALU.add, op1=ALU.mult,
                )
            if cc >= 2 and cc % 2 == 0:
                emit_wdown_pair(cc // 2 - 1)
            # mask halo cols at batch edges (conv zero-pad)
            nc.vector.tensor_scalar_mul(hf[:, RO:RO + 1], hf[:, RO:RO + 1], mask_p[:, 0:1])
            nc.vector.tensor_scalar_mul(hf[:, RO + 1:RO + 2], hf[:, RO + 1:RO + 2], mask_n[:, 0:1])
            # depthwise conv along rows: cols 0..511 owned, 512=prev, 513=next.
            # center tap on ACT (scale is per-partition), side taps DVE; the
            # final two taps write the fp8 DoubleRow pair tile directly.
            hfb = fsm_pool.tile([128, RO], BF16, tag="hfb")
            dst = hfc[cc // 2]
            jj = cc % 2
            nc.scalar.activation(hfb[:, 0:RO], hf[:, 0:RO], AF.Copy, scale=cw[:, 1:2])
            nc.vector.scalar_tensor_tensor(
                out=hfb[:, 1:RO], in0=hf[:, 0:RO - 1], scalar=cw[:, 0:1],
                in1=hfb[:, 1:RO], op0=ALU.mult, op1=ALU.add,
            )
            nc.vector.scalar_tensor_tensor(
                out=hfb[:, 0:1], in0=hf[:, RO:RO + 1], scalar=cw[:, 0:1],
                in1=hfb[:, 0:1], op0=ALU.mult, op1=ALU.add,
            )
            nc.vector.scalar_tensor_tensor(
                out=dst[:, jj, 0:RO - 1], in0=hf[:, 1:RO], scalar=cw[:, 2:3],
                in1=hfb[:, 0:RO - 1], op0=ALU.mult, op1=ALU.add,
            )
            nc.vector.scalar_tensor_tensor(
                out=dst[:, jj, RO - 1:RO], in0=hf[:, RO + 1:RO + 2], scalar=cw[:, 2:3],
                in1=hfb[:, RO - 1:RO], op0=ALU.mult, op1=ALU.add,
            )
    emit_wdown_pair(15)
    ffn_stack.close()

    # ---------------- Phase F4: Wdown second half + residual + output ----------
    out_pool = f2_stack.enter_context(tc.tile_pool(name="outp", bufs=4))
    wd1_pool = f2_stack.enter_context(tc.tile_pool(name="wd1", bufs=6))
    pp_d1 = f2_stack.enter_context(tc.tile_pool(name="pp_d1", bufs=1, space="PSUM"))
    psd1 = [pp_d1.tile([128, 512], F32, name=f"psd1_{i}") for i in range(4)]
    for t in range(16):
        wd_t = wd1_pool.tile([128, 2, 512], F8, tag="wd1")
        nc.sync.dma_start(out=wd_t[:, :, :], in_=p_wd18[t * 128:(t + 1) * 128, :])
        for i in range(4):
            nc.tensor.matmul(
                psd1[i][:], hfc[t][:, :, i * 128:(i + 1) * 128],
                wd_t[:, :, :], start=(t == 0), stop=(t == 15), perf_mode=DR,
            )
    for i in range(4):
        o_t = out_pool.tile([128, D], F32, tag="osb")
        nc.vector.scalar_tensor_tensor(
            out=o_t[:, 0:512], in0=psd0[i][:], scalar=WDINV,
            in1=qint[i][:, 0:512], op0=ALU.mult, op1=ALU.add,
        )
        nc.vector.scalar_tensor_tensor(
            out=o_t[:, 512:1024], in0=psd1[i][:], scalar=WDINV,
            in1=qint[i][:, 512:1024], op0=ALU.mult, op1=ALU.add,
        )
        out_q = (nc.sync, nc.scalar, nc.gpsimd, nc.sync)[i]
        out_q.dma_start(out=p_out[i * 128:(i + 1) * 128, :], in_=o_t[:])
    f2_stack.close()
    f34_stack.close()


def kernel(**inputs) -> np.ndarray:
    Q_in = np.ascontiguousarray(np.asarray(inputs["Q_in"], dtype=np.float32))
    X = np.ascontiguousarray(np.asarray(inputs["X"], dtype=np.float32))
    Wq = np.asarray(inputs["Wq"], dtype=np.float32)
    Wk = np.asarray(inputs["Wk"], dtype=np.float32)
    Wv = np.asarray(inputs["Wv"], dtype=np.float32)
    Wo = np.asarray(inputs["Wo"], dtype=np.float32)
    Wup = np.asarray(inputs["Wup"], dtype=np.float32)
    conv_w = np.asarray(inputs["conv_w"], dtype=np.float32)
    Wdown = np.asarray(inputs["Wdown"], dtype=np.float32)
    g1 = np.asarray(inputs["g1"], dtype=np.float32)
    b1 = np.asarray(inputs["b1"], dtype=np.float32)
    g2 = np.asarray(inputs["g2"], dtype=np.float32)
    b2 = np.asarray(inputs["b2"], dtype=np.float32)
    dt = float(np.asarray(inputs["dt"], dtype=np.float32))

    # softplus(dt) on host; baked into the NEFF as an immediate
    dt_safe = float(np.log1p(np.exp(dt)))

    # fold g2/b2 into Wup (LN2's affine commutes into the up-projection)
    wup_f = g2[:, None] * Wup
    bias_gu = np.ascontiguousarray(b2 @ Wup)
    cw3 = np.ascontiguousarray(conv_w[:, 0, :])

    key = round(dt_safe, 9)
    if key not in _cache:
        _cache[key] = _build(dt_safe)
    nc = _cache[key]

    bf = ml_dtypes.bfloat16
    f8 = ml_dtypes.float8_e4m3
    wq_b = np.ascontiguousarray(Wq.astype(bf))
    wk_b = np.ascontiguousarray(Wk.astype(bf))
    wv_b = np.ascontiguousarray(Wv.astype(bf))
    wo_b = np.ascontiguousarray(Wo.astype(bf))
    wup_b = np.ascontiguousarray(wup_f.astype(bf))

    # Wdown DoubleRow pair packing: tile-row t*128+r holds inner element
    # i = 128*(2t+j)+r in slot j; cols are (j, c).
    def pack_down(w):  # [INNER, 512] -> [INNER//2, D] fp8 pair layout
        t = w.reshape(16, 2, 128, 512)
        return np.ascontiguousarray(
            t.transpose(0, 2, 1, 3).reshape(INNER // 2, D).astype(f8))

    wd08 = pack_down(WDS * Wdown[:, 0:512])
    wd18 = pack_down(WDS * Wdown[:, 512:1024])

    in_maps = []
    for core in range(8):
        b, q = divmod(core, 4)
        qin = np.ascontiguousarray(Q_in[b, q * RO:(q + 1) * RO])
        xb1 = np.ascontiguousarray(X[b, q * RO:(q + 1) * RO] + b1[None, :])
        masks = np.array(
            [1.0 if q > 0 else 0.0, 1.0 if q < NQ - 1 else 0.0], dtype=np.float32
        )
        # maskmat.T @ gathered_boundaries = [prev-halo row; next-halo row]
        mm = np.zeros((2 * NQ, 2), dtype=np.float32)
        if q > 0:
            mm[2 * (q - 1) + 1, 0] = 1.0
        if q < NQ - 1:
            mm[2 * (q + 1), 1] = 1.0
        in_maps.append({
            "q_in": qin, "x_b1": xb1, "wq": wq_b, "wk": wk_b, "wv": wv_b,
            "wo": wo_b, "wup": wup_b, "bias_gu": bias_gu,
            "wd08": wd08, "wd18": wd18, "cw3": cw3,
            "g1": np.ascontiguousarray(g1), "masks": masks, "maskmat": mm,
        })

    global _last_in_maps
    _last_in_maps = in_maps
    res = run_bass_kernel_spmd(nc, in_maps, core_ids=list(range(8)))

    out = np.empty((B, N, D), dtype=np.float32)
    for core in range(8):
        b, q = divmod(core, 4)
        out[b, q * RO:(q + 1) * RO] = res.results[core]["out"]
    return out



# revision 58
# speedup vs baseline: 1.0806x; 1.0806x over previous
"""Distributed Trainium2 Bass kernel for nn_AMK_Block (kernelized-attention + ConvSwiGLU).

Sharding: sequence-parallel. Each of the 8 cores owns (batch b, query-row block q):
core = b*4 + q, rows q*512..q*512+511 of batch b, ALL heads. Each core computes
Q/K/V projections for its rows, AllGathers PhiK^T and V(+ones) across the 4 cores
of its batch group (fp8), then computes its 512 rows of attention, Wo, LN2 and
the full FFN locally. The depthwise-conv halo rows of Q_interact come from a tiny
third AllGather of boundary rows, extracted rank-agnostically with a mask-matrix
matmul. Weight matmuls run in bf16 (fp32 PSUM accumulation); the attention
kernel-matrix matmuls run in fp8 (PhiQ/PhiK/V/W^2 evicted as e4m3, Attr uses
DoubleRow packed k-pairs); norm/statistics in fp32. Wdown's first column-half is
interleaved into the FFN chunk stream so only half remains as a tail.
"""

import sys

sys.path.insert(0, "/opt/trn_rl_repo")

from contextlib import ExitStack

import ml_dtypes
import numpy as np

import concourse.bass as bass
import concourse.tile as tile
from concourse import bacc, mybir
from concourse.bass_utils import run_bass_kernel_spmd
from concourse.masks import make_identity

F32 = mybir.dt.float32
F32R = mybir.dt.float32r
BF16 = mybir.dt.bfloat16
F8 = mybir.dt.float8e4
ALU = mybir.AluOpType
AF = mybir.ActivationFunctionType
DR = mybir.MatmulPerfMode.DoubleRow

B, N, D = 2, 2048, 1024
H, DH = 16, 64
INNER = 4096
LN_EPS = 1e-5
WSC = 1.0 / 32.0          # W^2 is evicted as (W/32)^2 = W^2/1024 in fp8
WSC2 = WSC * WSC
UPS = 32.0                # Wup fp8 host scale (values ~N(0,1/32) -> ~N(0,1))
UPSC = 1.0 / UPS
WDS = 64.0                # Wdown fp8 host scale
WDINV = 1.0 / WDS

RO = 512          # owned rows per core
NQ = 4            # cores per batch group
GROUPS = [[0, 1, 2, 3], [4, 5, 6, 7]]
CHUNKS = [(0, 128), (128, 128), (256, 128), (384, 128)]
HALVES = [(0, 258), (258, 256)]  # even halves of 514; halo cols 512/513 in 2nd

_cache: dict[float, object] = {}
_last_in_maps: list | None = None


def _build(dt_safe: float):
    nc = bacc.Bacc("TRN2", target_bir_lowering=False, debug=False, num_devices=8)

    # ---------------- DRAM parameters (per-core shapes) ----------------
    p_qin = nc.declare_dram_parameter("q_in", [RO, D], F32, isOutput=False)
    p_xb1 = nc.declare_dram_parameter("x_b1", [RO, D], F32, isOutput=False)
    p_wq = nc.declare_dram_parameter("wq", [D, D], BF16, isOutput=False)
    p_wk = nc.declare_dram_parameter("wk", [D, D], BF16, isOutput=False)
    p_wv = nc.declare_dram_parameter("wv", [D, D], BF16, isOutput=False)
    p_wo = nc.declare_dram_parameter("wo", [D, D], BF16, isOutput=False)
    p_wup = nc.declare_dram_parameter("wup", [D, 2 * INNER], BF16, isOutput=False)
    p_bgu = nc.declare_dram_parameter("bias_gu", [2 * INNER], F32, isOutput=False)
    p_wd08 = nc.declare_dram_parameter("wd08", [INNER // 2, D], F8, isOutput=False)
    p_wd18 = nc.declare_dram_parameter("wd18", [INNER // 2, D], F8, isOutput=False)
    p_cw = nc.declare_dram_parameter("cw3", [INNER, 3], F32, isOutput=False)
    p_g1 = nc.declare_dram_parameter("g1", [D], F32, isOutput=False)
    p_mask = nc.declare_dram_parameter("masks", [2], F32, isOutput=False)
    p_mm = nc.declare_dram_parameter("maskmat", [2 * NQ, 2], F32R, isOutput=False)
    p_out = nc.declare_dram_parameter("out", [RO, D], F32, isOutput=True)

    with tile.TileContext(nc) as tc:
        build_ctx = ExitStack()
        with build_ctx:
            _emit(nc, tc, build_ctx, dt_safe, p_qin, p_xb1, p_wq, p_wk, p_wv,
                  p_wo, p_wup, p_bgu, p_wd08, p_wd18, p_cw, p_g1,
                  p_mask, p_mm, p_out)
    nc.finalize()
    return nc


def _emit(nc, tc, bctx, dt_safe, p_qin, p_xb1, p_wq, p_wk, p_wv, p_wo, p_wup,
          p_bgu, p_wd08, p_wd18, p_cw, p_g1, p_mask, p_mm, p_out):
    # ---------------- constant tiles ----------------
    consts = bctx.enter_context(tc.tile_pool(name="consts", bufs=1))
    g1b = consts.tile([128, D], F32, name="g1b")
    nc.sync.dma_start(
        out=g1b[:],
        in_=bass.AP(tensor=p_g1, offset=0, ap=[[0, 128], [1, D]]),
    )
    ident_f = consts.tile([128, 128], F32, name="ident_f")
    make_identity(nc, ident_f[:])
    ident_r = consts.tile([128, 128], F32R, name="ident_r")
    nc.gpsimd.dma_start(out=ident_r[:], in_=ident_f[:])
    mask_p = consts.tile([128, 1], F32, name="mask_p")
    nc.sync.dma_start(out=mask_p[:], in_=bass.AP(tensor=p_mask, offset=0, ap=[[0, 128], [1, 1]]))
    mask_n = consts.tile([128, 1], F32, name="mask_n")
    nc.sync.dma_start(out=mask_n[:], in_=bass.AP(tensor=p_mask, offset=1, ap=[[0, 128], [1, 1]]))
    ones_col = consts.tile([128, 1], F8, name="ones_col")
    nc.vector.memset(ones_col[:], 1.0)
    eps_t = consts.tile([128, 1], F32, name="eps_t")
    nc.vector.memset(eps_t[:], LN_EPS)
    maskmat = consts.tile([2 * NQ, 2], F32R, name="maskmat")
    nc.sync.dma_start(out=maskmat[:], in_=p_mm[:, :])

    # DRAM scratch for the collectives (fp8 payloads)
    dram = bctx.enter_context(tc.tile_pool(name="dram", bufs=1, space="DRAM"))
    kag_in = dram.tile([H * DH, RO], F8, name="kag_in")        # PhiK^T local slice
    kag1 = dram.tile([NQ * 512, RO], F8, name="kag1")          # gathered heads 0-7
    kag2 = dram.tile([NQ * 512, RO], F8, name="kag2")          # gathered heads 8-15
    vag_in = dram.tile([RO, H * 66], F8, name="vag_in")        # V(+ones) local rows
    vag1 = dram.tile([NQ * 256, H * 66], F8, name="vag1")      # gathered rows 0-255
    vag2 = dram.tile([NQ * 256, H * 66], F8, name="vag2")      # gathered rows 256-511
    hag_in = dram.tile([2, D], F32R, name="hag_in")            # my boundary Qint rows
    hag = dram.tile([2 * NQ, D], F32R, name="hag")             # gathered boundaries

    ev_state = [0]

    def evict_copy(dst_ap, src_ap):
        ev_state[0] += 1
        if ev_state[0] % 2 == 0:
            nc.vector.tensor_copy(dst_ap, src_ap)
        else:
            nc.scalar.activation(dst_ap, src_ap, AF.Copy)

    ln_pool = bctx.enter_context(tc.tile_pool(name="ln", bufs=3))

    def layernorm_to(x_ap, p):
        """Returns (mv, rstd) tiles: mean in mv[:,0:1], rstd [p,1], for x_ap [p, D]."""
        st = ln_pool.tile([128, 2, 6], F32, tag="bn_st")
        xr = x_ap.rearrange("p (s f) -> p s f", s=2)
        for s in range(2):
            nc.vector.bn_stats(st[:p, s, :], xr[:, s, :])
        mv = ln_pool.tile([128, 2], F32, tag="bn_mv")
        nc.vector.bn_aggr(mv[:p], st[:p])
        rstd = ln_pool.tile([128, 1], F32, tag="bn_rstd")
        nc.scalar.activation(rstd[:p], mv[:p, 1:2], AF.Sqrt, bias=eps_t[:p, 0:1])
        nc.vector.reciprocal(rstd[:p], rstd[:p])
        return mv, rstd

    # ---- lifetime stacks (must nest LIFO): f34 > av2 > av > phase stacks ----
    f34_stack = ExitStack()   # hfc + qint: from Wo until the end
    av2_stack = ExitStack()   # mTc: until end of Wo
    av_stack = ExitStack()    # vT, phiQ: until end of head loop
    hfc_pool = f34_stack.enter_context(tc.tile_pool(name="hfc", bufs=1))
    qint_pool = f34_stack.enter_context(tc.tile_pool(name="qint", bufs=1))
    mTc_pool = av2_stack.enter_context(tc.tile_pool(name="mTc", bufs=1))
    vT_pool = av_stack.enter_context(tc.tile_pool(name="vT", bufs=1))
    phiQ_pool = av_stack.enter_context(tc.tile_pool(name="phiQ", bufs=1))
    mTc = [mTc_pool.tile([128, RO], BF16, name=f"mTc{j}") for j in range(8)]
    vT = [vT_pool.tile([128, RO], F32R, name=f"vT{j}") for j in range(8)]
    phiQT = [phiQ_pool.tile([128, RO], F8, name=f"phiQT{j}") for j in range(8)]

    # ---------------- Phase P: LN1 + Hc + transposes ----------------
    p_stack = ExitStack()
    hcT_pool = p_stack.enter_context(tc.tile_pool(name="hcT", bufs=1))
    hcT = [hcT_pool.tile([128, RO], BF16, name=f"hcT{j}") for j in range(8)]
    io_pool = p_stack.enter_context(tc.tile_pool(name="io", bufs=4))
    hc_pool = p_stack.enter_context(tc.tile_pool(name="hc", bufs=2))
    # the PSUM phases of P (hc transposes / projections / V transposes) are
    # disjoint in time, so each gets its own short-lived 4-6 buffer pool
    t_stack = ExitStack()
    pp_t = t_stack.enter_context(tc.tile_pool(name="pp_t", bufs=4, space="PSUM"))

    # stream all qin/xb1 chunks up front on two queues
    qin_ts, xb1_ts = [], []
    for i, (r0, p) in enumerate(CHUNKS):
        qin_t = io_pool.tile([p, D], F32, tag="qin")
        nc.sync.dma_start(out=qin_t[:], in_=p_qin[r0:r0 + p, :])
        qin_ts.append(qin_t)
        xb1_t = io_pool.tile([p, D], F32, tag="xb1")
        nc.scalar.dma_start(out=xb1_t[:], in_=p_xb1[r0:r0 + p, :])
        xb1_ts.append(xb1_t)

    for i, (r0, p) in enumerate(CHUNKS):
        qin_t, xb1_t = qin_ts[i], xb1_ts[i]
        mv, rstd = layernorm_to(qin_t[:p, :], p)
        hc_t = hc_pool.tile([p, D], F32, tag="hc")
        nc.vector.tensor_scalar(
            out=hc_t[:p, :], in0=qin_t[:p, :], scalar1=mv[:p, 0:1],
            scalar2=rstd[:p, 0:1], op0=ALU.subtract, op1=ALU.mult,
        )
        nc.vector.tensor_mul(hc_t[:p, :], hc_t[:p, :], g1b[:p, :])
        nc.vector.tensor_add(hc_t[:p, :], hc_t[:p, :], xb1_t[:p, :])

        # transpose this row-chunk into the 8 hcT column tiles
        for j in range(8):
            tp = pp_t.tile([128, 128], F32, tag="tp")
            nc.tensor.transpose(tp[:128, :p], hc_t[:p, j * 128:(j + 1) * 128], ident_f[:p, :p])
            evict_copy(hcT[j][:, r0:r0 + p], tp[:128, :p])
    t_stack.close()

    # ---------------- Phase P: projections ----------------
    # Order: K -> K-AllGather (smallest latency to first collective), then Q
    # (needed with K for the W matmuls), then V -> V-AllGather. The rings
    # serialize on the collective lane, so K's goes first.
    wstream = p_stack.enter_context(tc.tile_pool(name="wstream", bufs=12))
    pr_stack = ExitStack()
    pp_a = pr_stack.enter_context(tc.tile_pool(name="pp_a", bufs=6, space="PSUM"))
    elu_pool = p_stack.enter_context(tc.tile_pool(name="elu", bufs=3))

    def elu1_evict(dst_ap, src_psum_ap, p, w):
        """dst = elu(src)+1 = relu(src) + exp(min(src,0)) (fp8 out)"""
        tmin = elu_pool.tile([128, 512], F32, tag="tmin")
        nc.vector.tensor_scalar_min(tmin[:p, :w], src_psum_ap, 0.0)
        texp = elu_pool.tile([128, 512], F32, tag="texp")
        nc.scalar.activation(texp[:p, :w], tmin[:p, :w], AF.Exp)
        nc.vector.scalar_tensor_tensor(
            out=dst_ap, in0=src_psum_ap, scalar=0.0, in1=texp[:p, :w],
            op0=ALU.max, op1=ALU.add,
        )

    # K^T -> PhiK^T (fp8) -> kag_in; two chunked AllGathers (heads 0-7, 8-15)
    wk_sb = []
    for k in range(8):
        w_t = wstream.tile([128, D], BF16, tag="wproj")
        nc.sync.dma_start(out=w_t[:], in_=p_wk[k * 128:(k + 1) * 128, :])
        wk_sb.append(w_t)
    phiK_pool = p_stack.enter_context(tc.tile_pool(name="phiK", bufs=4))
    for j in range(8):
        ps = pp_a.tile([128, 512], F32, tag="proj")
        for k in range(8):
            nc.tensor.matmul(
                ps[:], wk_sb[k][:, j * 128:(j + 1) * 128],
                hcT[k][:, 0:RO], start=(k == 0), stop=(k == 7),
            )
        phiK_t = phiK_pool.tile([128, RO], F8, tag="phiK")
        elu1_evict(phiK_t[:, :], ps[:], 128, RO)
        nc.sync.dma_start(out=kag_in[j * 128:(j + 1) * 128, :], in_=phiK_t[:])
        if j == 3:
            nc.gpsimd.collective_compute(
                "AllGather", ALU.bypass, replica_groups=GROUPS,
                ins=[kag_in[0:512, :].opt()], outs=[kag1[:].opt()],
            )
    nc.gpsimd.collective_compute(
        "AllGather", ALU.bypass, replica_groups=GROUPS,
        ins=[kag_in[512:1024, :].opt()], outs=[kag2[:].opt()],
    )

    # Q^T -> PhiQ^T (fp8, kept in SBUF)
    wq_sb = []
    for k in range(8):
        w_t = wstream.tile([128, D], BF16, tag="wproj")
        nc.scalar.dma_start(out=w_t[:], in_=p_wq[k * 128:(k + 1) * 128, :])
        wq_sb.append(w_t)
    for j in range(8):
        ps = pp_a.tile([128, 512], F32, tag="proj")
        for k in range(8):
            nc.tensor.matmul(
                ps[:], wq_sb[k][:, j * 128:(j + 1) * 128],
                hcT[k][:, 0:RO], start=(k == 0), stop=(k == 7),
            )
        elu1_evict(phiQT[j][:, :], ps[:], 128, RO)

    # V^T, then transpose back to row-major (+ones cols, fp8) and stage for AGs
    wv_sb = []
    for k in range(8):
        w_t = wstream.tile([128, D], BF16, tag="wproj")
        nc.sync.dma_start(out=w_t[:], in_=p_wv[k * 128:(k + 1) * 128, :])
        wv_sb.append(w_t)
    for j in range(8):
        ps = pp_a.tile([128, 512], F32, tag="proj")
        for k in range(8):
            nc.tensor.matmul(
                ps[:], wv_sb[k][:, j * 128:(j + 1) * 128],
                hcT[k][:, 0:RO], start=(k == 0), stop=(k == 7),
            )
        evict_copy(vT[j][:, :], ps[:])
    pr_stack.close()

    tv_stack = ExitStack()
    pp_tv = tv_stack.enter_context(tc.tile_pool(name="pp_tv", bufs=4, space="PSUM"))
    vs_pool = p_stack.enter_context(tc.tile_pool(name="vs", bufs=3))
    for i in range(4):
        r0 = i * 128
        vstage = vs_pool.tile([128, H * 66], F8, tag="vstage")
        for j in range(8):
            tpv = pp_tv.tile([128, 128], F32R, tag="tpv")
            nc.tensor.transpose(tpv[:], vT[j][:, r0:r0 + 128], ident_r[:])
            h0, h1 = 2 * j, 2 * j + 1
            evict_copy(vstage[:, h0 * 66:h0 * 66 + 64], tpv[:, 0:64])
            evict_copy(vstage[:, h1 * 66:h1 * 66 + 64], tpv[:, 64:128])
        # per-head ones column (64) + zero pad column (65), strided memsets
        vsr = vstage[:].rearrange("p (h d) -> p h d", h=H)
        nc.vector.memset(vsr[:, :, 64:65], 1.0)
        nc.vector.memset(vsr[:, :, 65:66], 0.0)
        nc.gpsimd.dma_start(out=vag_in[r0:r0 + 128, :], in_=vstage[:])
        if i == 1:
            nc.gpsimd.collective_compute(
                "AllGather", ALU.bypass, replica_groups=GROUPS,
                ins=[vag_in[0:256, :].opt()], outs=[vag1[:].opt()],
            )
    nc.gpsimd.collective_compute(
        "AllGather", ALU.bypass, replica_groups=GROUPS,
        ins=[vag_in[256:512, :].opt()], outs=[vag2[:].opt()],
    )
    tv_stack.close()

    p_stack.close()

    # ---------------- Phase A: attention ----------------
    # Per head-group g (4 heads): W(g) = 64 fp8 matmuls (K=64) evicted as
    # (W/32)^2 fp8 into DoubleRow pair tiles; Attr(g) = per head 8 fp8-DR
    # matmuls over (m-block pair, key) tiles. Emission order W0 W1 A0 W2 A1
    # W3 A2 A3 keeps the PE busy while the V AllGathers land.
    # Wo weights: pool created first (released after attention pools), loads
    # issued now so the Wo phase starts instantly
    wo_stack = ExitStack()
    wo_pool = wo_stack.enter_context(tc.tile_pool(name="wo", bufs=8))
    wo_sb = []
    for k in range(8):
        w_t = wo_pool.tile([128, D], BF16, tag="wo")
        nc.scalar.dma_start(out=w_t[:], in_=p_wo[k * 128:(k + 1) * 128, :])
        wo_sb.append(w_t)

    a_stack = ExitStack()
    kq_pool = a_stack.enter_context(tc.tile_pool(name="kq", bufs=3))
    vhd_pool = a_stack.enter_context(tc.tile_pool(name="vhd", bufs=16))
    # dual-fp8 LDWEIGHTS needs stationary width % 32 == 0: vhd is 96 wide
    # (V 0-63, ones 64, pad 65-95). DMA writes cols 0-65; zero the pad cols
    # once per pool buffer (round-robin reuse keeps them zero).
    for _ in range(16):
        vz = vhd_pool.tile([128, 8, 96], F8, tag="vhd")
        nc.vector.memset(vz[:, :, 66:96], 0.0)
    wt_pool = a_stack.enter_context(tc.tile_pool(name="wt", bufs=64))
    asm_pool = a_stack.enter_context(tc.tile_pool(name="asm", bufs=3))
    pp_w = a_stack.enter_context(tc.tile_pool(name="pp_w", bufs=4, space="PSUM"))
    pp_at = a_stack.enter_context(tc.tile_pool(name="pp_at", bufs=4, space="PSUM"))

    sq_state = [0]

    def square_evict(dst_ap, src_psum_ap):
        """dst = src^2 fp8; src is already W/32 (W > 0, relu is a no-op).
        Rotated 5:2 across ACT/DVE: ACT streams ~1 col/ns single-pass; the
        DVE two-pass path costs ~2x that, so it only soaks the overflow."""
        sq_state[0] = (sq_state[0] + 1) % 7
        if sq_state[0] < 5:
            nc.scalar.activation(dst_ap, src_psum_ap, AF.Square, scale=WSC)
        else:
            tr = asm_pool.tile([128, 512], BF16, tag="r2tmpv")
            nc.vector.tensor_scalar_mul(tr[:, :], src_psum_ap, WSC)
            nc.vector.tensor_mul(dst_ap, tr[:, :], tr[:, :])

    def emit_w(hg):
        """W^T for 4 heads of group hg -> wtp fp8 DoubleRow pair tiles.
        The two heads sharing a kq/phiQT tile (PE row halves 0-63 / 64-127)
        are interleaved: consecutive matmuls hit disjoint row groups, so
        LDWEIGHTS overlaps the in-flight matmul."""
        kag_t = kag1 if hg < 2 else kag2
        kq_sb = {}
        for j2 in (2 * hg, 2 * hg + 1):
            hrow = (j2 % 4) * 128  # row offset of head-pair j2 within kag_t
            kt = kq_pool.tile([128, NQ, RO], F8, tag="kq")
            ksrc = kag_t[:, :]
            nc.sync.dma_start(
                out=kt[:, :, :],
                in_=bass.AP(tensor=ksrc.tensor, offset=ksrc.offset + hrow * RO,
                            ap=[[RO, 128], [512 * RO, NQ], [1, RO]]),
            )
            kq_sb[j2] = kt
        wtp = {}
        for hh in range(4):
            h = hg * 4 + hh
            wtp[h] = [wt_pool.tile([128, 2, RO], F8, tag="wt", name=f"wt{h}_{t}")
                      for t in range(8)]
        for j2 in (2 * hg, 2 * hg + 1):
            hA, hB = 2 * j2, 2 * j2 + 1
            for m in range(16):
                qq, lc = m // 4, m % 4
                for off, h in ((0, hA), (64, hB)):
                    psw = pp_w.tile([128, 512], F32, tag="psw")
                    nc.tensor.matmul(
                        psw[:], kq_sb[j2][off:off + 64, qq, lc * 128:(lc + 1) * 128],
                        phiQT[j2][off:off + 64, :], start=True, stop=True,
                    )
                    square_evict(wtp[h][m // 2][:, m % 2, :], psw)
        return wtp

    # pair order follows the chunked V gathers: vag1 pairs (lc 0,1) first
    T_ORDER = [qq * 2 for qq in range(NQ)] + [qq * 2 + 1 for qq in range(NQ)]

    def emit_attr(hg, wtp):
        pats = []
        for hh in range(4):
            h = hg * 4 + hh
            pat = pp_at.tile([96, 512], F32, tag="pat", name=f"pat{h}")
            vh = {}
            for half in range(2):
                vsrc = vag1 if half == 0 else vag2
                vt = vhd_pool.tile([128, 8, 96], F8, tag="vhd")
                vap = vsrc[:, :]
                W16 = H * 66
                nc.sync.dma_start(
                    out=vt[:, :, 0:66],
                    in_=bass.AP(tensor=vap.tensor, offset=vap.offset + h * 66,
                                ap=[[W16, 128], [128 * W16, 8], [1, 66]]))
                vh[half] = vt
            for ti, t in enumerate(T_ORDER):
                qq, half = t // 2, t % 2
                nc.tensor.matmul(
                    pat[:], vh[half][:, 2 * qq:2 * qq + 2, :], wtp[h][t][:, :, :],
                    start=(ti == 0), stop=(ti == 7), perf_mode=DR,
                )
            pats.append(pat)
        for hh in range(4):
            h = hg * 4 + hh
            j2, off = h // 2, (h % 2) * 64
            nrm = asm_pool.tile([1, RO], F32, tag="nrm")
            nc.vector.tensor_scalar_add(nrm[0:1, :], pats[hh][64:65, :], WSC2)
            nc.vector.reciprocal_approx_fast(out=nrm[:], in_=nrm[:])
            rcb = asm_pool.tile([64, RO], F32, tag="rcb")
            nc.gpsimd.partition_broadcast(rcb[:], nrm[:])
            tm = asm_pool.tile([128, RO], F32, tag="tm")
            nc.vector.tensor_mul(tm[off:off + 64, :], pats[hh][0:64, :], rcb[:, :])
            nc.gpsimd.tensor_sub(
                mTc[j2][off:off + 64, :], tm[off:off + 64, :],
                vT[j2][off:off + 64, :],
            )

    wtp_q = [emit_w(0), emit_w(1)]
    for hg in range(4):
        emit_attr(hg, wtp_q[hg])
        if hg + 2 < 4:
            wtp_q.append(emit_w(hg + 2))
    a_stack.close()

    # ---------------- Phase A5: Wo + Q_interact ----------------
    # Chunk order 0,3,1,2 so the conv-halo boundary rows exist after two
    # chunks and their AllGather overlaps the rest of Wo + LN2.
    a5_stack = ExitStack()
    qi_pool = a5_stack.enter_context(tc.tile_pool(name="qi", bufs=3))
    pp_o = a5_stack.enter_context(tc.tile_pool(name="pp_o", bufs=4, space="PSUM"))
    qint = [None] * 4
    for oi, i in enumerate((0, 3, 1, 2)):
        r0, p = CHUNKS[i]
        qin_t = qi_pool.tile([p, D], F32, tag="qin2")
        nc.sync.dma_start(out=qin_t[:], in_=p_qin[r0:r0 + p, :])
        qi = qint_pool.tile([p, D], F32, name=f"qint{i}")
        for half in range(2):
            pso = pp_o.tile([128, 512], F32, tag="pso")
            for k in range(8):
                nc.tensor.matmul(
                    pso[:p, :], mTc[k][:, r0:r0 + p],
                    wo_sb[k][:, half * 512:(half + 1) * 512],
                    start=(k == 0), stop=(k == 7),
                )
            nc.vector.scalar_tensor_tensor(
                out=qi[:p, half * 512:(half + 1) * 512], in0=pso[:p, :],
                scalar=dt_safe, in1=qin_t[:p, half * 512:(half + 1) * 512],
                op0=ALU.mult, op1=ALU.add,
            )
        qint[i] = qi
        if oi == 1:
            # boundary rows ready: stage + AllGather (conv halo exchange)
            nc.gpsimd.dma_start(out=hag_in[0:1, :], in_=qint[0][0:1, :])
            nc.gpsimd.dma_start(out=hag_in[1:2, :], in_=qint[3][127:128, :])
            nc.gpsimd.collective_compute(
                "AllGather", ALU.bypass, replica_groups=GROUPS,
                ins=[hag_in[:].opt()], outs=[hag[:].opt()],
            )
    a5_stack.close()
    wo_stack.close()
    av_stack.close()   # frees vT, phiQ
    av2_stack.close()  # frees mTc

    # ---------------- Phase F: LN2 + transpose + FFN ----------------
    hfc = []

    qn2T_pool = f34_stack.enter_context(tc.tile_pool(name="qn2T", bufs=1))
    f_stack = ExitStack()
    qn2_pool = f_stack.enter_context(tc.tile_pool(name="qn2", bufs=2))
    pp_f = f_stack.enter_context(tc.tile_pool(name="pp_f", bufs=4, space="PSUM"))
    # qn2T cols: 0..511 owned rows, 512 = prev-halo row, 513 = next-halo row
    qn2T = [qn2T_pool.tile([128, RO + 2], BF16, name=f"qn2T{j}") for j in range(8)]
    for i, (r0, p) in enumerate(CHUNKS):
        mv, rstd = layernorm_to(qint[i][:p, :], p)
        qn2_t = qn2_pool.tile([p, D], F32, tag="qn2")
        nc.vector.tensor_scalar(
            out=qn2_t[:p, :], in0=qint[i][:p, :], scalar1=mv[:p, 0:1],
            scalar2=rstd[:p, 0:1], op0=ALU.subtract, op1=ALU.mult,
        )
        for j in range(8):
            tp = pp_f.tile([128, 128], F32, tag="tpf")
            nc.tensor.transpose(tp[:128, :p], qn2_t[:p, j * 128:(j + 1) * 128], ident_f[:p, :p])
            evict_copy(qn2T[j][:, r0:r0 + p], tp[:128, :p])

    # halo rows: extract prev/next boundary rows via maskmat.T @ gathered,
    # then LN2 + transpose into qn2T cols 512/513
    pp_h = f_stack.enter_context(tc.tile_pool(name="pp_h", bufs=1, space="PSUM"))
    hg_sb = qn2_pool.tile([2 * NQ, D], F32R, name="hg_sb")
    nc.sync.dma_start(out=hg_sb[:], in_=hag[:, :])
    qih = qn2_pool.tile([2, D], F32, name="qih")
    for half in range(2):
        ph = pp_h.tile([2, 512], F32, tag="psh", name=f"ph{half}")
        nc.tensor.matmul(
            ph[:], maskmat[:], hg_sb[:, half * 512:(half + 1) * 512],
            start=True, stop=True,
        )
        nc.vector.tensor_copy(qih[:, half * 512:(half + 1) * 512], ph[:])
    mv, rstd = layernorm_to(qih[:2, :], 2)
    qn2h = qn2_pool.tile([2, D], F32, name="qn2h")
    nc.vector.tensor_scalar(
        out=qn2h[:2, :], in0=qih[:2, :], scalar1=mv[:2, 0:1],
        scalar2=rstd[:2, 0:1], op0=ALU.subtract, op1=ALU.mult,
    )
    for j in range(8):
        tp = pp_f.tile([128, 128], F32, tag="tpf")
        nc.tensor.transpose(tp[:128, :2], qn2h[:2, j * 128:(j + 1) * 128], ident_f[:2, :2])
        evict_copy(qn2T[j][:, RO:RO + 2], tp[:128, :2])
    f_stack.close()

    # Wup (fp8 DoubleRow, K=256 per matmul) + SwiGLU + depthwise conv, in
    # 512-col superchunks; Wdown's first column-half rides along, one inner
    # pair behind the conv. Scales: wup carries x32, wdown x64 (host side);
    # the 1/32 descale folds into the Silu input scale / U bias / conv taps,
    # the 1/64 into the output eviction.
    f2_stack = ExitStack()
    pp_d = f2_stack.enter_context(tc.tile_pool(name="pp_d", bufs=1, space="PSUM"))
    ffn_stack = ExitStack()
    pp_g = ffn_stack.enter_context(tc.tile_pool(name="pp_g", bufs=2, space="PSUM"))
    pp_u = ffn_stack.enter_context(tc.tile_pool(name="pp_u", bufs=2, space="PSUM"))
    wup_pool = ffn_stack.enter_context(tc.tile_pool(name="wup", bufs=12))
    wupu_pool = ffn_stack.enter_context(tc.tile_pool(name="wupu", bufs=12))
    fsm_pool = ffn_stack.enter_context(tc.tile_pool(name="fsm", bufs=3))
    bias_pool = ffn_stack.enter_context(tc.tile_pool(name="bias", bufs=6))
    wd0_pool = ffn_stack.enter_context(tc.tile_pool(name="wd0", bufs=4))

    psd0 = [pp_d.tile([128, 512], F32, name=f"psd0_{i}") for i in range(4)]
    # hfc: fp8 DoubleRow pair tiles; pair t holds inner blocks (2t, 2t+1)
    for t in range(16):
        hfc.append(hfc_pool.tile([128, 2, RO], F8, name=f"hfc{t}"))
    wd0_sb = {}

    def emit_wdown_pair(t):
        wd_t = wd0_sb.pop(t)
        for i in range(4):
            nc.tensor.matmul(
                psd0[i][:], hfc[t][:, :, i * 128:(i + 1) * 128],
                wd_t[:, :, :], start=(t == 0), stop=(t == 15), perf_mode=DR,
            )

    for sc in range(8):
        wupg_sb, wupu_sb = [], []
        for k in range(8):
            wg = wup_pool.tile([128, 512], BF16, tag="wupg")
            nc.sync.dma_start(
                out=wg[:], in_=p_wup[k * 128:(k + 1) * 128, sc * 512:(sc + 1) * 512]
            )
            wupg_sb.append(wg)
            wu = wupu_pool.tile([128, 512], BF16, tag="wupu")
            nc.scalar.dma_start(
                out=wu[:], in_=p_wup[k * 128:(k + 1) * 128, INNER + sc * 512:INNER + (sc + 1) * 512]
            )
            wupu_sb.append(wu)
        for c in range(4):
            cc = sc * 4 + c
            if cc % 2 == 0:
                tn = cc // 2
                wd_t = wd0_pool.tile([128, 2, 512], F8, tag="wd0")
                nc.sync.dma_start(out=wd_t[:, :, :], in_=p_wd08[tn * 128:(tn + 1) * 128, :])
                wd0_sb[tn] = wd_t
            bg = bias_pool.tile([128, 1], F32, tag="bg")
            nc.sync.dma_start(out=bg[:], in_=bass.AP(tensor=p_bgu, offset=cc * 128, ap=[[1, 128], [1, 1]]))
            bu = bias_pool.tile([128, 1], F32, tag="bu")
            nc.sync.dma_start(out=bu[:], in_=bass.AP(tensor=p_bgu, offset=INNER + cc * 128, ap=[[1, 128], [1, 1]]))
            cw = bias_pool.tile([128, 3], F32, tag="cw")
            nc.sync.dma_start(out=cw[:], in_=p_cw[cc * 128:(cc + 1) * 128, :])

            gact = fsm_pool.tile([128, RO + 2], F32, tag="gact")
            hf = fsm_pool.tile([128, RO + 2], F32, tag="hf")
            for h0, w in HALVES:
                psg = pp_g.tile([128, 258], F32, tag="psg")
                for k in range(8):
                    nc.tensor.matmul(
                        psg[:, :w], wupg_sb[k][:, c * 128:(c + 1) * 128],
                        qn2T[k][:, h0:h0 + w], start=(k == 0), stop=(k == 7),
                    )
                nc.scalar.activation(gact[:, h0:h0 + w], psg[:, :w], AF.Silu, bias=bg[:, 0:1])
                psu = pp_u.tile([128, 258], F32, tag="psu")
                for k in range(8):
                    nc.tensor.matmul(
                        psu[:, :w], wupu_sb[k][:, c * 128:(c + 1) * 128],
                        qn2T[k][:, h0:h0 + w], start=(k == 0), stop=(k == 7),
                    )
                nc.vector.scalar_tensor_tensor(
                    out=hf[:, h0:h0 + w], in0=psu[:, :w], scalar=bu[:, 0:1],
                    in1=gact[:, h0:h0 + w], op0=ALU.add, op1=ALU.mult,
                )
            if cc >= 2 and cc % 2 == 0:
                emit_wdown_pair(cc // 2 - 1)
            # mask halo cols at batch edges (conv zero-pad)
            nc.vector.tensor_scalar_mul(hf[:, RO:RO + 1], hf[:, RO:RO + 1], mask_p[:, 0:1])
            nc.vector.tensor_scalar_mul(hf[:, RO + 1:RO + 2], hf[:, RO + 1:RO + 2], mask_n[:, 0:1])
            # depthwise conv along rows: cols 0..511 owned, 512=prev, 513=next.
            # center tap on ACT (scale is per-partition), side taps DVE; the
            # final two taps write the fp8 DoubleRow pair tile directly.
            hfb = fsm_pool.tile([128, RO], BF16, tag="hfb")
            dst = hfc[cc // 2]
            jj = cc % 2
            nc.scalar.activation(hfb[:, 0:RO], hf[:, 0:RO], AF.Copy, scale=cw[:, 1:2])
            nc.vector.scalar_tensor_tensor(
                out=hfb[:, 1:RO], in0=hf[:, 0:RO - 1], scalar=cw[:, 0:1],
                in1=hfb[:, 1:RO], op0=ALU.mult, op1=ALU.add,
            )
            nc.vector.scalar_tensor_tensor(
                out=hfb[:, 0:1], in0=hf[:, RO:RO + 1], scalar=cw[:, 0:1],
                in1=hfb[:, 0:1], op0=ALU.mult, op1=ALU.add,
            )
            nc.vector.scalar_tensor_tensor(
                out=dst[:, jj, 0:RO - 1], in0=hf[:, 1:RO], scalar=cw[:, 2:3],
                in1=hfb[:, 0:RO - 1], op0=ALU.mult, op1=ALU.add,
            )
            nc.vector.scalar_tensor_tensor(
                out=dst[:, jj, RO - 1:RO], in0=hf[:, RO + 1:RO + 2], scalar=cw[:, 2:3],
                in1=hfb[:, RO - 1:RO], op0=ALU.mult, op1=ALU.add,
            )
    emit_wdown_pair(15)
    ffn_stack.close()

    # ---------------- Phase F4: Wdown second half + residual + output ----------
    out_pool = f2_stack.enter_context(tc.tile_pool(name="outp", bufs=4))
    wd1_pool = f2_stack.enter_context(tc.tile_pool(name="wd1", bufs=6))
    pp_d1 = f2_stack.enter_context(tc.tile_pool(name="pp_d1", bufs=1, space="PSUM"))
    psd1 = [pp_d1.tile([128, 512], F32, name=f"psd1_{i}") for i in range(4)]
    for t in range(16):
        wd_t = wd1_pool.tile([128, 2, 512], F8, tag="wd1")
        nc.sync.dma_start(out=wd_t[:, :, :], in_=p_wd18[t * 128:(t + 1) * 128, :])
        for i in range(4):
            nc.tensor.matmul(
                psd1[i][:], hfc[t][:, :, i * 128:(i + 1) * 128],
                wd_t[:, :, :], start=(t == 0), stop=(t == 15), perf_mode=DR,
            )
    for i in range(4):
        o_t = out_pool.tile([128, D], F32, tag="osb")
        nc.vector.scalar_tensor_tensor(
            out=o_t[:, 0:512], in0=psd0[i][:], scalar=WDINV,
            in1=qint[i][:, 0:512], op0=ALU.mult, op1=ALU.add,
        )
        nc.vector.scalar_tensor_tensor(
            out=o_t[:, 512:1024], in0=psd1[i][:], scalar=WDINV,
            in1=qint[i][:, 512:1024], op0=ALU.mult, op1=ALU.add,
        )
        out_q = (nc.sync, nc.scalar, nc.gpsimd, nc.sync)[i]
        out_q.dma_start(out=p_out[i * 128:(i + 1) * 128, :], in_=o_t[:])
    f2_stack.close()
    f34_stack.close()


def kernel(**inputs) -> np.ndarray:
    Q_in = np.ascontiguousarray(np.asarray(inputs["Q_in"], dtype=np.float32))
    X = np.ascontiguousarray(np.asarray(inputs["X"], dtype=np.float32))
    Wq = np.asarray(inputs["Wq"], dtype=np.float32)
    Wk = np.asarray(inputs["Wk"], dtype=np.float32)
    Wv = np.asarray(inputs["Wv"], dtype=np.float32)
    Wo = np.asarray(inputs["Wo"], dtype=np.float32)
    Wup = np.asarray(inputs["Wup"], dtype=np.float32)
    conv_w = np.asarray(inputs["conv_w"], dtype=np.float32)
    Wdown = np.asarray(inputs["Wdown"], dtype=np.float32)
    g1 = np.asarray(inputs["g1"], dtype=np.float32)
    b1 = np.asarray(inputs["b1"], dtype=np.float32)
    g2 = np.asarray(inputs["g2"], dtype=np.float32)
    b2 = np.asarray(inputs["b2"], dtype=np.float32)
    dt = float(np.asarray(inputs["dt"], dtype=np.float32))

    # softplus(dt) on host; baked into the NEFF as an immediate
    dt_safe = float(np.log1p(np.exp(dt)))

    # fold g2/b2 into Wup (LN2's affine commutes into the up-projection)
    wup_f = g2[:, None] * Wup
    bias_gu = np.ascontiguousarray(b2 @ Wup)
    cw3 = np.ascontiguousarray(conv_w[:, 0, :])

    key = round(dt_safe, 9)
    if key not in _cache:
        _cache[key] = _build(dt_safe)
    nc = _cache[key]

    bf = ml_dtypes.bfloat16
    f8 = ml_dtypes.float8_e4m3
    wq_b = np.ascontiguousarray(Wq.astype(bf))
    wk_b = np.ascontiguousarray(Wk.astype(bf))
    wv_b = np.ascontiguousarray(Wv.astype(bf))
    wo_b = np.ascontiguousarray(Wo.astype(bf))
    wup_b = np.ascontiguousarray(wup_f.astype(bf))

    # Wdown DoubleRow pair packing: tile-row t*128+r holds inner element
    # i = 128*(2t+j)+r in slot j; cols are (j, c).
    def pack_down(w):  # [INNER, 512] -> [INNER//2, D] fp8 pair layout
        t = w.reshape(16, 2, 128, 512)
        return np.ascontiguousarray(
            t.transpose(0, 2, 1, 3).reshape(INNER // 2, D).astype(f8))

    wd08 = pack_down(WDS * Wdown[:, 0:512])
    wd18 = pack_down(WDS * Wdown[:, 512:1024])

    in_maps = []
    for core in range(8):
        b, q = divmod(core, 4)
        qin = np.ascontiguousarray(Q_in[b, q * RO:(q + 1) * RO])
        xb1 = np.ascontiguousarray(X[b, q * RO:(q + 1) * RO] + b1[None, :])
        masks = np.array(
            [1.0 if q > 0 else 0.0, 1.0 if q < NQ - 1 else 0.0], dtype=np.float32
        )
        # maskmat.T @ gathered_boundaries = [prev-halo row; next-halo row]
        mm = np.zeros((2 * NQ, 2), dtype=np.float32)
        if q > 0:
            mm[2 * (q - 1) + 1, 0] = 1.0
        if q < NQ - 1:
            mm[2 * (q + 1), 1] = 1.0
        in_maps.append({
            "q_in": qin, "x_b1": xb1, "wq": wq_b, "wk": wk_b, "wv": wv_b,
            "wo": wo_b, "wup": wup_b, "bias_gu": bias_gu,
            "wd08": wd08, "wd18": wd18, "cw3": cw3,
            "g1": np.ascontiguousarray(g1), "masks": masks, "maskmat": mm,
        })

    global _last_in_maps
    _last_in_maps = in_maps
    res = run_bass_kernel_spmd(nc, in_maps, core_ids=list(range(8)))

    out = np.empty((B, N, D), dtype=np.float32)
    for core in range(8):
        b, q = divmod(core, 4)
        out[b, q * RO:(q + 1) * RO] = res.results[core]["out"]
    return out



# revision 59
# speedup vs baseline: 1.3209x; 1.2224x over previous
"""Distributed Trainium2 Bass kernel for nn_AMK_Block (kernelized-attention + ConvSwiGLU).

Sharding: sequence-parallel. Each of the 8 cores owns (batch b, query-row block q):
core = b*4 + q, rows q*512..q*512+511 of batch b, ALL heads. Each core computes
Q/K/V projections for its rows, AllGathers PhiK^T and V(+ones) across the 4 cores
of its batch group (fp8), then computes its 512 rows of attention, Wo, LN2 and
the full FFN locally. The depthwise-conv halo rows of Q_interact come from a tiny
third AllGather of boundary rows, extracted rank-agnostically with a mask-matrix
matmul. Weight matmuls run in bf16 (fp32 PSUM accumulation); the attention
kernel-matrix matmuls run in fp8 (PhiQ/PhiK/V/W^2 evicted as e4m3, Attr uses
DoubleRow packed k-pairs); norm/statistics in fp32. Wdown's first column-half is
interleaved into the FFN chunk stream so only half remains as a tail.
"""

import sys

sys.path.insert(0, "/opt/trn_rl_repo")

from contextlib import ExitStack

import ml_dtypes
import numpy as np

import concourse.bass as bass
import concourse.tile as tile
from concourse import bacc, mybir
from concourse.bass_utils import run_bass_kernel_spmd
from concourse.masks import make_identity

F32 = mybir.dt.float32
F32R = mybir.dt.float32r
BF16 = mybir.dt.bfloat16
F8 = mybir.dt.float8e4
ALU = mybir.AluOpType
AF = mybir.ActivationFunctionType
DR = mybir.MatmulPerfMode.DoubleRow

B, N, D = 2, 2048, 1024
H, DH = 16, 64
INNER = 4096
LN_EPS = 1e-5
WSC = 1.0 / 32.0          # W^2 is evicted as (W/32)^2 = W^2/1024 in fp8
WSC2 = WSC * WSC
UPS = 32.0                # Wup fp8 host scale (values ~N(0,1/32) -> ~N(0,1))
UPSC = 1.0 / UPS
WDS = 64.0                # Wdown fp8 host scale
WDINV = 1.0 / WDS

RO = 512          # owned rows per core
NQ = 4            # cores per batch group
GROUPS = [[0, 1, 2, 3], [4, 5, 6, 7]]
CHUNKS = [(0, 128), (128, 128), (256, 128), (384, 128)]
HALVES = [(0, 258), (258, 256)]  # even halves of 514; halo cols 512/513 in 2nd

_cache: dict[float, object] = {}
_last_in_maps: list | None = None


def _build(dt_safe: float):
    nc = bacc.Bacc("TRN2", target_bir_lowering=False, debug=False, num_devices=8)

    # ---------------- DRAM parameters (per-core shapes) ----------------
    p_qin = nc.declare_dram_parameter("q_in", [RO, D], F32, isOutput=False)
    p_xb1 = nc.declare_dram_parameter("x_b1", [RO, D], F32, isOutput=False)
    p_wq = nc.declare_dram_parameter("wq", [D, D], BF16, isOutput=False)
    p_wk = nc.declare_dram_parameter("wk", [D, D], BF16, isOutput=False)
    p_wv = nc.declare_dram_parameter("wv", [D, D], BF16, isOutput=False)
    p_wo = nc.declare_dram_parameter("wo", [D, D], BF16, isOutput=False)
    p_wup = nc.declare_dram_parameter("wup", [D, 2 * INNER], BF16, isOutput=False)
    p_bgu = nc.declare_dram_parameter("bias_gu", [2 * INNER], F32, isOutput=False)
    p_wd08 = nc.declare_dram_parameter("wd08", [INNER // 2, D], F8, isOutput=False)
    p_wd18 = nc.declare_dram_parameter("wd18", [INNER // 2, D], F8, isOutput=False)
    p_cw = nc.declare_dram_parameter("cw3", [INNER, 3], F32, isOutput=False)
    p_g1 = nc.declare_dram_parameter("g1", [D], F32, isOutput=False)
    p_mask = nc.declare_dram_parameter("masks", [2], F32, isOutput=False)
    p_mm = nc.declare_dram_parameter("maskmat", [2 * NQ, 2], F32R, isOutput=False)
    p_out = nc.declare_dram_parameter("out", [RO, D], F32, isOutput=True)

    with tile.TileContext(nc) as tc:
        build_ctx = ExitStack()
        with build_ctx:
            _emit(nc, tc, build_ctx, dt_safe, p_qin, p_xb1, p_wq, p_wk, p_wv,
                  p_wo, p_wup, p_bgu, p_wd08, p_wd18, p_cw, p_g1,
                  p_mask, p_mm, p_out)
    nc.finalize()
    return nc


def _emit(nc, tc, bctx, dt_safe, p_qin, p_xb1, p_wq, p_wk, p_wv, p_wo, p_wup,
          p_bgu, p_wd08, p_wd18, p_cw, p_g1, p_mask, p_mm, p_out):
    # ---------------- constant tiles ----------------
    consts = bctx.enter_context(tc.tile_pool(name="consts", bufs=1))
    g1b = consts.tile([128, D], F32, name="g1b")
    nc.sync.dma_start(
        out=g1b[:],
        in_=bass.AP(tensor=p_g1, offset=0, ap=[[0, 128], [1, D]]),
    )
    ident_f = consts.tile([128, 128], F32, name="ident_f")
    make_identity(nc, ident_f[:])
    ident_r = consts.tile([128, 128], F32R, name="ident_r")
    nc.gpsimd.dma_start(out=ident_r[:], in_=ident_f[:])
    mask_p = consts.tile([128, 1], F32, name="mask_p")
    nc.sync.dma_start(out=mask_p[:], in_=bass.AP(tensor=p_mask, offset=0, ap=[[0, 128], [1, 1]]))
    mask_n = consts.tile([128, 1], F32, name="mask_n")
    nc.sync.dma_start(out=mask_n[:], in_=bass.AP(tensor=p_mask, offset=1, ap=[[0, 128], [1, 1]]))
    ones_col = consts.tile([128, 1], F8, name="ones_col")
    nc.vector.memset(ones_col[:], 1.0)
    eps_t = consts.tile([128, 1], F32, name="eps_t")
    nc.vector.memset(eps_t[:], LN_EPS)
    maskmat = consts.tile([2 * NQ, 2], F32R, name="maskmat")
    nc.sync.dma_start(out=maskmat[:], in_=p_mm[:, :])

    # DRAM scratch for the collectives (fp8 payloads)
    dram = bctx.enter_context(tc.tile_pool(name="dram", bufs=1, space="DRAM"))
    kag_in = dram.tile([H * DH, RO], F8, name="kag_in")        # PhiK^T local slice
    kag1 = dram.tile([NQ * 512, RO], F8, name="kag1")          # gathered heads 0-7
    kag2 = dram.tile([NQ * 512, RO], F8, name="kag2")          # gathered heads 8-15
    vag_in = dram.tile([RO, H * 66], F8, name="vag_in")        # V(+ones) local rows
    vag1 = dram.tile([NQ * 256, H * 66], F8, name="vag1")      # gathered rows 0-255
    vag2 = dram.tile([NQ * 256, H * 66], F8, name="vag2")      # gathered rows 256-511
    hag_in = dram.tile([2, D], F32R, name="hag_in")            # my boundary Qint rows
    hag = dram.tile([2 * NQ, D], F32R, name="hag")             # gathered boundaries

    ev_state = [0]

    def evict_copy(dst_ap, src_ap):
        ev_state[0] += 1
        if ev_state[0] % 2 == 0:
            nc.vector.tensor_copy(dst_ap, src_ap)
        else:
            nc.scalar.activation(dst_ap, src_ap, AF.Copy)

    ln_pool = bctx.enter_context(tc.tile_pool(name="ln", bufs=3))

    def layernorm_to(x_ap, p):
        """Returns (mv, rstd) tiles: mean in mv[:,0:1], rstd [p,1], for x_ap [p, D]."""
        st = ln_pool.tile([128, 2, 6], F32, tag="bn_st")
        xr = x_ap.rearrange("p (s f) -> p s f", s=2)
        for s in range(2):
            nc.vector.bn_stats(st[:p, s, :], xr[:, s, :])
        mv = ln_pool.tile([128, 2], F32, tag="bn_mv")
        nc.vector.bn_aggr(mv[:p], st[:p])
        rstd = ln_pool.tile([128, 1], F32, tag="bn_rstd")
        nc.scalar.activation(rstd[:p], mv[:p, 1:2], AF.Sqrt, bias=eps_t[:p, 0:1])
        nc.vector.reciprocal(rstd[:p], rstd[:p])
        return mv, rstd

    # ---- lifetime stacks (must nest LIFO): f34 > av2 > av > phase stacks ----
    f34_stack = ExitStack()   # hfc + qint: from Wo until the end
    av2_stack = ExitStack()   # mTc: until end of Wo
    av_stack = ExitStack()    # vT, phiQ: until end of head loop
    hfc_pool = f34_stack.enter_context(tc.tile_pool(name="hfc", bufs=1))
    qint_pool = f34_stack.enter_context(tc.tile_pool(name="qint", bufs=1))
    mTc_pool = av2_stack.enter_context(tc.tile_pool(name="mTc", bufs=1))
    vT_pool = av_stack.enter_context(tc.tile_pool(name="vT", bufs=1))
    phiQ_pool = av_stack.enter_context(tc.tile_pool(name="phiQ", bufs=1))
    mTc = [mTc_pool.tile([128, RO], BF16, name=f"mTc{j}") for j in range(8)]
    vT = [vT_pool.tile([128, RO], F32R, name=f"vT{j}") for j in range(8)]
    phiQT = [phiQ_pool.tile([128, RO], F8, name=f"phiQT{j}") for j in range(8)]

    # ---------------- Phase P: LN1 + Hc + transposes ----------------
    p_stack = ExitStack()
    hcT_pool = p_stack.enter_context(tc.tile_pool(name="hcT", bufs=1))
    hcT = [hcT_pool.tile([128, RO], BF16, name=f"hcT{j}") for j in range(8)]
    io_pool = p_stack.enter_context(tc.tile_pool(name="io", bufs=4))
    hc_pool = p_stack.enter_context(tc.tile_pool(name="hc", bufs=2))
    # the PSUM phases of P (hc transposes / projections / V transposes) are
    # disjoint in time, so each gets its own short-lived 4-6 buffer pool
    t_stack = ExitStack()
    pp_t = t_stack.enter_context(tc.tile_pool(name="pp_t", bufs=4, space="PSUM"))

    # stream all qin/xb1 chunks up front on two queues
    qin_ts, xb1_ts = [], []
    for i, (r0, p) in enumerate(CHUNKS):
        qin_t = io_pool.tile([p, D], F32, tag="qin")
        nc.sync.dma_start(out=qin_t[:], in_=p_qin[r0:r0 + p, :])
        qin_ts.append(qin_t)
        xb1_t = io_pool.tile([p, D], F32, tag="xb1")
        nc.scalar.dma_start(out=xb1_t[:], in_=p_xb1[r0:r0 + p, :])
        xb1_ts.append(xb1_t)

    for i, (r0, p) in enumerate(CHUNKS):
        qin_t, xb1_t = qin_ts[i], xb1_ts[i]
        mv, rstd = layernorm_to(qin_t[:p, :], p)
        hc_t = hc_pool.tile([p, D], F32, tag="hc")
        nc.vector.tensor_scalar(
            out=hc_t[:p, :], in0=qin_t[:p, :], scalar1=mv[:p, 0:1],
            scalar2=rstd[:p, 0:1], op0=ALU.subtract, op1=ALU.mult,
        )
        nc.vector.tensor_mul(hc_t[:p, :], hc_t[:p, :], g1b[:p, :])
        nc.vector.tensor_add(hc_t[:p, :], hc_t[:p, :], xb1_t[:p, :])

        # transpose this row-chunk into the 8 hcT column tiles
        for j in range(8):
            tp = pp_t.tile([128, 128], F32, tag="tp")
            nc.tensor.transpose(tp[:128, :p], hc_t[:p, j * 128:(j + 1) * 128], ident_f[:p, :p])
            evict_copy(hcT[j][:, r0:r0 + p], tp[:128, :p])
    t_stack.close()

    # ---------------- Phase P: projections ----------------
    # Order: K -> K-AllGather (smallest latency to first collective), then Q
    # (needed with K for the W matmuls), then V -> V-AllGather. The rings
    # serialize on the collective lane, so K's goes first.
    wstream = p_stack.enter_context(tc.tile_pool(name="wstream", bufs=12))
    pr_stack = ExitStack()
    pp_a = pr_stack.enter_context(tc.tile_pool(name="pp_a", bufs=6, space="PSUM"))
    elu_pool = p_stack.enter_context(tc.tile_pool(name="elu", bufs=3))

    def elu1_evict(dst_ap, src_psum_ap, p, w):
        """dst = elu(src)+1 = relu(src) + exp(min(src,0)) (fp8 out)"""
        tmin = elu_pool.tile([128, 512], F32, tag="tmin")
        nc.vector.tensor_scalar_min(tmin[:p, :w], src_psum_ap, 0.0)
        texp = elu_pool.tile([128, 512], F32, tag="texp")
        nc.scalar.activation(texp[:p, :w], tmin[:p, :w], AF.Exp)
        nc.vector.scalar_tensor_tensor(
            out=dst_ap, in0=src_psum_ap, scalar=0.0, in1=texp[:p, :w],
            op0=ALU.max, op1=ALU.add,
        )

    # K^T -> PhiK^T (fp8) -> kag_in; two chunked AllGathers (heads 0-7, 8-15)
    wk_sb = []
    for k in range(8):
        w_t = wstream.tile([128, D], BF16, tag="wproj")
        nc.sync.dma_start(out=w_t[:], in_=p_wk[k * 128:(k + 1) * 128, :])
        wk_sb.append(w_t)
    phiK_pool = p_stack.enter_context(tc.tile_pool(name="phiK", bufs=4))
    for j in range(8):
        ps = pp_a.tile([128, 512], F32, tag="proj")
        for k in range(8):
            nc.tensor.matmul(
                ps[:], wk_sb[k][:, j * 128:(j + 1) * 128],
                hcT[k][:, 0:RO], start=(k == 0), stop=(k == 7),
            )
        phiK_t = phiK_pool.tile([128, RO], F8, tag="phiK")
        elu1_evict(phiK_t[:, :], ps[:], 128, RO)
        nc.sync.dma_start(out=kag_in[j * 128:(j + 1) * 128, :], in_=phiK_t[:])
        if j == 3:
            nc.gpsimd.collective_compute(
                "AllGather", ALU.bypass, replica_groups=GROUPS,
                ins=[kag_in[0:512, :].opt()], outs=[kag1[:].opt()],
            )
    nc.gpsimd.collective_compute(
        "AllGather", ALU.bypass, replica_groups=GROUPS,
        ins=[kag_in[512:1024, :].opt()], outs=[kag2[:].opt()],
    )

    # Q^T -> PhiQ^T (fp8, kept in SBUF)
    wq_sb = []
    for k in range(8):
        w_t = wstream.tile([128, D], BF16, tag="wproj")
        nc.scalar.dma_start(out=w_t[:], in_=p_wq[k * 128:(k + 1) * 128, :])
        wq_sb.append(w_t)
    for j in range(8):
        ps = pp_a.tile([128, 512], F32, tag="proj")
        for k in range(8):
            nc.tensor.matmul(
                ps[:], wq_sb[k][:, j * 128:(j + 1) * 128],
                hcT[k][:, 0:RO], start=(k == 0), stop=(k == 7),
            )
        elu1_evict(phiQT[j][:, :], ps[:], 128, RO)

    # V^T, then transpose back to row-major (+ones cols, fp8) and stage for AGs
    wv_sb = []
    for k in range(8):
        w_t = wstream.tile([128, D], BF16, tag="wproj")
        nc.sync.dma_start(out=w_t[:], in_=p_wv[k * 128:(k + 1) * 128, :])
        wv_sb.append(w_t)
    for j in range(8):
        ps = pp_a.tile([128, 512], F32, tag="proj")
        for k in range(8):
            nc.tensor.matmul(
                ps[:], wv_sb[k][:, j * 128:(j + 1) * 128],
                hcT[k][:, 0:RO], start=(k == 0), stop=(k == 7),
            )
        evict_copy(vT[j][:, :], ps[:])
    pr_stack.close()

    tv_stack = ExitStack()
    pp_tv = tv_stack.enter_context(tc.tile_pool(name="pp_tv", bufs=4, space="PSUM"))
    vs_pool = p_stack.enter_context(tc.tile_pool(name="vs", bufs=3))
    for i in range(4):
        r0 = i * 128
        vstage = vs_pool.tile([128, H * 66], F8, tag="vstage")
        for j in range(8):
            tpv = pp_tv.tile([128, 128], F32R, tag="tpv")
            nc.tensor.transpose(tpv[:], vT[j][:, r0:r0 + 128], ident_r[:])
            h0, h1 = 2 * j, 2 * j + 1
            evict_copy(vstage[:, h0 * 66:h0 * 66 + 64], tpv[:, 0:64])
            evict_copy(vstage[:, h1 * 66:h1 * 66 + 64], tpv[:, 64:128])
        # per-head ones column (64) + zero pad column (65), strided memsets
        vsr = vstage[:].rearrange("p (h d) -> p h d", h=H)
        nc.vector.memset(vsr[:, :, 64:65], 1.0)
        nc.vector.memset(vsr[:, :, 65:66], 0.0)
        nc.gpsimd.dma_start(out=vag_in[r0:r0 + 128, :], in_=vstage[:])
        if i == 1:
            nc.gpsimd.collective_compute(
                "AllGather", ALU.bypass, replica_groups=GROUPS,
                ins=[vag_in[0:256, :].opt()], outs=[vag1[:].opt()],
            )
    nc.gpsimd.collective_compute(
        "AllGather", ALU.bypass, replica_groups=GROUPS,
        ins=[vag_in[256:512, :].opt()], outs=[vag2[:].opt()],
    )
    tv_stack.close()

    p_stack.close()

    # ---------------- Phase A: attention ----------------
    # Per head-group g (4 heads): W(g) = 64 fp8 matmuls (K=64) evicted as
    # (W/32)^2 fp8 into DoubleRow pair tiles; Attr(g) = per head 8 fp8-DR
    # matmuls over (m-block pair, key) tiles. Emission order W0 W1 A0 W2 A1
    # W3 A2 A3 keeps the PE busy while the V AllGathers land.
    # Wo weights: pool created first (released after attention pools), loads
    # issued now so the Wo phase starts instantly
    wo_stack = ExitStack()
    wo_pool = wo_stack.enter_context(tc.tile_pool(name="wo", bufs=8))
    wo_sb = []
    for k in range(8):
        w_t = wo_pool.tile([128, D], BF16, tag="wo")
        nc.scalar.dma_start(out=w_t[:], in_=p_wo[k * 128:(k + 1) * 128, :])
        wo_sb.append(w_t)

    a_stack = ExitStack()
    kq_pool = a_stack.enter_context(tc.tile_pool(name="kq", bufs=3))
    vhd_pool = a_stack.enter_context(tc.tile_pool(name="vhd", bufs=16))
    # dual-fp8 LDWEIGHTS needs stationary width % 32 == 0: vhd is 96 wide
    # (V 0-63, ones 64, pad 65-95). DMA writes cols 0-65; zero the pad cols
    # once per pool buffer (round-robin reuse keeps them zero).
    for _ in range(16):
        vz = vhd_pool.tile([128, 8, 96], F8, tag="vhd")
        nc.vector.memset(vz[:, :, 66:96], 0.0)
    wt_pool = a_stack.enter_context(tc.tile_pool(name="wt", bufs=64))
    asm_pool = a_stack.enter_context(tc.tile_pool(name="asm", bufs=3))
    pp_w = a_stack.enter_context(tc.tile_pool(name="pp_w", bufs=4, space="PSUM"))
    pp_at = a_stack.enter_context(tc.tile_pool(name="pp_at", bufs=4, space="PSUM"))

    sq_state = [0]

    def square_evict(dst_ap, src_psum_ap):
        """dst = src^2 fp8; src is already W/32 (W > 0, relu is a no-op).
        Rotated 5:2 across ACT/DVE: ACT streams ~1 col/ns single-pass; the
        DVE two-pass path costs ~2x that, so it only soaks the overflow."""
        sq_state[0] = (sq_state[0] + 1) % 7
        if sq_state[0] < 5:
            nc.scalar.activation(dst_ap, src_psum_ap, AF.Square, scale=WSC)
        else:
            tr = asm_pool.tile([128, 512], BF16, tag="r2tmpv")
            nc.vector.tensor_scalar_mul(tr[:, :], src_psum_ap, WSC)
            nc.vector.tensor_mul(dst_ap, tr[:, :], tr[:, :])

    def emit_w(hg):
        """W^T for 4 heads of group hg -> wtp fp8 DoubleRow pair tiles.
        The two heads sharing a kq/phiQT tile (PE row halves 0-63 / 64-127)
        are interleaved: consecutive matmuls hit disjoint row groups, so
        LDWEIGHTS overlaps the in-flight matmul."""
        kag_t = kag1 if hg < 2 else kag2
        kq_sb = {}
        for j2 in (2 * hg, 2 * hg + 1):
            hrow = (j2 % 4) * 128  # row offset of head-pair j2 within kag_t
            kt = kq_pool.tile([128, NQ, RO], F8, tag="kq")
            ksrc = kag_t[:, :]
            nc.sync.dma_start(
                out=kt[:, :, :],
                in_=bass.AP(tensor=ksrc.tensor, offset=ksrc.offset + hrow * RO,
                            ap=[[RO, 128], [512 * RO, NQ], [1, RO]]),
            )
            kq_sb[j2] = kt
        wtp = {}
        for hh in range(4):
            h = hg * 4 + hh
            wtp[h] = [wt_pool.tile([128, 2, RO], F8, tag="wt", name=f"wt{h}_{t}")
                      for t in range(8)]
        for j2 in (2 * hg, 2 * hg + 1):
            hA, hB = 2 * j2, 2 * j2 + 1
            for m in range(16):
                qq, lc = m // 4, m % 4
                for off, h in ((0, hA), (64, hB)):
                    psw = pp_w.tile([128, 512], F32, tag="psw")
                    nc.tensor.matmul(
                        psw[:], kq_sb[j2][off:off + 64, qq, lc * 128:(lc + 1) * 128],
                        phiQT[j2][off:off + 64, :], start=True, stop=True,
                    )
                    square_evict(wtp[h][m // 2][:, m % 2, :], psw)
        return wtp

    # pair order follows the chunked V gathers: vag1 pairs (lc 0,1) first
    T_ORDER = [qq * 2 for qq in range(NQ)] + [qq * 2 + 1 for qq in range(NQ)]

    def emit_attr(hg, wtp):
        pats = []
        for hh in range(4):
            h = hg * 4 + hh
            pat = pp_at.tile([96, 512], F32, tag="pat", name=f"pat{h}")
            vh = {}
            for half in range(2):
                vsrc = vag1 if half == 0 else vag2
                vt = vhd_pool.tile([128, 8, 96], F8, tag="vhd")
                vap = vsrc[:, :]
                W16 = H * 66
                nc.sync.dma_start(
                    out=vt[:, :, 0:66],
                    in_=bass.AP(tensor=vap.tensor, offset=vap.offset + h * 66,
                                ap=[[W16, 128], [128 * W16, 8], [1, 66]]))
                vh[half] = vt
            for ti, t in enumerate(T_ORDER):
                qq, half = t // 2, t % 2
                nc.tensor.matmul(
                    pat[:], vh[half][:, 2 * qq:2 * qq + 2, :], wtp[h][t][:, :, :],
                    start=(ti == 0), stop=(ti == 7), perf_mode=DR,
                )
            pats.append(pat)
        for hh in range(4):
            h = hg * 4 + hh
            j2, off = h // 2, (h % 2) * 64
            nrm = asm_pool.tile([1, RO], F32, tag="nrm")
            nc.vector.tensor_scalar_add(nrm[0:1, :], pats[hh][64:65, :], WSC2)
            nc.vector.reciprocal_approx_fast(out=nrm[:], in_=nrm[:])
            rcb = asm_pool.tile([64, RO], F32, tag="rcb")
            nc.gpsimd.partition_broadcast(rcb[:], nrm[:])
            tm = asm_pool.tile([128, RO], F32, tag="tm")
            nc.vector.tensor_mul(tm[off:off + 64, :], pats[hh][0:64, :], rcb[:, :])
            nc.vector.tensor_sub(
                mTc[j2][off:off + 64, :], tm[off:off + 64, :],
                vT[j2][off:off + 64, :],
            )

    wtp_q = [emit_w(0), emit_w(1)]
    for hg in range(4):
        emit_attr(hg, wtp_q[hg])
        if hg + 2 < 4:
            wtp_q.append(emit_w(hg + 2))
    a_stack.close()

    # ---------------- Phase A5: Wo + Q_interact ----------------
    # Chunk order 0,3,1,2 so the conv-halo boundary rows exist after two
    # chunks and their AllGather overlaps the rest of Wo + LN2.
    a5_stack = ExitStack()
    qi_pool = a5_stack.enter_context(tc.tile_pool(name="qi", bufs=3))
    pp_o = a5_stack.enter_context(tc.tile_pool(name="pp_o", bufs=4, space="PSUM"))
    qint = [None] * 4
    for oi, i in enumerate((0, 3, 1, 2)):
        r0, p = CHUNKS[i]
        qin_t = qi_pool.tile([p, D], F32, tag="qin2")
        nc.sync.dma_start(out=qin_t[:], in_=p_qin[r0:r0 + p, :])
        qi = qint_pool.tile([p, D], F32, name=f"qint{i}")
        for half in range(2):
            pso = pp_o.tile([128, 512], F32, tag="pso")
            for k in range(8):
                nc.tensor.matmul(
                    pso[:p, :], mTc[k][:, r0:r0 + p],
                    wo_sb[k][:, half * 512:(half + 1) * 512],
                    start=(k == 0), stop=(k == 7),
                )
            nc.vector.scalar_tensor_tensor(
                out=qi[:p, half * 512:(half + 1) * 512], in0=pso[:p, :],
                scalar=dt_safe, in1=qin_t[:p, half * 512:(half + 1) * 512],
                op0=ALU.mult, op1=ALU.add,
            )
        qint[i] = qi
        if oi == 1:
            # boundary rows ready: stage + AllGather (conv halo exchange)
            nc.gpsimd.dma_start(out=hag_in[0:1, :], in_=qint[0][0:1, :])
            nc.gpsimd.dma_start(out=hag_in[1:2, :], in_=qint[3][127:128, :])
            nc.gpsimd.collective_compute(
                "AllGather", ALU.bypass, replica_groups=GROUPS,
                ins=[hag_in[:].opt()], outs=[hag[:].opt()],
            )
    a5_stack.close()
    wo_stack.close()
    av_stack.close()   # frees vT, phiQ
    av2_stack.close()  # frees mTc

    # ---------------- Phase F: LN2 + transpose + FFN ----------------
    hfc = []

    qn2T_pool = f34_stack.enter_context(tc.tile_pool(name="qn2T", bufs=1))
    f_stack = ExitStack()
    qn2_pool = f_stack.enter_context(tc.tile_pool(name="qn2", bufs=2))
    pp_f = f_stack.enter_context(tc.tile_pool(name="pp_f", bufs=4, space="PSUM"))
    # qn2T cols: 0..511 owned rows, 512 = prev-halo row, 513 = next-halo row
    qn2T = [qn2T_pool.tile([128, RO + 2], BF16, name=f"qn2T{j}") for j in range(8)]
    for i, (r0, p) in enumerate(CHUNKS):
        mv, rstd = layernorm_to(qint[i][:p, :], p)
        qn2_t = qn2_pool.tile([p, D], F32, tag="qn2")
        nc.vector.tensor_scalar(
            out=qn2_t[:p, :], in0=qint[i][:p, :], scalar1=mv[:p, 0:1],
            scalar2=rstd[:p, 0:1], op0=ALU.subtract, op1=ALU.mult,
        )
        for j in range(8):
            tp = pp_f.tile([128, 128], F32, tag="tpf")
            nc.tensor.transpose(tp[:128, :p], qn2_t[:p, j * 128:(j + 1) * 128], ident_f[:p, :p])
            evict_copy(qn2T[j][:, r0:r0 + p], tp[:128, :p])

    # halo rows: extract prev/next boundary rows via maskmat.T @ gathered,
    # then LN2 + transpose into qn2T cols 512/513
    pp_h = f_stack.enter_context(tc.tile_pool(name="pp_h", bufs=1, space="PSUM"))
    hg_sb = qn2_pool.tile([2 * NQ, D], F32R, name="hg_sb")
    nc.sync.dma_start(out=hg_sb[:], in_=hag[:, :])
    qih = qn2_pool.tile([2, D], F32, name="qih")
    for half in range(2):
        ph = pp_h.tile([2, 512], F32, tag="psh", name=f"ph{half}")
        nc.tensor.matmul(
            ph[:], maskmat[:], hg_sb[:, half * 512:(half + 1) * 512],
            start=True, stop=True,
        )
        nc.vector.tensor_copy(qih[:, half * 512:(half + 1) * 512], ph[:])
    mv, rstd = layernorm_to(qih[:2, :], 2)
    qn2h = qn2_pool.tile([2, D], F32, name="qn2h")
    nc.vector.tensor_scalar(
        out=qn2h[:2, :], in0=qih[:2, :], scalar1=mv[:2, 0:1],
        scalar2=rstd[:2, 0:1], op0=ALU.subtract, op1=ALU.mult,
    )
    for j in range(8):
        tp = pp_f.tile([128, 128], F32, tag="tpf")
        nc.tensor.transpose(tp[:128, :2], qn2h[:2, j * 128:(j + 1) * 128], ident_f[:2, :2])
        evict_copy(qn2T[j][:, RO:RO + 2], tp[:128, :2])
    f_stack.close()

    # Wup (fp8 DoubleRow, K=256 per matmul) + SwiGLU + depthwise conv, in
    # 512-col superchunks; Wdown's first column-half rides along, one inner
    # pair behind the conv. Scales: wup carries x32, wdown x64 (host side);
    # the 1/32 descale folds into the Silu input scale / U bias / conv taps,
    # the 1/64 into the output eviction.
    f2_stack = ExitStack()
    pp_d = f2_stack.enter_context(tc.tile_pool(name="pp_d", bufs=1, space="PSUM"))
    ffn_stack = ExitStack()
    pp_g = ffn_stack.enter_context(tc.tile_pool(name="pp_g", bufs=2, space="PSUM"))
    pp_u = ffn_stack.enter_context(tc.tile_pool(name="pp_u", bufs=2, space="PSUM"))
    wup_pool = ffn_stack.enter_context(tc.tile_pool(name="wup", bufs=12))
    wupu_pool = ffn_stack.enter_context(tc.tile_pool(name="wupu", bufs=12))
    fsm_pool = ffn_stack.enter_context(tc.tile_pool(name="fsm", bufs=3))
    bias_pool = ffn_stack.enter_context(tc.tile_pool(name="bias", bufs=6))
    wd0_pool = ffn_stack.enter_context(tc.tile_pool(name="wd0", bufs=4))

    psd0 = [pp_d.tile([128, 512], F32, name=f"psd0_{i}") for i in range(4)]
    # hfc: fp8 DoubleRow pair tiles; pair t holds inner blocks (2t, 2t+1)
    for t in range(16):
        hfc.append(hfc_pool.tile([128, 2, RO], F8, name=f"hfc{t}"))
    wd0_sb = {}

    def emit_wdown_pair(t):
        wd_t = wd0_sb.pop(t)
        for i in range(4):
            nc.tensor.matmul(
                psd0[i][:], hfc[t][:, :, i * 128:(i + 1) * 128],
                wd_t[:, :, :], start=(t == 0), stop=(t == 15), perf_mode=DR,
            )

    for sc in range(8):
        wupg_sb, wupu_sb = [], []
        for k in range(8):
            wg = wup_pool.tile([128, 512], BF16, tag="wupg")
            nc.sync.dma_start(
                out=wg[:], in_=p_wup[k * 128:(k + 1) * 128, sc * 512:(sc + 1) * 512]
            )
            wupg_sb.append(wg)
            wu = wupu_pool.tile([128, 512], BF16, tag="wupu")
            nc.scalar.dma_start(
                out=wu[:], in_=p_wup[k * 128:(k + 1) * 128, INNER + sc * 512:INNER + (sc + 1) * 512]
            )
            wupu_sb.append(wu)
        for c in range(4):
            cc = sc * 4 + c
            if cc % 2 == 0:
                tn = cc // 2
                wd_t = wd0_pool.tile([128, 2, 512], F8, tag="wd0")
                nc.sync.dma_start(out=wd_t[:, :, :], in_=p_wd08[tn * 128:(tn + 1) * 128, :])
                wd0_sb[tn] = wd_t
            bg = bias_pool.tile([128, 1], F32, tag="bg")
            nc.sync.dma_start(out=bg[:], in_=bass.AP(tensor=p_bgu, offset=cc * 128, ap=[[1, 128], [1, 1]]))
            bu = bias_pool.tile([128, 1], F32, tag="bu")
            nc.sync.dma_start(out=bu[:], in_=bass.AP(tensor=p_bgu, offset=INNER + cc * 128, ap=[[1, 128], [1, 1]]))
            cw = bias_pool.tile([128, 3], F32, tag="cw")
            nc.sync.dma_start(out=cw[:], in_=p_cw[cc * 128:(cc + 1) * 128, :])

            gact = fsm_pool.tile([128, RO + 2], F32, tag="gact")
            hf = fsm_pool.tile([128, RO + 2], F32, tag="hf")
            for h0, w in HALVES:
                psg = pp_g.tile([128, 258], F32, tag="psg")
                for k in range(8):
                    nc.tensor.matmul(
                        psg[:, :w], wupg_sb[k][:, c * 128:(c + 1) * 128],
                        qn2T[k][:, h0:h0 + w], start=(k == 0), stop=(k == 7),
                    )
                nc.scalar.activation(gact[:, h0:h0 + w], psg[:, :w], AF.Silu, bias=bg[:, 0:1])
                psu = pp_u.tile([128, 258], F32, tag="psu")
                for k in range(8):
                    nc.tensor.matmul(
                        psu[:, :w], wupu_sb[k][:, c * 128:(c + 1) * 128],
                        qn2T[k][:, h0:h0 + w], start=(k == 0), stop=(k == 7),
                    )
                nc.vector.scalar_tensor_tensor(
                    out=hf[:, h0:h0 + w], in0=psu[:, :w], scalar=bu[:, 0:1],
                    in1=gact[:, h0:h0 + w], op0=ALU.add, op1=ALU.mult,
                )
            if cc >= 2 and cc % 2 == 0:
                emit_wdown_pair(cc // 2 - 1)
            # mask halo cols at batch edges (conv zero-pad)
            nc.vector.tensor_scalar_mul(hf[:, RO:RO + 1], hf[:, RO:RO + 1], mask_p[:, 0:1])
            nc.vector.tensor_scalar_mul(hf[:, RO + 1:RO + 2], hf[:, RO + 1:RO + 2], mask_n[:, 0:1])
            # depthwise conv along rows: cols 0..511 owned, 512=prev, 513=next.
            # center tap on ACT (scale is per-partition), side taps DVE; the
            # final two taps write the fp8 DoubleRow pair tile directly.
            hfb = fsm_pool.tile([128, RO], BF16, tag="hfb")
            dst = hfc[cc // 2]
            jj = cc % 2
            nc.scalar.activation(hfb[:, 0:RO], hf[:, 0:RO], AF.Copy, scale=cw[:, 1:2])
            nc.vector.scalar_tensor_tensor(
                out=hfb[:, 1:RO], in0=hf[:, 0:RO - 1], scalar=cw[:, 0:1],
                in1=hfb[:, 1:RO], op0=ALU.mult, op1=ALU.add,
            )
            nc.vector.scalar_tensor_tensor(
                out=hfb[:, 0:1], in0=hf[:, RO:RO + 1], scalar=cw[:, 0:1],
                in1=hfb[:, 0:1], op0=ALU.mult, op1=ALU.add,
            )
            nc.vector.scalar_tensor_tensor(
                out=dst[:, jj, 0:RO - 1], in0=hf[:, 1:RO], scalar=cw[:, 2:3],
                in1=hfb[:, 0:RO - 1], op0=ALU.mult, op1=ALU.add,
            )
            nc.vector.scalar_tensor_tensor(
                out=dst[:, jj, RO - 1:RO], in0=hf[:, RO + 1:RO + 2], scalar=cw[:, 2:3],
                in1=hfb[:, RO - 1:RO], op0=ALU.mult, op1=ALU.add,
            )
    emit_wdown_pair(15)
    ffn_stack.close()

    # ---------------- Phase F4: Wdown second half + residual + output ----------
    out_pool = f2_stack.enter_context(tc.tile_pool(name="outp", bufs=4))
    wd1_pool = f2_stack.enter_context(tc.tile_pool(name="wd1", bufs=6))
    pp_d1 = f2_stack.enter_context(tc.tile_pool(name="pp_d1", bufs=1, space="PSUM"))
    psd1 = [pp_d1.tile([128, 512], F32, name=f"psd1_{i}") for i in range(4)]
    for t in range(16):
        wd_t = wd1_pool.tile([128, 2, 512], F8, tag="wd1")
        nc.sync.dma_start(out=wd_t[:, :, :], in_=p_wd18[t * 128:(t + 1) * 128, :])
        for i in range(4):
            nc.tensor.matmul(
                psd1[i][:], hfc[t][:, :, i * 128:(i + 1) * 128],
                wd_t[:, :, :], start=(t == 0), stop=(t == 15), perf_mode=DR,
            )
    for i in range(4):
        o_t = out_pool.tile([128, D], F32, tag="osb")
        nc.vector.scalar_tensor_tensor(
            out=o_t[:, 0:512], in0=psd0[i][:], scalar=WDINV,
            in1=qint[i][:, 0:512], op0=ALU.mult, op1=ALU.add,
        )
        nc.vector.scalar_tensor_tensor(
            out=o_t[:, 512:1024], in0=psd1[i][:], scalar=WDINV,
            in1=qint[i][:, 512:1024], op0=ALU.mult, op1=ALU.add,
        )
        out_q = (nc.sync, nc.scalar, nc.gpsimd, nc.sync)[i]
        out_q.dma_start(out=p_out[i * 128:(i + 1) * 128, :], in_=o_t[:])
    f2_stack.close()
    f34_stack.close()


def kernel(**inputs) -> np.ndarray:
    Q_in = np.ascontiguousarray(np.asarray(inputs["Q_in"], dtype=np.float32))
    X = np.ascontiguousarray(np.asarray(inputs["X"], dtype=np.float32))
    Wq = np.asarray(inputs["Wq"], dtype=np.float32)
    Wk = np.asarray(inputs["Wk"], dtype=np.float32)
    Wv = np.asarray(inputs["Wv"], dtype=np.float32)
    Wo = np.asarray(inputs["Wo"], dtype=np.float32)
    Wup = np.asarray(inputs["Wup"], dtype=np.float32)
    conv_w = np.asarray(inputs["conv_w"], dtype=np.float32)
    Wdown = np.asarray(inputs["Wdown"], dtype=np.float32)
    g1 = np.asarray(inputs["g1"], dtype=np.float32)
    b1 = np.asarray(inputs["b1"], dtype=np.float32)
    g2 = np.asarray(inputs["g2"], dtype=np.float32)
    b2 = np.asarray(inputs["b2"], dtype=np.float32)
    dt = float(np.asarray(inputs["dt"], dtype=np.float32))

    # softplus(dt) on host; baked into the NEFF as an immediate
    dt_safe = float(np.log1p(np.exp(dt)))

    # fold g2/b2 into Wup (LN2's affine commutes into the up-projection)
    wup_f = g2[:, None] * Wup
    bias_gu = np.ascontiguousarray(b2 @ Wup)
    cw3 = np.ascontiguousarray(conv_w[:, 0, :])

    key = round(dt_safe, 9)
    if key not in _cache:
        _cache[key] = _build(dt_safe)
    nc = _cache[key]

    bf = ml_dtypes.bfloat16
    f8 = ml_dtypes.float8_e4m3
    wq_b = np.ascontiguousarray(Wq.astype(bf))
    wk_b = np.ascontiguousarray(Wk.astype(bf))
    wv_b = np.ascontiguousarray(Wv.astype(bf))
    wo_b = np.ascontiguousarray(Wo.astype(bf))
    wup_b = np.ascontiguousarray(wup_f.astype(bf))

    # Wdown DoubleRow pair packing: tile-row t*128+r holds inner element
    # i = 128*(2t+j)+r in slot j; cols are (j, c).
    def pack_down(w):  # [INNER, 512] -> [INNER//2, D] fp8 pair layout
        t = w.reshape(16, 2, 128, 512)
        return np.ascontiguousarray(
            t.transpose(0, 2, 1, 3).reshape(INNER // 2, D).astype(f8))

    wd08 = pack_down(WDS * Wdown[:, 0:512])
    wd18 = pack_down(WDS * Wdown[:, 512:1024])

    in_maps = []
    for core in range(8):
        b, q = divmod(core, 4)
        qin = np.ascontiguousarray(Q_in[b, q * RO:(q + 1) * RO])
        xb1 = np.ascontiguousarray(X[b, q * RO:(q + 1) * RO] + b1[None, :])
        masks = np.array(
            [1.0 if q > 0 else 0.0, 1.0 if q < NQ - 1 else 0.0], dtype=np.float32
        )
        # maskmat.T @ gathered_boundaries = [prev-halo row; next-halo row]
        mm = np.zeros((2 * NQ, 2), dtype=np.float32)
        if q > 0:
            mm[2 * (q - 1) + 1, 0] = 1.0
        if q < NQ - 1:
            mm[2 * (q + 1), 1] = 1.0
        in_maps.append({
            "q_in": qin, "x_b1": xb1, "wq": wq_b, "wk": wk_b, "wv": wv_b,
            "wo": wo_b, "wup": wup_b, "bias_gu": bias_gu,
            "wd08": wd08, "wd18": wd18, "cw3": cw3,
            "g1": np.ascontiguousarray(g1), "masks": masks, "maskmat": mm,
        })

    global _last_in_maps
    _last_in_maps = in_maps
    res = run_bass_kernel_spmd(nc, in_maps, core_ids=list(range(8)))

    out = np.empty((B, N, D), dtype=np.float32)
    for core in range(8):
        b, q = divmod(core, 4)
        out[b, q * RO:(q + 1) * RO] = res.results[core]["out"]
    return out



# revision 68
# speedup vs baseline: 1.3357x; 1.0112x over previous
"""Distributed Trainium2 Bass kernel for nn_AMK_Block (kernelized-attention + ConvSwiGLU).

Sharding: sequence-parallel. Each of the 8 cores owns (batch b, query-row block q):
core = b*4 + q, rows q*512..q*512+511 of batch b, ALL heads. Each core computes
Q/K/V projections for its rows, AllGathers PhiK^T and V(+ones) across the 4 cores
of its batch group (fp8), then computes its 512 rows of attention, Wo, LN2 and
the full FFN locally. The depthwise-conv halo rows of Q_interact come from a tiny
third AllGather of boundary rows, extracted rank-agnostically with a mask-matrix
matmul. Weight matmuls run in bf16 (fp32 PSUM accumulation); the attention
kernel-matrix matmuls run in fp8 (PhiQ/PhiK/V/W^2 evicted as e4m3, Attr uses
DoubleRow packed k-pairs); norm/statistics in fp32. Wdown's first column-half is
interleaved into the FFN chunk stream so only half remains as a tail.
"""

import sys

sys.path.insert(0, "/opt/trn_rl_repo")

from contextlib import ExitStack

import ml_dtypes
import numpy as np

import concourse.bass as bass
import concourse.tile as tile
from concourse import bacc, mybir
from concourse.bass_utils import run_bass_kernel_spmd
from concourse.masks import make_identity

F32 = mybir.dt.float32
F32R = mybir.dt.float32r
BF16 = mybir.dt.bfloat16
F8 = mybir.dt.float8e4
ALU = mybir.AluOpType
AF = mybir.ActivationFunctionType
DR = mybir.MatmulPerfMode.DoubleRow

B, N, D = 2, 2048, 1024
H, DH = 16, 64
INNER = 4096
LN_EPS = 1e-5
WSC = 1.0 / 32.0          # W^2 is evicted as (W/32)^2 = W^2/1024 in fp8
WSC2 = WSC * WSC
UPS = 32.0                # Wup fp8 host scale (values ~N(0,1/32) -> ~N(0,1))
UPSC = 1.0 / UPS
WDS = 64.0                # Wdown fp8 host scale
WDINV = 1.0 / WDS

RO = 512          # owned rows per core
NQ = 4            # cores per batch group
GROUPS = [[0, 1, 2, 3], [4, 5, 6, 7]]
CHUNKS = [(0, 128), (128, 128), (256, 128), (384, 128)]
HALVES = [(0, 258), (258, 256)]  # even halves of 514; halo cols 512/513 in 2nd

_cache: dict[float, object] = {}
_last_in_maps: list | None = None


def _build(dt_safe: float):
    nc = bacc.Bacc("TRN2", target_bir_lowering=False, debug=False, num_devices=8)

    # ---------------- DRAM parameters (per-core shapes) ----------------
    p_qin = nc.declare_dram_parameter("q_in", [RO, D], F32, isOutput=False)
    p_xb1 = nc.declare_dram_parameter("x_b1", [RO, D], F32, isOutput=False)
    p_wq = nc.declare_dram_parameter("wq", [D, D], BF16, isOutput=False)
    p_wk = nc.declare_dram_parameter("wk", [D, D], BF16, isOutput=False)
    p_wv = nc.declare_dram_parameter("wv", [D, D], BF16, isOutput=False)
    p_wo = nc.declare_dram_parameter("wo", [D, D], BF16, isOutput=False)
    p_wup = nc.declare_dram_parameter("wup", [D, 2 * INNER], BF16, isOutput=False)
    p_bgu = nc.declare_dram_parameter("bias_gu", [2 * INNER], F32, isOutput=False)
    p_wd08 = nc.declare_dram_parameter("wd08", [INNER // 2, D], F8, isOutput=False)
    p_wd18 = nc.declare_dram_parameter("wd18", [INNER // 2, D], F8, isOutput=False)
    p_cw = nc.declare_dram_parameter("cw3", [INNER, 3], F32, isOutput=False)
    p_g1 = nc.declare_dram_parameter("g1", [D], F32, isOutput=False)
    p_mask = nc.declare_dram_parameter("masks", [2], F32, isOutput=False)
    p_mm = nc.declare_dram_parameter("maskmat", [2 * NQ, 2], F32R, isOutput=False)
    p_out = nc.declare_dram_parameter("out", [RO, D], F32, isOutput=True)

    with tile.TileContext(nc) as tc:
        build_ctx = ExitStack()
        with build_ctx:
            _emit(nc, tc, build_ctx, dt_safe, p_qin, p_xb1, p_wq, p_wk, p_wv,
                  p_wo, p_wup, p_bgu, p_wd08, p_wd18, p_cw, p_g1,
                  p_mask, p_mm, p_out)
    nc.finalize()
    return nc


def _emit(nc, tc, bctx, dt_safe, p_qin, p_xb1, p_wq, p_wk, p_wv, p_wo, p_wup,
          p_bgu, p_wd08, p_wd18, p_cw, p_g1, p_mask, p_mm, p_out):
    # ---------------- constant tiles (gpsimd queue: off the qin path) ------
    consts = bctx.enter_context(tc.tile_pool(name="consts", bufs=1))
    g1b = consts.tile([128, D], F32, name="g1b")
    nc.gpsimd.dma_start(
        out=g1b[:],
        in_=bass.AP(tensor=p_g1, offset=0, ap=[[0, 128], [1, D]]),
    )
    ident_f = consts.tile([128, 128], F32, name="ident_f")
    make_identity(nc, ident_f[:])
    ident_r = consts.tile([128, 128], F32R, name="ident_r")
    nc.gpsimd.dma_start(out=ident_r[:], in_=ident_f[:])
    mask_p = consts.tile([128, 1], F32, name="mask_p")
    nc.gpsimd.dma_start(out=mask_p[:], in_=bass.AP(tensor=p_mask, offset=0, ap=[[0, 128], [1, 1]]))
    mask_n = consts.tile([128, 1], F32, name="mask_n")
    nc.gpsimd.dma_start(out=mask_n[:], in_=bass.AP(tensor=p_mask, offset=1, ap=[[0, 128], [1, 1]]))
    ones_col = consts.tile([128, 1], F8, name="ones_col")
    nc.vector.memset(ones_col[:], 1.0)
    eps_t = consts.tile([128, 1], F32, name="eps_t")
    nc.vector.memset(eps_t[:], LN_EPS)
    maskmat = consts.tile([2 * NQ, 2], F32R, name="maskmat")
    nc.gpsimd.dma_start(out=maskmat[:], in_=p_mm[:, :])

    # DRAM scratch for the collectives (fp8 payloads)
    dram = bctx.enter_context(tc.tile_pool(name="dram", bufs=1, space="DRAM"))
    kag_in = dram.tile([H * DH, RO], F8, name="kag_in")        # PhiK^T local slice
    kag1 = dram.tile([NQ * 512, RO], F8, name="kag1")          # gathered heads 0-7
    kag2 = dram.tile([NQ * 512, RO], F8, name="kag2")          # gathered heads 8-15
    vag_in = dram.tile([RO, H * 66], F8, name="vag_in")        # V(+ones) local rows
    vag1 = dram.tile([NQ * 256, H * 66], F8, name="vag1")      # gathered rows 0-255
    vag2 = dram.tile([NQ * 256, H * 66], F8, name="vag2")      # gathered rows 256-511
    hag_in = dram.tile([2, D], F32R, name="hag_in")            # my boundary Qint rows
    hag = dram.tile([2 * NQ, D], F32R, name="hag")             # gathered boundaries

    ev_state = [0]

    def evict_copy(dst_ap, src_ap):
        ev_state[0] += 1
        if ev_state[0] % 2 == 0:
            nc.vector.tensor_copy(dst_ap, src_ap)
        else:
            nc.scalar.activation(dst_ap, src_ap, AF.Copy)

    ln_pool = bctx.enter_context(tc.tile_pool(name="ln", bufs=3))

    def layernorm_to(x_ap, p):
        """Returns (mv, rstd) tiles: mean in mv[:,0:1], rstd [p,1], for x_ap [p, D]."""
        st = ln_pool.tile([128, 2, 6], F32, tag="bn_st")
        xr = x_ap.rearrange("p (s f) -> p s f", s=2)
        for s in range(2):
            nc.vector.bn_stats(st[:p, s, :], xr[:, s, :])
        mv = ln_pool.tile([128, 2], F32, tag="bn_mv")
        nc.vector.bn_aggr(mv[:p], st[:p])
        rstd = ln_pool.tile([128, 1], F32, tag="bn_rstd")
        nc.scalar.activation(rstd[:p], mv[:p, 1:2], AF.Sqrt, bias=eps_t[:p, 0:1])
        nc.vector.reciprocal(rstd[:p], rstd[:p])
        return mv, rstd

    # ---- lifetime stacks (must nest LIFO): f34 > av2 > av > phase stacks ----
    f34_stack = ExitStack()   # hfc + qint: from Wo until the end
    av2_stack = ExitStack()   # mTc: until end of Wo
    av_stack = ExitStack()    # vT, phiQ: until end of head loop
    hfc_pool = f34_stack.enter_context(tc.tile_pool(name="hfc", bufs=1))
    qint_pool = f34_stack.enter_context(tc.tile_pool(name="qint", bufs=1))
    wup_pool = f34_stack.enter_context(tc.tile_pool(name="wup", bufs=12))
    mTc_pool = av2_stack.enter_context(tc.tile_pool(name="mTc", bufs=1))
    vT_pool = av_stack.enter_context(tc.tile_pool(name="vT", bufs=1))
    phiQ_pool = av_stack.enter_context(tc.tile_pool(name="phiQ", bufs=1))
    mTc = [mTc_pool.tile([128, RO], BF16, name=f"mTc{j}") for j in range(8)]
    vT = [vT_pool.tile([128, RO], F32R, name=f"vT{j}") for j in range(8)]
    phiQT = [phiQ_pool.tile([128, RO], F8, name=f"phiQT{j}") for j in range(8)]

    # ---------------- Phase P: LN1 + Hc + transposes ----------------
    p_stack = ExitStack()
    hcT_pool = p_stack.enter_context(tc.tile_pool(name="hcT", bufs=1))
    hcT = [hcT_pool.tile([128, RO], BF16, name=f"hcT{j}") for j in range(8)]
    io_pool = p_stack.enter_context(tc.tile_pool(name="io", bufs=4))
    hc_pool = p_stack.enter_context(tc.tile_pool(name="hc", bufs=2))
    # the PSUM phases of P (hc transposes / projections / V transposes) are
    # disjoint in time, so each gets its own short-lived 4-6 buffer pool
    t_stack = ExitStack()
    pp_t = t_stack.enter_context(tc.tile_pool(name="pp_t", bufs=4, space="PSUM"))

    # stream all qin/xb1 chunks up front; chunk 0 is the critical path, so
    # its rows are split across both hardware DMA queues
    qin_ts, xb1_ts = [], []
    for i, (r0, p) in enumerate(CHUNKS):
        qin_t = io_pool.tile([p, D], F32, tag="qin")
        if i == 0:
            h = p // 2
            nc.sync.dma_start(out=qin_t[0:h, :], in_=p_qin[r0:r0 + h, :])
            nc.scalar.dma_start(out=qin_t[h:p, :], in_=p_qin[r0 + h:r0 + p, :])
        else:
            nc.sync.dma_start(out=qin_t[:], in_=p_qin[r0:r0 + p, :])
        qin_ts.append(qin_t)
        xb1_t = io_pool.tile([p, D], F32, tag="xb1")
        nc.scalar.dma_start(out=xb1_t[:], in_=p_xb1[r0:r0 + p, :])
        xb1_ts.append(xb1_t)

    for i, (r0, p) in enumerate(CHUNKS):
        qin_t, xb1_t = qin_ts[i], xb1_ts[i]
        mv, rstd = layernorm_to(qin_t[:p, :], p)
        hc_t = hc_pool.tile([p, D], F32, tag="hc")
        nc.vector.tensor_scalar(
            out=hc_t[:p, :], in0=qin_t[:p, :], scalar1=mv[:p, 0:1],
            scalar2=rstd[:p, 0:1], op0=ALU.subtract, op1=ALU.mult,
        )
        nc.vector.tensor_mul(hc_t[:p, :], hc_t[:p, :], g1b[:p, :])
        nc.vector.tensor_add(hc_t[:p, :], hc_t[:p, :], xb1_t[:p, :])

        # transpose this row-chunk into the 8 hcT column tiles
        for j in range(8):
            tp = pp_t.tile([128, 128], F32, tag="tp")
            nc.tensor.transpose(tp[:128, :p], hc_t[:p, j * 128:(j + 1) * 128], ident_f[:p, :p])
            evict_copy(hcT[j][:, r0:r0 + p], tp[:128, :p])
    t_stack.close()

    # ---------------- Phase P: projections ----------------
    # Order: K -> K-AllGather (smallest latency to first collective), then Q
    # (needed with K for the W matmuls), then V -> V-AllGather. The rings
    # serialize on the collective lane, so K's goes first.
    wstream = p_stack.enter_context(tc.tile_pool(name="wstream", bufs=12))
    pr_stack = ExitStack()
    pp_a = pr_stack.enter_context(tc.tile_pool(name="pp_a", bufs=6, space="PSUM"))
    elu_pool = p_stack.enter_context(tc.tile_pool(name="elu", bufs=3))

    def elu1_evict(dst_ap, src_psum_ap, p, w):
        """dst = elu(src)+1 = relu(src) + exp(min(src,0)) (fp8 out)"""
        tmin = elu_pool.tile([128, 512], F32, tag="tmin")
        nc.vector.tensor_scalar_min(tmin[:p, :w], src_psum_ap, 0.0)
        texp = elu_pool.tile([128, 512], F32, tag="texp")
        nc.scalar.activation(texp[:p, :w], tmin[:p, :w], AF.Exp)
        nc.vector.scalar_tensor_tensor(
            out=dst_ap, in0=src_psum_ap, scalar=0.0, in1=texp[:p, :w],
            op0=ALU.max, op1=ALU.add,
        )

    # K^T -> PhiK^T (fp8) -> kag_in; two chunked AllGathers (heads 0-7, 8-15)
    wk_sb = []
    for k in range(8):
        w_t = wstream.tile([128, D], BF16, tag="wproj")
        nc.sync.dma_start(out=w_t[:], in_=p_wk[k * 128:(k + 1) * 128, :])
        wk_sb.append(w_t)
    phiK_pool = p_stack.enter_context(tc.tile_pool(name="phiK", bufs=4))
    for j in range(8):
        ps = pp_a.tile([128, 512], F32, tag="proj")
        for k in range(8):
            nc.tensor.matmul(
                ps[:], wk_sb[k][:, j * 128:(j + 1) * 128],
                hcT[k][:, 0:RO], start=(k == 0), stop=(k == 7),
            )
        phiK_t = phiK_pool.tile([128, RO], F8, tag="phiK")
        elu1_evict(phiK_t[:, :], ps[:], 128, RO)
        nc.sync.dma_start(out=kag_in[j * 128:(j + 1) * 128, :], in_=phiK_t[:])
        if j == 3:
            nc.gpsimd.collective_compute(
                "AllGather", ALU.bypass, replica_groups=GROUPS,
                ins=[kag_in[0:512, :].opt()], outs=[kag1[:].opt()],
            )
    nc.gpsimd.collective_compute(
        "AllGather", ALU.bypass, replica_groups=GROUPS,
        ins=[kag_in[512:1024, :].opt()], outs=[kag2[:].opt()],
    )

    # Q^T -> PhiQ^T (fp8, kept in SBUF)
    wq_sb = []
    for k in range(8):
        w_t = wstream.tile([128, D], BF16, tag="wproj")
        nc.scalar.dma_start(out=w_t[:], in_=p_wq[k * 128:(k + 1) * 128, :])
        wq_sb.append(w_t)
    for j in range(8):
        ps = pp_a.tile([128, 512], F32, tag="proj")
        for k in range(8):
            nc.tensor.matmul(
                ps[:], wq_sb[k][:, j * 128:(j + 1) * 128],
                hcT[k][:, 0:RO], start=(k == 0), stop=(k == 7),
            )
        elu1_evict(phiQT[j][:, :], ps[:], 128, RO)

    # V^T, then transpose back to row-major (+ones cols, fp8) and stage for AGs
    wv_sb = []
    for k in range(8):
        w_t = wstream.tile([128, D], BF16, tag="wproj")
        nc.sync.dma_start(out=w_t[:], in_=p_wv[k * 128:(k + 1) * 128, :])
        wv_sb.append(w_t)
    for j in range(8):
        ps = pp_a.tile([128, 512], F32, tag="proj")
        for k in range(8):
            nc.tensor.matmul(
                ps[:], wv_sb[k][:, j * 128:(j + 1) * 128],
                hcT[k][:, 0:RO], start=(k == 0), stop=(k == 7),
            )
        evict_copy(vT[j][:, :], ps[:])
    pr_stack.close()

    tv_stack = ExitStack()
    pp_tv = tv_stack.enter_context(tc.tile_pool(name="pp_tv", bufs=4, space="PSUM"))
    vs_pool = p_stack.enter_context(tc.tile_pool(name="vs", bufs=3))
    for i in range(4):
        r0 = i * 128
        vstage = vs_pool.tile([128, H * 66], F8, tag="vstage")
        for j in range(8):
            tpv = pp_tv.tile([128, 128], F32R, tag="tpv")
            nc.tensor.transpose(tpv[:], vT[j][:, r0:r0 + 128], ident_r[:])
            h0, h1 = 2 * j, 2 * j + 1
            evict_copy(vstage[:, h0 * 66:h0 * 66 + 64], tpv[:, 0:64])
            evict_copy(vstage[:, h1 * 66:h1 * 66 + 64], tpv[:, 64:128])
        # per-head ones column (64) + zero pad column (65), strided memsets
        vsr = vstage[:].rearrange("p (h d) -> p h d", h=H)
        nc.vector.memset(vsr[:, :, 64:65], 1.0)
        nc.vector.memset(vsr[:, :, 65:66], 0.0)
        nc.gpsimd.dma_start(out=vag_in[r0:r0 + 128, :], in_=vstage[:])
        if i == 1:
            nc.gpsimd.collective_compute(
                "AllGather", ALU.bypass, replica_groups=GROUPS,
                ins=[vag_in[0:256, :].opt()], outs=[vag1[:].opt()],
            )
    nc.gpsimd.collective_compute(
        "AllGather", ALU.bypass, replica_groups=GROUPS,
        ins=[vag_in[256:512, :].opt()], outs=[vag2[:].opt()],
    )
    tv_stack.close()

    p_stack.close()

    # ---------------- Phase A: attention ----------------
    # Per head-group g (4 heads): W(g) = 64 fp8 matmuls (K=64) evicted as
    # (W/32)^2 fp8 into DoubleRow pair tiles; Attr(g) = per head 8 fp8-DR
    # matmuls over (m-block pair, key) tiles. Emission order W0 W1 A0 W2 A1
    # W3 A2 A3 keeps the PE busy while the V AllGathers land.
    # Wo weights: pool created first (released after attention pools), loads
    # issued now so the Wo phase starts instantly
    wo_stack = ExitStack()
    wo_pool = wo_stack.enter_context(tc.tile_pool(name="wo", bufs=8))
    wo_sb = []
    for k in range(8):
        w_t = wo_pool.tile([128, D], BF16, tag="wo")
        nc.scalar.dma_start(out=w_t[:], in_=p_wo[k * 128:(k + 1) * 128, :])
        wo_sb.append(w_t)
    # prefetch the first FFN superchunk's gate weights during attention
    wupg_pf = []
    for k in range(8):
        wg = wup_pool.tile([128, 512], BF16, tag="wupg")
        nc.sync.dma_start(out=wg[:], in_=p_wup[k * 128:(k + 1) * 128, 0:512])
        wupg_pf.append(wg)

    a_stack = ExitStack()
    kq_pool = a_stack.enter_context(tc.tile_pool(name="kq", bufs=3))
    vhd_pool = a_stack.enter_context(tc.tile_pool(name="vhd", bufs=16))
    # dual-fp8 LDWEIGHTS needs stationary width % 32 == 0: vhd is 96 wide
    # (V 0-63, ones 64, pad 65-95). DMA writes cols 0-65; zero the pad cols
    # once per pool buffer (round-robin reuse keeps them zero).
    for _ in range(16):
        vz = vhd_pool.tile([128, 8, 96], F8, tag="vhd")
        nc.vector.memset(vz[:, :, 66:96], 0.0)
    wt_pool = a_stack.enter_context(tc.tile_pool(name="wt", bufs=64))
    asm_pool = a_stack.enter_context(tc.tile_pool(name="asm", bufs=3))
    pp_w = a_stack.enter_context(tc.tile_pool(name="pp_w", bufs=4, space="PSUM"))
    pp_at = a_stack.enter_context(tc.tile_pool(name="pp_at", bufs=4, space="PSUM"))

    sq_state = [0]

    def square_evict(dst_ap, src_psum_ap):
        """dst = src^2 fp8; src is already W/32 (W > 0, relu is a no-op).
        Rotated 5:2 across ACT/DVE: ACT streams ~1 col/ns single-pass; the
        DVE two-pass path costs ~2x that, so it only soaks the overflow."""
        sq_state[0] = (sq_state[0] + 1) % 7
        if sq_state[0] < 5:
            nc.scalar.activation(dst_ap, src_psum_ap, AF.Square, scale=WSC)
        else:
            tr = asm_pool.tile([128, 512], BF16, tag="r2tmpv")
            nc.vector.tensor_scalar_mul(tr[:, :], src_psum_ap, WSC)
            nc.vector.tensor_mul(dst_ap, tr[:, :], tr[:, :])

    def emit_w(hg):
        """W^T for 4 heads of group hg -> wtp fp8 DoubleRow pair tiles.
        The two heads sharing a kq/phiQT tile (PE row halves 0-63 / 64-127)
        are interleaved: consecutive matmuls hit disjoint row groups, so
        LDWEIGHTS overlaps the in-flight matmul."""
        kag_t = kag1 if hg < 2 else kag2
        kq_sb = {}
        for j2 in (2 * hg, 2 * hg + 1):
            hrow = (j2 % 4) * 128  # row offset of head-pair j2 within kag_t
            kt = kq_pool.tile([128, NQ, RO], F8, tag="kq")
            ksrc = kag_t[:, :]
            nc.sync.dma_start(
                out=kt[:, :, :],
                in_=bass.AP(tensor=ksrc.tensor, offset=ksrc.offset + hrow * RO,
                            ap=[[RO, 128], [512 * RO, NQ], [1, RO]]),
            )
            kq_sb[j2] = kt
        wtp = {}
        for hh in range(4):
            h = hg * 4 + hh
            wtp[h] = [wt_pool.tile([128, 2, RO], F8, tag="wt", name=f"wt{h}_{t}")
                      for t in range(8)]
        for j2 in (2 * hg, 2 * hg + 1):
            hA, hB = 2 * j2, 2 * j2 + 1
            for m in range(16):
                qq, lc = m // 4, m % 4
                for off, h in ((0, hA), (64, hB)):
                    psw = pp_w.tile([128, 512], F32, tag="psw")
                    nc.tensor.matmul(
                        psw[:], kq_sb[j2][off:off + 64, qq, lc * 128:(lc + 1) * 128],
                        phiQT[j2][off:off + 64, :], start=True, stop=True,
                    )
                    square_evict(wtp[h][m // 2][:, m % 2, :], psw)
        return wtp

    # pair order follows the chunked V gathers: vag1 pairs (lc 0,1) first
    T_ORDER = [qq * 2 for qq in range(NQ)] + [qq * 2 + 1 for qq in range(NQ)]

    def emit_attr(hg, wtp):
        pats = []
        for hh in range(4):
            h = hg * 4 + hh
            pat = pp_at.tile([96, 512], F32, tag="pat", name=f"pat{h}")
            vh = {}
            for half in range(2):
                vsrc = vag1 if half == 0 else vag2
                vt = vhd_pool.tile([128, 8, 96], F8, tag="vhd")
                vap = vsrc[:, :]
                W16 = H * 66
                nc.sync.dma_start(
                    out=vt[:, :, 0:66],
                    in_=bass.AP(tensor=vap.tensor, offset=vap.offset + h * 66,
                                ap=[[W16, 128], [128 * W16, 8], [1, 66]]))
                vh[half] = vt
            for ti, t in enumerate(T_ORDER):
                qq, half = t // 2, t % 2
                nc.tensor.matmul(
                    pat[:], vh[half][:, 2 * qq:2 * qq + 2, :], wtp[h][t][:, :, :],
                    start=(ti == 0), stop=(ti == 7), perf_mode=DR,
                )
            pats.append(pat)
        for hh in range(4):
            h = hg * 4 + hh
            j2, off = h // 2, (h % 2) * 64
            nrm = asm_pool.tile([1, RO], F32, tag="nrm")
            nc.vector.tensor_scalar_add(nrm[0:1, :], pats[hh][64:65, :], WSC2)
            nc.vector.reciprocal_approx_fast(out=nrm[:], in_=nrm[:])
            rcb = asm_pool.tile([64, RO], F32, tag="rcb")
            nc.gpsimd.partition_broadcast(rcb[:], nrm[:])
            tm = asm_pool.tile([128, RO], F32, tag="tm")
            nc.vector.tensor_mul(tm[off:off + 64, :], pats[hh][0:64, :], rcb[:, :])
            nc.vector.tensor_sub(
                mTc[j2][off:off + 64, :], tm[off:off + 64, :],
                vT[j2][off:off + 64, :],
            )

    wtp_q = [emit_w(0), emit_w(1)]
    for hg in range(4):
        emit_attr(hg, wtp_q[hg])
        if hg + 2 < 4:
            wtp_q.append(emit_w(hg + 2))
    a_stack.close()

    # ---------------- Phase A5: Wo + Q_interact ----------------
    # Chunk order 0,3,1,2 so the conv-halo boundary rows exist after two
    # chunks and their AllGather overlaps the rest of Wo + LN2.
    a5_stack = ExitStack()
    qi_pool = a5_stack.enter_context(tc.tile_pool(name="qi", bufs=3))
    pp_o = a5_stack.enter_context(tc.tile_pool(name="pp_o", bufs=4, space="PSUM"))
    qint = [None] * 4
    for oi, i in enumerate((0, 3, 1, 2)):
        r0, p = CHUNKS[i]
        qin_t = qi_pool.tile([p, D], F32, tag="qin2")
        nc.sync.dma_start(out=qin_t[:], in_=p_qin[r0:r0 + p, :])
        qi = qint_pool.tile([p, D], F32, name=f"qint{i}")
        for half in range(2):
            pso = pp_o.tile([128, 512], F32, tag="pso")
            for k in range(8):
                nc.tensor.matmul(
                    pso[:p, :], mTc[k][:, r0:r0 + p],
                    wo_sb[k][:, half * 512:(half + 1) * 512],
                    start=(k == 0), stop=(k == 7),
                )
            nc.vector.scalar_tensor_tensor(
                out=qi[:p, half * 512:(half + 1) * 512], in0=pso[:p, :],
                scalar=dt_safe, in1=qin_t[:p, half * 512:(half + 1) * 512],
                op0=ALU.mult, op1=ALU.add,
            )
        qint[i] = qi
        if oi == 1:
            # boundary rows ready: stage + AllGather (conv halo exchange)
            nc.gpsimd.dma_start(out=hag_in[0:1, :], in_=qint[0][0:1, :])
            nc.gpsimd.dma_start(out=hag_in[1:2, :], in_=qint[3][127:128, :])
            nc.gpsimd.collective_compute(
                "AllGather", ALU.bypass, replica_groups=GROUPS,
                ins=[hag_in[:].opt()], outs=[hag[:].opt()],
            )
    a5_stack.close()
    wo_stack.close()
    av_stack.close()   # frees vT, phiQ
    av2_stack.close()  # frees mTc

    # ---------------- Phase F: LN2 + transpose + FFN ----------------
    hfc = []

    qn2T_pool = f34_stack.enter_context(tc.tile_pool(name="qn2T", bufs=1))
    f_stack = ExitStack()
    qn2_pool = f_stack.enter_context(tc.tile_pool(name="qn2", bufs=2))
    pp_f = f_stack.enter_context(tc.tile_pool(name="pp_f", bufs=4, space="PSUM"))
    # qn2T cols: 0..511 owned rows, 512 = prev-halo row, 513 = next-halo row
    qn2T = [qn2T_pool.tile([128, RO + 2], BF16, name=f"qn2T{j}") for j in range(8)]
    for i, (r0, p) in enumerate(CHUNKS):
        mv, rstd = layernorm_to(qint[i][:p, :], p)
        qn2_t = qn2_pool.tile([p, D], F32, tag="qn2")
        nc.vector.tensor_scalar(
            out=qn2_t[:p, :], in0=qint[i][:p, :], scalar1=mv[:p, 0:1],
            scalar2=rstd[:p, 0:1], op0=ALU.subtract, op1=ALU.mult,
        )
        for j in range(8):
            tp = pp_f.tile([128, 128], F32, tag="tpf")
            nc.tensor.transpose(tp[:128, :p], qn2_t[:p, j * 128:(j + 1) * 128], ident_f[:p, :p])
            evict_copy(qn2T[j][:, r0:r0 + p], tp[:128, :p])

    # halo rows: extract prev/next boundary rows via maskmat.T @ gathered,
    # then LN2 + transpose into qn2T cols 512/513
    pp_h = f_stack.enter_context(tc.tile_pool(name="pp_h", bufs=1, space="PSUM"))
    hg_sb = qn2_pool.tile([2 * NQ, D], F32R, name="hg_sb")
    nc.sync.dma_start(out=hg_sb[:], in_=hag[:, :])
    qih = qn2_pool.tile([2, D], F32, name="qih")
    for half in range(2):
        ph = pp_h.tile([2, 512], F32, tag="psh", name=f"ph{half}")
        nc.tensor.matmul(
            ph[:], maskmat[:], hg_sb[:, half * 512:(half + 1) * 512],
            start=True, stop=True,
        )
        nc.vector.tensor_copy(qih[:, half * 512:(half + 1) * 512], ph[:])
    mv, rstd = layernorm_to(qih[:2, :], 2)
    qn2h = qn2_pool.tile([2, D], F32, name="qn2h")
    nc.vector.tensor_scalar(
        out=qn2h[:2, :], in0=qih[:2, :], scalar1=mv[:2, 0:1],
        scalar2=rstd[:2, 0:1], op0=ALU.subtract, op1=ALU.mult,
    )
    for j in range(8):
        tp = pp_f.tile([128, 128], F32, tag="tpf")
        nc.tensor.transpose(tp[:128, :2], qn2h[:2, j * 128:(j + 1) * 128], ident_f[:2, :2])
        evict_copy(qn2T[j][:, RO:RO + 2], tp[:128, :2])
    f_stack.close()

    # Wup (fp8 DoubleRow, K=256 per matmul) + SwiGLU + depthwise conv, in
    # 512-col superchunks; Wdown's first column-half rides along, one inner
    # pair behind the conv. Scales: wup carries x32, wdown x64 (host side);
    # the 1/32 descale folds into the Silu input scale / U bias / conv taps,
    # the 1/64 into the output eviction.
    f2_stack = ExitStack()
    pp_d = f2_stack.enter_context(tc.tile_pool(name="pp_d", bufs=1, space="PSUM"))
    ffn_stack = ExitStack()
    gu_stack = ExitStack()
    pp_g = gu_stack.enter_context(tc.tile_pool(name="pp_g", bufs=2, space="PSUM"))
    pp_u = gu_stack.enter_context(tc.tile_pool(name="pp_u", bufs=2, space="PSUM"))
    wupu_pool = ffn_stack.enter_context(tc.tile_pool(name="wupu", bufs=12))
    fsm_pool = ffn_stack.enter_context(tc.tile_pool(name="fsm", bufs=3))
    bias_pool = ffn_stack.enter_context(tc.tile_pool(name="bias", bufs=6))
    wd0_pool = ffn_stack.enter_context(tc.tile_pool(name="wd0", bufs=4))
    wd1_pool = ffn_stack.enter_context(tc.tile_pool(name="wd1", bufs=16))

    psd0 = [pp_d.tile([128, 512], F32, name=f"psd0_{i}") for i in range(4)]
    # hfc: fp8 DoubleRow pair tiles; pair t holds inner blocks (2t, 2t+1)
    for t in range(16):
        hfc.append(hfc_pool.tile([128, 2, RO], F8, name=f"hfc{t}"))
    wd0_sb = {}

    def emit_wdown_pair(t):
        wd_t = wd0_sb.pop(t)
        for i in range(4):
            nc.tensor.matmul(
                psd0[i][:], hfc[t][:, :, i * 128:(i + 1) * 128],
                wd_t[:, :, :], start=(t == 0), stop=(t == 15), perf_mode=DR,
            )

    wd1_sb = []
    for sc in range(8):
        if sc == 0:
            wupg_sb = wupg_pf
        else:
            wupg_sb = []
            for k in range(8):
                wg = wup_pool.tile([128, 512], BF16, tag="wupg")
                nc.sync.dma_start(
                    out=wg[:], in_=p_wup[k * 128:(k + 1) * 128, sc * 512:(sc + 1) * 512]
                )
                wupg_sb.append(wg)
        wupu_sb = []
        for k in range(8):
            wu = wupu_pool.tile([128, 512], BF16, tag="wupu")
            nc.scalar.dma_start(
                out=wu[:], in_=p_wup[k * 128:(k + 1) * 128, INNER + sc * 512:INNER + (sc + 1) * 512]
            )
            wupu_sb.append(wu)
        for c in range(4):
            cc = sc * 4 + c
            if cc % 2 == 0:
                tn = cc // 2
                wd_t = wd0_pool.tile([128, 2, 512], F8, tag="wd0")
                nc.gpsimd.dma_start(out=wd_t[:, :, :], in_=p_wd08[tn * 128:(tn + 1) * 128, :])
                wd0_sb[tn] = wd_t
                # prefetch the matching second-half tile for the tail phase
                wd1_t = wd1_pool.tile([128, 2, 512], F8, tag="wd1")
                nc.gpsimd.dma_start(out=wd1_t[:, :, :], in_=p_wd18[tn * 128:(tn + 1) * 128, :])
                wd1_sb.append(wd1_t)
            bg = bias_pool.tile([128, 1], F32, tag="bg")
            nc.gpsimd.dma_start(out=bg[:], in_=bass.AP(tensor=p_bgu, offset=cc * 128, ap=[[1, 128], [1, 1]]))
            bu = bias_pool.tile([128, 1], F32, tag="bu")
            nc.gpsimd.dma_start(out=bu[:], in_=bass.AP(tensor=p_bgu, offset=INNER + cc * 128, ap=[[1, 128], [1, 1]]))
            cw = bias_pool.tile([128, 3], F32, tag="cw")
            nc.gpsimd.dma_start(out=cw[:], in_=p_cw[cc * 128:(cc + 1) * 128, :])

            gact = fsm_pool.tile([128, RO + 2], F32, tag="gact")
            hf = fsm_pool.tile([128, RO + 2], F32, tag="hf")
            for h0, w in HALVES:
                psg = pp_g.tile([128, 258], F32, tag="psg")
                for k in range(8):
                    nc.tensor.matmul(
                        psg[:, :w], wupg_sb[k][:, c * 128:(c + 1) * 128],
                        qn2T[k][:, h0:h0 + w], start=(k == 0), stop=(k == 7),
                    )
                nc.scalar.activation(gact[:, h0:h0 + w], psg[:, :w], AF.Silu, bias=bg[:, 0:1])
                psu = pp_u.tile([128, 258], F32, tag="psu")
                for k in range(8):
                    nc.tensor.matmul(
                        psu[:, :w], wupu_sb[k][:, c * 128:(c + 1) * 128],
                        qn2T[k][:, h0:h0 + w], start=(k == 0), stop=(k == 7),
                    )
                nc.vector.scalar_tensor_tensor(
                    out=hf[:, h0:h0 + w], in0=psu[:, :w], scalar=bu[:, 0:1],
                    in1=gact[:, h0:h0 + w], op0=ALU.add, op1=ALU.mult,
                )
            if cc >= 2 and cc % 2 == 0:
                emit_wdown_pair(cc // 2 - 1)
            # mask halo cols at batch edges (conv zero-pad)
            nc.vector.tensor_scalar_mul(hf[:, RO:RO + 1], hf[:, RO:RO + 1], mask_p[:, 0:1])
            nc.vector.tensor_scalar_mul(hf[:, RO + 1:RO + 2], hf[:, RO + 1:RO + 2], mask_n[:, 0:1])
            # depthwise conv along rows: cols 0..511 owned, 512=prev, 513=next.
            # center tap on ACT (scale is per-partition), side taps DVE; the
            # final two taps write the fp8 DoubleRow pair tile directly.
            hfb = fsm_pool.tile([128, RO], BF16, tag="hfb")
            dst = hfc[cc // 2]
            jj = cc % 2
            nc.scalar.activation(hfb[:, 0:RO], hf[:, 0:RO], AF.Copy, scale=cw[:, 1:2])
            nc.vector.scalar_tensor_tensor(
                out=hfb[:, 1:RO], in0=hf[:, 0:RO - 1], scalar=cw[:, 0:1],
                in1=hfb[:, 1:RO], op0=ALU.mult, op1=ALU.add,
            )
            nc.vector.scalar_tensor_tensor(
                out=hfb[:, 0:1], in0=hf[:, RO:RO + 1], scalar=cw[:, 0:1],
                in1=hfb[:, 0:1], op0=ALU.mult, op1=ALU.add,
            )
            nc.vector.scalar_tensor_tensor(
                out=dst[:, jj, 0:RO - 1], in0=hf[:, 1:RO], scalar=cw[:, 2:3],
                in1=hfb[:, 0:RO - 1], op0=ALU.mult, op1=ALU.add,
            )
            nc.vector.scalar_tensor_tensor(
                out=dst[:, jj, RO - 1:RO], in0=hf[:, RO + 1:RO + 2], scalar=cw[:, 2:3],
                in1=hfb[:, RO - 1:RO], op0=ALU.mult, op1=ALU.add,
            )
    emit_wdown_pair(15)
    gu_stack.close()

    # ---------------- Phase F4: Wdown second half + residual + output ----------
    pp_d1 = f2_stack.enter_context(tc.tile_pool(name="pp_d1", bufs=1, space="PSUM"))
    psd1 = [pp_d1.tile([128, 512], F32, name=f"psd1_{i}") for i in range(4)]
    for t in range(16):
        for i in range(4):
            nc.tensor.matmul(
                psd1[i][:], hfc[t][:, :, i * 128:(i + 1) * 128],
                wd1_sb[t][:, :, :], start=(t == 0), stop=(t == 15), perf_mode=DR,
            )
    ffn_stack.close()
    out_pool = f2_stack.enter_context(tc.tile_pool(name="outp", bufs=4))
    for i in range(4):
        o_t = out_pool.tile([128, D], F32, tag="osb")
        nc.vector.scalar_tensor_tensor(
            out=o_t[:, 0:512], in0=psd0[i][:], scalar=WDINV,
            in1=qint[i][:, 0:512], op0=ALU.mult, op1=ALU.add,
        )
        nc.vector.scalar_tensor_tensor(
            out=o_t[:, 512:1024], in0=psd1[i][:], scalar=WDINV,
            in1=qint[i][:, 512:1024], op0=ALU.mult, op1=ALU.add,
        )
        out_q = (nc.sync, nc.scalar, nc.gpsimd, nc.sync)[i]
        out_q.dma_start(out=p_out[i * 128:(i + 1) * 128, :], in_=o_t[:])
    f2_stack.close()
    f34_stack.close()


def kernel(**inputs) -> np.ndarray:
    Q_in = np.ascontiguousarray(np.asarray(inputs["Q_in"], dtype=np.float32))
    X = np.ascontiguousarray(np.asarray(inputs["X"], dtype=np.float32))
    Wq = np.asarray(inputs["Wq"], dtype=np.float32)
    Wk = np.asarray(inputs["Wk"], dtype=np.float32)
    Wv = np.asarray(inputs["Wv"], dtype=np.float32)
    Wo = np.asarray(inputs["Wo"], dtype=np.float32)
    Wup = np.asarray(inputs["Wup"], dtype=np.float32)
    conv_w = np.asarray(inputs["conv_w"], dtype=np.float32)
    Wdown = np.asarray(inputs["Wdown"], dtype=np.float32)
    g1 = np.asarray(inputs["g1"], dtype=np.float32)
    b1 = np.asarray(inputs["b1"], dtype=np.float32)
    g2 = np.asarray(inputs["g2"], dtype=np.float32)
    b2 = np.asarray(inputs["b2"], dtype=np.float32)
    dt = float(np.asarray(inputs["dt"], dtype=np.float32))

    # softplus(dt) on host; baked into the NEFF as an immediate
    dt_safe = float(np.log1p(np.exp(dt)))

    # fold g2/b2 into Wup (LN2's affine commutes into the up-projection)
    wup_f = g2[:, None] * Wup
    bias_gu = np.ascontiguousarray(b2 @ Wup)
    cw3 = np.ascontiguousarray(conv_w[:, 0, :])

    key = round(dt_safe, 9)
    if key not in _cache:
        _cache[key] = _build(dt_safe)
    nc = _cache[key]

    bf = ml_dtypes.bfloat16
    f8 = ml_dtypes.float8_e4m3
    wq_b = np.ascontiguousarray(Wq.astype(bf))
    wk_b = np.ascontiguousarray(Wk.astype(bf))
    wv_b = np.ascontiguousarray(Wv.astype(bf))
    wo_b = np.ascontiguousarray(Wo.astype(bf))
    wup_b = np.ascontiguousarray(wup_f.astype(bf))

    # Wdown DoubleRow pair packing: tile-row t*128+r holds inner element
    # i = 128*(2t+j)+r in slot j; cols are (j, c).
    def pack_down(w):  # [INNER, 512] -> [INNER//2, D] fp8 pair layout
        t = w.reshape(16, 2, 128, 512)
        return np.ascontiguousarray(
            t.transpose(0, 2, 1, 3).reshape(INNER // 2, D).astype(f8))

    wd08 = pack_down(WDS * Wdown[:, 0:512])
    wd18 = pack_down(WDS * Wdown[:, 512:1024])

    in_maps = []
    for core in range(8):
        b, q = divmod(core, 4)
        qin = np.ascontiguousarray(Q_in[b, q * RO:(q + 1) * RO])
        xb1 = np.ascontiguousarray(X[b, q * RO:(q + 1) * RO] + b1[None, :])
        masks = np.array(
            [1.0 if q > 0 else 0.0, 1.0 if q < NQ - 1 else 0.0], dtype=np.float32
        )
        # maskmat.T @ gathered_boundaries = [prev-halo row; next-halo row]
        mm = np.zeros((2 * NQ, 2), dtype=np.float32)
        if q > 0:
            mm[2 * (q - 1) + 1, 0] = 1.0
        if q < NQ - 1:
            mm[2 * (q + 1), 1] = 1.0
        in_maps.append({
            "q_in": qin, "x_b1": xb1, "wq": wq_b, "wk": wk_b, "wv": wv_b,
            "wo": wo_b, "wup": wup_b, "bias_gu": bias_gu,
            "wd08": wd08, "wd18": wd18, "cw3": cw3,
            "g1": np.ascontiguousarray(g1), "masks": masks, "maskmat": mm,
        })

    global _last_in_maps
    _last_in_maps = in_maps
    res = run_bass_kernel_spmd(nc, in_maps, core_ids=list(range(8)))

    out = np.empty((B, N, D), dtype=np.float32)
    for core in range(8):
        b, q = divmod(core, 4)
        out[b, q * RO:(q + 1) * RO] = res.results[core]["out"]
    return out



# revision 70
# speedup vs baseline: 1.3546x; 1.0142x over previous
"""Distributed Trainium2 Bass kernel for nn_AMK_Block (kernelized-attention + ConvSwiGLU).

Sharding: sequence-parallel. Each of the 8 cores owns (batch b, query-row block q):
core = b*4 + q, rows q*512..q*512+511 of batch b, ALL heads. Each core computes
Q/K/V projections for its rows, AllGathers PhiK^T and V(+ones) across the 4 cores
of its batch group (fp8), then computes its 512 rows of attention, Wo, LN2 and
the full FFN locally. The depthwise-conv halo rows of Q_interact come from a tiny
third AllGather of boundary rows, extracted rank-agnostically with a mask-matrix
matmul. Weight matmuls run in bf16 (fp32 PSUM accumulation); the attention
kernel-matrix matmuls run in fp8 (PhiQ/PhiK/V/W^2 evicted as e4m3, Attr uses
DoubleRow packed k-pairs); norm/statistics in fp32. Wdown's first column-half is
interleaved into the FFN chunk stream so only half remains as a tail.
"""

import sys

sys.path.insert(0, "/opt/trn_rl_repo")

from contextlib import ExitStack

import ml_dtypes
import numpy as np

import concourse.bass as bass
import concourse.tile as tile
from concourse import bacc, mybir
from concourse.bass_utils import run_bass_kernel_spmd
from concourse.masks import make_identity

F32 = mybir.dt.float32
F32R = mybir.dt.float32r
BF16 = mybir.dt.bfloat16
F8 = mybir.dt.float8e4
ALU = mybir.AluOpType
AF = mybir.ActivationFunctionType
DR = mybir.MatmulPerfMode.DoubleRow

B, N, D = 2, 2048, 1024
H, DH = 16, 64
INNER = 4096
LN_EPS = 1e-5
WSC = 1.0 / 32.0          # W^2 is evicted as (W/32)^2 = W^2/1024 in fp8
WSC2 = WSC * WSC
UPS = 32.0                # Wup fp8 host scale (values ~N(0,1/32) -> ~N(0,1))
UPSC = 1.0 / UPS
WDS = 64.0                # Wdown fp8 host scale
WDINV = 1.0 / WDS

RO = 512          # owned rows per core
NQ = 4            # cores per batch group
GROUPS = [[0, 1, 2, 3], [4, 5, 6, 7]]
CHUNKS = [(0, 128), (128, 128), (256, 128), (384, 128)]
HALVES = [(0, 258), (258, 256)]  # even halves of 514; halo cols 512/513 in 2nd

_cache: dict[float, object] = {}
_last_in_maps: list | None = None


def _build(dt_safe: float):
    nc = bacc.Bacc("TRN2", target_bir_lowering=False, debug=False, num_devices=8)

    # ---------------- DRAM parameters (per-core shapes) ----------------
    p_qin = nc.declare_dram_parameter("q_in", [RO, D], F32, isOutput=False)
    p_xb1 = nc.declare_dram_parameter("x_b1", [RO, D], F32, isOutput=False)
    p_wq = nc.declare_dram_parameter("wq", [D, D], BF16, isOutput=False)
    p_wk = nc.declare_dram_parameter("wk", [D, D], BF16, isOutput=False)
    p_wv = nc.declare_dram_parameter("wv", [D, D], BF16, isOutput=False)
    p_wo = nc.declare_dram_parameter("wo", [D, D], BF16, isOutput=False)
    p_wup = nc.declare_dram_parameter("wup", [D, 2 * INNER], BF16, isOutput=False)
    p_bgu = nc.declare_dram_parameter("bias_gu", [2 * INNER], F32, isOutput=False)
    p_wd08 = nc.declare_dram_parameter("wd08", [INNER // 2, D], F8, isOutput=False)
    p_wd18 = nc.declare_dram_parameter("wd18", [INNER // 2, D], F8, isOutput=False)
    p_cw = nc.declare_dram_parameter("cw3", [INNER, 3], F32, isOutput=False)
    p_g1 = nc.declare_dram_parameter("g1", [D], F32, isOutput=False)
    p_mask = nc.declare_dram_parameter("masks", [2], F32, isOutput=False)
    p_mm = nc.declare_dram_parameter("maskmat", [2 * NQ, 2], F32R, isOutput=False)
    p_out = nc.declare_dram_parameter("out", [RO, D], F32, isOutput=True)

    with tile.TileContext(nc) as tc:
        build_ctx = ExitStack()
        with build_ctx:
            _emit(nc, tc, build_ctx, dt_safe, p_qin, p_xb1, p_wq, p_wk, p_wv,
                  p_wo, p_wup, p_bgu, p_wd08, p_wd18, p_cw, p_g1,
                  p_mask, p_mm, p_out)
    nc.finalize()
    return nc


def _emit(nc, tc, bctx, dt_safe, p_qin, p_xb1, p_wq, p_wk, p_wv, p_wo, p_wup,
          p_bgu, p_wd08, p_wd18, p_cw, p_g1, p_mask, p_mm, p_out):
    # ---------------- constant tiles (gpsimd queue: off the qin path) ------
    consts = bctx.enter_context(tc.tile_pool(name="consts", bufs=1))
    g1b = consts.tile([128, D], F32, name="g1b")
    nc.gpsimd.dma_start(
        out=g1b[:],
        in_=bass.AP(tensor=p_g1, offset=0, ap=[[0, 128], [1, D]]),
    )
    ident_f = consts.tile([128, 128], F32, name="ident_f")
    make_identity(nc, ident_f[:])
    ident_r = consts.tile([128, 128], F32R, name="ident_r")
    nc.gpsimd.dma_start(out=ident_r[:], in_=ident_f[:])
    mask_p = consts.tile([128, 1], F32, name="mask_p")
    nc.gpsimd.dma_start(out=mask_p[:], in_=bass.AP(tensor=p_mask, offset=0, ap=[[0, 128], [1, 1]]))
    mask_n = consts.tile([128, 1], F32, name="mask_n")
    nc.gpsimd.dma_start(out=mask_n[:], in_=bass.AP(tensor=p_mask, offset=1, ap=[[0, 128], [1, 1]]))
    ones_col = consts.tile([128, 1], F8, name="ones_col")
    nc.vector.memset(ones_col[:], 1.0)
    eps_t = consts.tile([128, 1], F32, name="eps_t")
    nc.vector.memset(eps_t[:], LN_EPS)
    maskmat = consts.tile([2 * NQ, 2], F32R, name="maskmat")
    nc.gpsimd.dma_start(out=maskmat[:], in_=p_mm[:, :])

    # DRAM scratch for the collectives (fp8 payloads)
    dram = bctx.enter_context(tc.tile_pool(name="dram", bufs=1, space="DRAM"))
    kag_in = dram.tile([H * DH, RO], F8, name="kag_in")        # PhiK^T local slice
    kag1 = dram.tile([NQ * 512, RO], F8, name="kag1")          # gathered heads 0-7
    kag2 = dram.tile([NQ * 512, RO], F8, name="kag2")          # gathered heads 8-15
    vag_in = dram.tile([RO, H * 66], F8, name="vag_in")        # V(+ones) local rows
    vag1 = dram.tile([NQ * 256, H * 66], F8, name="vag1")      # gathered rows 0-255
    vag2 = dram.tile([NQ * 256, H * 66], F8, name="vag2")      # gathered rows 256-511
    hag_in = dram.tile([2, D], F32R, name="hag_in")            # my boundary Qint rows
    hag = dram.tile([2 * NQ, D], F32R, name="hag")             # gathered boundaries

    ev_state = [0]

    def evict_copy(dst_ap, src_ap):
        ev_state[0] += 1
        if ev_state[0] % 2 == 0:
            nc.vector.tensor_copy(dst_ap, src_ap)
        else:
            nc.scalar.activation(dst_ap, src_ap, AF.Copy)

    ln_pool = bctx.enter_context(tc.tile_pool(name="ln", bufs=3))

    def layernorm_to(x_ap, p):
        """Returns (mv, rstd) tiles: mean in mv[:,0:1], rstd [p,1], for x_ap [p, D]."""
        st = ln_pool.tile([128, 2, 6], F32, tag="bn_st")
        xr = x_ap.rearrange("p (s f) -> p s f", s=2)
        for s in range(2):
            nc.vector.bn_stats(st[:p, s, :], xr[:, s, :])
        mv = ln_pool.tile([128, 2], F32, tag="bn_mv")
        nc.vector.bn_aggr(mv[:p], st[:p])
        rstd = ln_pool.tile([128, 1], F32, tag="bn_rstd")
        nc.scalar.activation(rstd[:p], mv[:p, 1:2], AF.Sqrt, bias=eps_t[:p, 0:1])
        nc.vector.reciprocal(rstd[:p], rstd[:p])
        return mv, rstd

    # ---- lifetime stacks (must nest LIFO): f34 > av2 > av > phase stacks ----
    f34_stack = ExitStack()   # hfc + qint: from Wo until the end
    av2_stack = ExitStack()   # mTc: until end of Wo
    av_stack = ExitStack()    # vT, phiQ: until end of head loop
    hfc_pool = f34_stack.enter_context(tc.tile_pool(name="hfc", bufs=1))
    qint_pool = f34_stack.enter_context(tc.tile_pool(name="qint", bufs=1))
    wup_pool = f34_stack.enter_context(tc.tile_pool(name="wup", bufs=12))
    mTc_pool = av2_stack.enter_context(tc.tile_pool(name="mTc", bufs=1))
    vT_pool = av_stack.enter_context(tc.tile_pool(name="vT", bufs=1))
    phiQ_pool = av_stack.enter_context(tc.tile_pool(name="phiQ", bufs=1))
    mTc = [mTc_pool.tile([128, RO], BF16, name=f"mTc{j}") for j in range(8)]
    vT = [vT_pool.tile([128, RO], F32R, name=f"vT{j}") for j in range(8)]
    phiQT = [phiQ_pool.tile([128, RO], F8, name=f"phiQT{j}") for j in range(8)]

    # ---------------- Phase P: LN1 + Hc + transposes ----------------
    p_stack = ExitStack()
    hcT_pool = p_stack.enter_context(tc.tile_pool(name="hcT", bufs=1))
    hcT = [hcT_pool.tile([128, RO], BF16, name=f"hcT{j}") for j in range(8)]
    io_pool = p_stack.enter_context(tc.tile_pool(name="io", bufs=4))
    hc_pool = p_stack.enter_context(tc.tile_pool(name="hc", bufs=2))
    # the PSUM phases of P (hc transposes / projections / V transposes) are
    # disjoint in time, so each gets its own short-lived 4-6 buffer pool
    t_stack = ExitStack()
    pp_t = t_stack.enter_context(tc.tile_pool(name="pp_t", bufs=4, space="PSUM"))

    # stream all qin/xb1 chunks up front; chunk 0 is the critical path, so
    # its rows are split across both hardware DMA queues
    qin_ts, xb1_ts = [], []
    for i, (r0, p) in enumerate(CHUNKS):
        qin_t = io_pool.tile([p, D], F32, tag="qin")
        if i == 0:
            h = p // 2
            nc.sync.dma_start(out=qin_t[0:h, :], in_=p_qin[r0:r0 + h, :])
            nc.scalar.dma_start(out=qin_t[h:p, :], in_=p_qin[r0 + h:r0 + p, :])
        else:
            nc.sync.dma_start(out=qin_t[:], in_=p_qin[r0:r0 + p, :])
        qin_ts.append(qin_t)
        xb1_t = io_pool.tile([p, D], F32, tag="xb1")
        nc.scalar.dma_start(out=xb1_t[:], in_=p_xb1[r0:r0 + p, :])
        xb1_ts.append(xb1_t)

    for i, (r0, p) in enumerate(CHUNKS):
        qin_t, xb1_t = qin_ts[i], xb1_ts[i]
        mv, rstd = layernorm_to(qin_t[:p, :], p)
        hc_t = hc_pool.tile([p, D], F32, tag="hc")
        nc.vector.tensor_scalar(
            out=hc_t[:p, :], in0=qin_t[:p, :], scalar1=mv[:p, 0:1],
            scalar2=rstd[:p, 0:1], op0=ALU.subtract, op1=ALU.mult,
        )
        nc.vector.tensor_mul(hc_t[:p, :], hc_t[:p, :], g1b[:p, :])
        nc.vector.tensor_add(hc_t[:p, :], hc_t[:p, :], xb1_t[:p, :])

        # transpose this row-chunk into the 8 hcT column tiles
        for j in range(8):
            tp = pp_t.tile([128, 128], F32, tag="tp")
            nc.tensor.transpose(tp[:128, :p], hc_t[:p, j * 128:(j + 1) * 128], ident_f[:p, :p])
            evict_copy(hcT[j][:, r0:r0 + p], tp[:128, :p])
    t_stack.close()

    # ---------------- Phase P: projections ----------------
    # Order: K -> K-AllGather (smallest latency to first collective), then Q
    # (needed with K for the W matmuls), then V -> V-AllGather. The rings
    # serialize on the collective lane, so K's goes first.
    # wk/wv/wq all live at once (wv is re-read by the late V^T pass)
    wstream = p_stack.enter_context(tc.tile_pool(name="wstream", bufs=24))
    pr_stack = ExitStack()
    pp_a = pr_stack.enter_context(tc.tile_pool(name="pp_a", bufs=6, space="PSUM"))
    elu_pool = p_stack.enter_context(tc.tile_pool(name="elu", bufs=3))

    def elu1_evict(dst_ap, src_psum_ap, p, w):
        """dst = elu(src)+1 = relu(src) + exp(min(src,0)) (fp8 out)"""
        tmin = elu_pool.tile([128, 512], F32, tag="tmin")
        nc.vector.tensor_scalar_min(tmin[:p, :w], src_psum_ap, 0.0)
        texp = elu_pool.tile([128, 512], F32, tag="texp")
        nc.scalar.activation(texp[:p, :w], tmin[:p, :w], AF.Exp)
        nc.vector.scalar_tensor_tensor(
            out=dst_ap, in0=src_psum_ap, scalar=0.0, in1=texp[:p, :w],
            op0=ALU.max, op1=ALU.add,
        )

    # K^T -> PhiK^T (fp8) -> kag_in; two chunked AllGathers (heads 0-7, 8-15)
    wk_sb = []
    for k in range(8):
        w_t = wstream.tile([128, D], BF16, tag="wproj")
        nc.sync.dma_start(out=w_t[:], in_=p_wk[k * 128:(k + 1) * 128, :])
        wk_sb.append(w_t)
    phiK_pool = p_stack.enter_context(tc.tile_pool(name="phiK", bufs=4))
    for j in range(8):
        ps = pp_a.tile([128, 512], F32, tag="proj")
        for k in range(8):
            nc.tensor.matmul(
                ps[:], wk_sb[k][:, j * 128:(j + 1) * 128],
                hcT[k][:, 0:RO], start=(k == 0), stop=(k == 7),
            )
        phiK_t = phiK_pool.tile([128, RO], F8, tag="phiK")
        elu1_evict(phiK_t[:, :], ps[:], 128, RO)
        nc.sync.dma_start(out=kag_in[j * 128:(j + 1) * 128, :], in_=phiK_t[:])
        if j == 3:
            nc.gpsimd.collective_compute(
                "AllGather", ALU.bypass, replica_groups=GROUPS,
                ins=[kag_in[0:512, :].opt()], outs=[kag1[:].opt()],
            )
    nc.gpsimd.collective_compute(
        "AllGather", ALU.bypass, replica_groups=GROUPS,
        ins=[kag_in[512:1024, :].opt()], outs=[kag2[:].opt()],
    )

    # V in ROW-major, straight into the AllGather staging tiles: stationary
    # hcT row-block, moving Wv. One strided eviction per [128, 512] psum
    # lands 8 heads' 64-wide slices; no PE transposes, and the V AllGathers
    # launch ~35us earlier than the old V^T+transpose pipeline.
    wv_sb = []
    for k in range(8):
        w_t = wstream.tile([128, D], BF16, tag="wproj")
        nc.scalar.dma_start(out=w_t[:], in_=p_wv[k * 128:(k + 1) * 128, :])
        wv_sb.append(w_t)
    vs_pool = p_stack.enter_context(tc.tile_pool(name="vs", bufs=3))
    for i in range(4):
        r0 = i * 128
        vstage = vs_pool.tile([128, H * 66], F8, tag="vstage")
        vsr = vstage[:].rearrange("p (h d) -> p h d", h=H)
        for half in range(2):
            ps = pp_a.tile([128, 512], F32, tag="proj")
            for k in range(8):
                nc.tensor.matmul(
                    ps[:], hcT[k][:, r0:r0 + 128],
                    wv_sb[k][:, half * 512:(half + 1) * 512],
                    start=(k == 0), stop=(k == 7),
                )
            # psum col c -> head (half*8 + c//64), dim c%64
            evict_copy(vsr[:, half * 8:half * 8 + 8, 0:64], ps[:])
        nc.vector.memset(vsr[:, :, 64:65], 1.0)
        nc.vector.memset(vsr[:, :, 65:66], 0.0)
        nc.gpsimd.dma_start(out=vag_in[r0:r0 + 128, :], in_=vstage[:])
        if i == 1:
            nc.gpsimd.collective_compute(
                "AllGather", ALU.bypass, replica_groups=GROUPS,
                ins=[vag_in[0:256, :].opt()], outs=[vag1[:].opt()],
            )
    nc.gpsimd.collective_compute(
        "AllGather", ALU.bypass, replica_groups=GROUPS,
        ins=[vag_in[256:512, :].opt()], outs=[vag2[:].opt()],
    )

    # Q^T -> PhiQ^T (fp8, kept in SBUF)
    wq_sb = []
    for k in range(8):
        w_t = wstream.tile([128, D], BF16, tag="wproj")
        nc.scalar.dma_start(out=w_t[:], in_=p_wq[k * 128:(k + 1) * 128, :])
        wq_sb.append(w_t)
    for j in range(8):
        ps = pp_a.tile([128, 512], F32, tag="proj")
        for k in range(8):
            nc.tensor.matmul(
                ps[:], wq_sb[k][:, j * 128:(j + 1) * 128],
                hcT[k][:, 0:RO], start=(k == 0), stop=(k == 7),
            )
        elu1_evict(phiQT[j][:, :], ps[:], 128, RO)

    # V^T (only the attention-post subtraction needs it)
    for j in range(8):
        ps = pp_a.tile([128, 512], F32, tag="proj")
        for k in range(8):
            nc.tensor.matmul(
                ps[:], wv_sb[k][:, j * 128:(j + 1) * 128],
                hcT[k][:, 0:RO], start=(k == 0), stop=(k == 7),
            )
        evict_copy(vT[j][:, :], ps[:])
    pr_stack.close()

    p_stack.close()

    # ---------------- Phase A: attention ----------------
    # Per head-group g (4 heads): W(g) = 64 fp8 matmuls (K=64) evicted as
    # (W/32)^2 fp8 into DoubleRow pair tiles; Attr(g) = per head 8 fp8-DR
    # matmuls over (m-block pair, key) tiles. Emission order W0 W1 A0 W2 A1
    # W3 A2 A3 keeps the PE busy while the V AllGathers land.
    # Wo weights: pool created first (released after attention pools), loads
    # issued now so the Wo phase starts instantly
    wo_stack = ExitStack()
    wo_pool = wo_stack.enter_context(tc.tile_pool(name="wo", bufs=8))
    wo_sb = []
    for k in range(8):
        w_t = wo_pool.tile([128, D], BF16, tag="wo")
        nc.scalar.dma_start(out=w_t[:], in_=p_wo[k * 128:(k + 1) * 128, :])
        wo_sb.append(w_t)
    # prefetch the first FFN superchunk's gate weights during attention
    wupg_pf = []
    for k in range(8):
        wg = wup_pool.tile([128, 512], BF16, tag="wupg")
        nc.sync.dma_start(out=wg[:], in_=p_wup[k * 128:(k + 1) * 128, 0:512])
        wupg_pf.append(wg)

    a_stack = ExitStack()
    kq_pool = a_stack.enter_context(tc.tile_pool(name="kq", bufs=3))
    vhd_pool = a_stack.enter_context(tc.tile_pool(name="vhd", bufs=16))
    # dual-fp8 LDWEIGHTS needs stationary width % 32 == 0: vhd is 96 wide
    # (V 0-63, ones 64, pad 65-95). DMA writes cols 0-65; zero the pad cols
    # once per pool buffer (round-robin reuse keeps them zero).
    for _ in range(16):
        vz = vhd_pool.tile([128, 8, 96], F8, tag="vhd")
        nc.vector.memset(vz[:, :, 66:96], 0.0)
    wt_pool = a_stack.enter_context(tc.tile_pool(name="wt", bufs=64))
    asm_pool = a_stack.enter_context(tc.tile_pool(name="asm", bufs=3))
    pp_w = a_stack.enter_context(tc.tile_pool(name="pp_w", bufs=4, space="PSUM"))
    pp_at = a_stack.enter_context(tc.tile_pool(name="pp_at", bufs=4, space="PSUM"))

    sq_state = [0]

    def square_evict(dst_ap, src_psum_ap):
        """dst = src^2 fp8; src is already W/32 (W > 0, relu is a no-op).
        Rotated 5:2 across ACT/DVE: ACT streams ~1 col/ns single-pass; the
        DVE two-pass path costs ~2x that, so it only soaks the overflow."""
        sq_state[0] = (sq_state[0] + 1) % 7
        if sq_state[0] < 5:
            nc.scalar.activation(dst_ap, src_psum_ap, AF.Square, scale=WSC)
        else:
            tr = asm_pool.tile([128, 512], BF16, tag="r2tmpv")
            nc.vector.tensor_scalar_mul(tr[:, :], src_psum_ap, WSC)
            nc.vector.tensor_mul(dst_ap, tr[:, :], tr[:, :])

    def emit_w(hg):
        """W^T for 4 heads of group hg -> wtp fp8 DoubleRow pair tiles.
        The two heads sharing a kq/phiQT tile (PE row halves 0-63 / 64-127)
        are interleaved: consecutive matmuls hit disjoint row groups, so
        LDWEIGHTS overlaps the in-flight matmul."""
        kag_t = kag1 if hg < 2 else kag2
        kq_sb = {}
        for j2 in (2 * hg, 2 * hg + 1):
            hrow = (j2 % 4) * 128  # row offset of head-pair j2 within kag_t
            kt = kq_pool.tile([128, NQ, RO], F8, tag="kq")
            ksrc = kag_t[:, :]
            nc.sync.dma_start(
                out=kt[:, :, :],
                in_=bass.AP(tensor=ksrc.tensor, offset=ksrc.offset + hrow * RO,
                            ap=[[RO, 128], [512 * RO, NQ], [1, RO]]),
            )
            kq_sb[j2] = kt
        wtp = {}
        for hh in range(4):
            h = hg * 4 + hh
            wtp[h] = [wt_pool.tile([128, 2, RO], F8, tag="wt", name=f"wt{h}_{t}")
                      for t in range(8)]
        for j2 in (2 * hg, 2 * hg + 1):
            hA, hB = 2 * j2, 2 * j2 + 1
            for m in range(16):
                qq, lc = m // 4, m % 4
                for off, h in ((0, hA), (64, hB)):
                    psw = pp_w.tile([128, 512], F32, tag="psw")
                    nc.tensor.matmul(
                        psw[:], kq_sb[j2][off:off + 64, qq, lc * 128:(lc + 1) * 128],
                        phiQT[j2][off:off + 64, :], start=True, stop=True,
                    )
                    square_evict(wtp[h][m // 2][:, m % 2, :], psw)
        return wtp

    # pair order follows the chunked V gathers: vag1 pairs (lc 0,1) first
    T_ORDER = [qq * 2 for qq in range(NQ)] + [qq * 2 + 1 for qq in range(NQ)]

    def emit_attr(hg, wtp):
        pats = []
        for hh in range(4):
            h = hg * 4 + hh
            pat = pp_at.tile([96, 512], F32, tag="pat", name=f"pat{h}")
            vh = {}
            for half in range(2):
                vsrc = vag1 if half == 0 else vag2
                vt = vhd_pool.tile([128, 8, 96], F8, tag="vhd")
                vap = vsrc[:, :]
                W16 = H * 66
                nc.sync.dma_start(
                    out=vt[:, :, 0:66],
                    in_=bass.AP(tensor=vap.tensor, offset=vap.offset + h * 66,
                                ap=[[W16, 128], [128 * W16, 8], [1, 66]]))
                vh[half] = vt
            for ti, t in enumerate(T_ORDER):
                qq, half = t // 2, t % 2
                nc.tensor.matmul(
                    pat[:], vh[half][:, 2 * qq:2 * qq + 2, :], wtp[h][t][:, :, :],
                    start=(ti == 0), stop=(ti == 7), perf_mode=DR,
                )
            pats.append(pat)
        for hh in range(4):
            h = hg * 4 + hh
            j2, off = h // 2, (h % 2) * 64
            nrm = asm_pool.tile([1, RO], F32, tag="nrm")
            nc.vector.tensor_scalar_add(nrm[0:1, :], pats[hh][64:65, :], WSC2)
            nc.vector.reciprocal_approx_fast(out=nrm[:], in_=nrm[:])
            rcb = asm_pool.tile([64, RO], F32, tag="rcb")
            nc.gpsimd.partition_broadcast(rcb[:], nrm[:])
            tm = asm_pool.tile([128, RO], F32, tag="tm")
            nc.vector.tensor_mul(tm[off:off + 64, :], pats[hh][0:64, :], rcb[:, :])
            nc.vector.tensor_sub(
                mTc[j2][off:off + 64, :], tm[off:off + 64, :],
                vT[j2][off:off + 64, :],
            )

    wtp_q = [emit_w(0), emit_w(1)]
    for hg in range(4):
        emit_attr(hg, wtp_q[hg])
        if hg + 2 < 4:
            wtp_q.append(emit_w(hg + 2))
    a_stack.close()

    # ---------------- Phase A5: Wo + Q_interact ----------------
    # Chunk order 0,3,1,2 so the conv-halo boundary rows exist after two
    # chunks and their AllGather overlaps the rest of Wo + LN2.
    a5_stack = ExitStack()
    qi_pool = a5_stack.enter_context(tc.tile_pool(name="qi", bufs=3))
    pp_o = a5_stack.enter_context(tc.tile_pool(name="pp_o", bufs=4, space="PSUM"))
    qint = [None] * 4
    for oi, i in enumerate((0, 3, 1, 2)):
        r0, p = CHUNKS[i]
        qin_t = qi_pool.tile([p, D], F32, tag="qin2")
        nc.sync.dma_start(out=qin_t[:], in_=p_qin[r0:r0 + p, :])
        qi = qint_pool.tile([p, D], F32, name=f"qint{i}")
        for half in range(2):
            pso = pp_o.tile([128, 512], F32, tag="pso")
            for k in range(8):
                nc.tensor.matmul(
                    pso[:p, :], mTc[k][:, r0:r0 + p],
                    wo_sb[k][:, half * 512:(half + 1) * 512],
                    start=(k == 0), stop=(k == 7),
                )
            nc.vector.scalar_tensor_tensor(
                out=qi[:p, half * 512:(half + 1) * 512], in0=pso[:p, :],
                scalar=dt_safe, in1=qin_t[:p, half * 512:(half + 1) * 512],
                op0=ALU.mult, op1=ALU.add,
            )
        qint[i] = qi
        if oi == 1:
            # boundary rows ready: stage + AllGather (conv halo exchange)
            nc.gpsimd.dma_start(out=hag_in[0:1, :], in_=qint[0][0:1, :])
            nc.gpsimd.dma_start(out=hag_in[1:2, :], in_=qint[3][127:128, :])
            nc.gpsimd.collective_compute(
                "AllGather", ALU.bypass, replica_groups=GROUPS,
                ins=[hag_in[:].opt()], outs=[hag[:].opt()],
            )
    a5_stack.close()
    wo_stack.close()
    av_stack.close()   # frees vT, phiQ
    av2_stack.close()  # frees mTc

    # ---------------- Phase F: LN2 + transpose + FFN ----------------
    hfc = []

    qn2T_pool = f34_stack.enter_context(tc.tile_pool(name="qn2T", bufs=1))
    f_stack = ExitStack()
    qn2_pool = f_stack.enter_context(tc.tile_pool(name="qn2", bufs=2))
    pp_f = f_stack.enter_context(tc.tile_pool(name="pp_f", bufs=4, space="PSUM"))
    # qn2T cols: 0..511 owned rows, 512 = prev-halo row, 513 = next-halo row
    qn2T = [qn2T_pool.tile([128, RO + 2], BF16, name=f"qn2T{j}") for j in range(8)]
    for i, (r0, p) in enumerate(CHUNKS):
        mv, rstd = layernorm_to(qint[i][:p, :], p)
        qn2_t = qn2_pool.tile([p, D], F32, tag="qn2")
        nc.vector.tensor_scalar(
            out=qn2_t[:p, :], in0=qint[i][:p, :], scalar1=mv[:p, 0:1],
            scalar2=rstd[:p, 0:1], op0=ALU.subtract, op1=ALU.mult,
        )
        for j in range(8):
            tp = pp_f.tile([128, 128], F32, tag="tpf")
            nc.tensor.transpose(tp[:128, :p], qn2_t[:p, j * 128:(j + 1) * 128], ident_f[:p, :p])
            evict_copy(qn2T[j][:, r0:r0 + p], tp[:128, :p])

    # halo rows: extract prev/next boundary rows via maskmat.T @ gathered,
    # then LN2 + transpose into qn2T cols 512/513
    pp_h = f_stack.enter_context(tc.tile_pool(name="pp_h", bufs=1, space="PSUM"))
    hg_sb = qn2_pool.tile([2 * NQ, D], F32R, name="hg_sb")
    nc.sync.dma_start(out=hg_sb[:], in_=hag[:, :])
    qih = qn2_pool.tile([2, D], F32, name="qih")
    for half in range(2):
        ph = pp_h.tile([2, 512], F32, tag="psh", name=f"ph{half}")
        nc.tensor.matmul(
            ph[:], maskmat[:], hg_sb[:, half * 512:(half + 1) * 512],
            start=True, stop=True,
        )
        nc.vector.tensor_copy(qih[:, half * 512:(half + 1) * 512], ph[:])
    mv, rstd = layernorm_to(qih[:2, :], 2)
    qn2h = qn2_pool.tile([2, D], F32, name="qn2h")
    nc.vector.tensor_scalar(
        out=qn2h[:2, :], in0=qih[:2, :], scalar1=mv[:2, 0:1],
        scalar2=rstd[:2, 0:1], op0=ALU.subtract, op1=ALU.mult,
    )
    for j in range(8):
        tp = pp_f.tile([128, 128], F32, tag="tpf")
        nc.tensor.transpose(tp[:128, :2], qn2h[:2, j * 128:(j + 1) * 128], ident_f[:2, :2])
        evict_copy(qn2T[j][:, RO:RO + 2], tp[:128, :2])
    f_stack.close()

    # Wup (fp8 DoubleRow, K=256 per matmul) + SwiGLU + depthwise conv, in
    # 512-col superchunks; Wdown's first column-half rides along, one inner
    # pair behind the conv. Scales: wup carries x32, wdown x64 (host side);
    # the 1/32 descale folds into the Silu input scale / U bias / conv taps,
    # the 1/64 into the output eviction.
    f2_stack = ExitStack()
    pp_d = f2_stack.enter_context(tc.tile_pool(name="pp_d", bufs=1, space="PSUM"))
    ffn_stack = ExitStack()
    gu_stack = ExitStack()
    pp_g = gu_stack.enter_context(tc.tile_pool(name="pp_g", bufs=2, space="PSUM"))
    pp_u = gu_stack.enter_context(tc.tile_pool(name="pp_u", bufs=2, space="PSUM"))
    wupu_pool = ffn_stack.enter_context(tc.tile_pool(name="wupu", bufs=12))
    fsm_pool = ffn_stack.enter_context(tc.tile_pool(name="fsm", bufs=3))
    bias_pool = ffn_stack.enter_context(tc.tile_pool(name="bias", bufs=6))
    wd0_pool = ffn_stack.enter_context(tc.tile_pool(name="wd0", bufs=4))
    wd1_pool = ffn_stack.enter_context(tc.tile_pool(name="wd1", bufs=16))

    psd0 = [pp_d.tile([128, 512], F32, name=f"psd0_{i}") for i in range(4)]
    # hfc: fp8 DoubleRow pair tiles; pair t holds inner blocks (2t, 2t+1)
    for t in range(16):
        hfc.append(hfc_pool.tile([128, 2, RO], F8, name=f"hfc{t}"))
    wd0_sb = {}

    def emit_wdown_pair(t):
        wd_t = wd0_sb.pop(t)
        for i in range(4):
            nc.tensor.matmul(
                psd0[i][:], hfc[t][:, :, i * 128:(i + 1) * 128],
                wd_t[:, :, :], start=(t == 0), stop=(t == 15), perf_mode=DR,
            )

    wd1_sb = []
    for sc in range(8):
        if sc == 0:
            wupg_sb = wupg_pf
        else:
            wupg_sb = []
            for k in range(8):
                wg = wup_pool.tile([128, 512], BF16, tag="wupg")
                nc.sync.dma_start(
                    out=wg[:], in_=p_wup[k * 128:(k + 1) * 128, sc * 512:(sc + 1) * 512]
                )
                wupg_sb.append(wg)
        wupu_sb = []
        for k in range(8):
            wu = wupu_pool.tile([128, 512], BF16, tag="wupu")
            nc.scalar.dma_start(
                out=wu[:], in_=p_wup[k * 128:(k + 1) * 128, INNER + sc * 512:INNER + (sc + 1) * 512]
            )
            wupu_sb.append(wu)
        for c in range(4):
            cc = sc * 4 + c
            if cc % 2 == 0:
                tn = cc // 2
                wd_t = wd0_pool.tile([128, 2, 512], F8, tag="wd0")
                nc.gpsimd.dma_start(out=wd_t[:, :, :], in_=p_wd08[tn * 128:(tn + 1) * 128, :])
                wd0_sb[tn] = wd_t
                # prefetch the matching second-half tile for the tail phase
                wd1_t = wd1_pool.tile([128, 2, 512], F8, tag="wd1")
                nc.gpsimd.dma_start(out=wd1_t[:, :, :], in_=p_wd18[tn * 128:(tn + 1) * 128, :])
                wd1_sb.append(wd1_t)
            bg = bias_pool.tile([128, 1], F32, tag="bg")
            nc.gpsimd.dma_start(out=bg[:], in_=bass.AP(tensor=p_bgu, offset=cc * 128, ap=[[1, 128], [1, 1]]))
            bu = bias_pool.tile([128, 1], F32, tag="bu")
            nc.gpsimd.dma_start(out=bu[:], in_=bass.AP(tensor=p_bgu, offset=INNER + cc * 128, ap=[[1, 128], [1, 1]]))
            cw = bias_pool.tile([128, 3], F32, tag="cw")
            nc.gpsimd.dma_start(out=cw[:], in_=p_cw[cc * 128:(cc + 1) * 128, :])

            gact = fsm_pool.tile([128, RO + 2], F32, tag="gact")
            hf = fsm_pool.tile([128, RO + 2], F32, tag="hf")
            for h0, w in HALVES:
                psg = pp_g.tile([128, 258], F32, tag="psg")
                for k in range(8):
                    nc.tensor.matmul(
                        psg[:, :w], wupg_sb[k][:, c * 128:(c + 1) * 128],
                        qn2T[k][:, h0:h0 + w], start=(k == 0), stop=(k == 7),
                    )
                nc.scalar.activation(gact[:, h0:h0 + w], psg[:, :w], AF.Silu, bias=bg[:, 0:1])
                psu = pp_u.tile([128, 258], F32, tag="psu")
                for k in range(8):
                    nc.tensor.matmul(
                        psu[:, :w], wupu_sb[k][:, c * 128:(c + 1) * 128],
                        qn2T[k][:, h0:h0 + w], start=(k == 0), stop=(k == 7),
                    )
                nc.vector.scalar_tensor_tensor(
                    out=hf[:, h0:h0 + w], in0=psu[:, :w], scalar=bu[:, 0:1],
                    in1=gact[:, h0:h0 + w], op0=ALU.add, op1=ALU.mult,
                )
            if cc >= 2 and cc % 2 == 0:
                emit_wdown_pair(cc // 2 - 1)
            # mask halo cols at batch edges (conv zero-pad)
            nc.vector.tensor_scalar_mul(hf[:, RO:RO + 1], hf[:, RO:RO + 1], mask_p[:, 0:1])
            nc.vector.tensor_scalar_mul(hf[:, RO + 1:RO + 2], hf[:, RO + 1:RO + 2], mask_n[:, 0:1])
            # depthwise conv along rows: cols 0..511 owned, 512=prev, 513=next.
            # center tap on ACT (scale is per-partition), side taps DVE; the
            # final two taps write the fp8 DoubleRow pair tile directly.
            hfb = fsm_pool.tile([128, RO], BF16, tag="hfb")
            dst = hfc[cc // 2]
            jj = cc % 2
            nc.scalar.activation(hfb[:, 0:RO], hf[:, 0:RO], AF.Copy, scale=cw[:, 1:2])
            nc.vector.scalar_tensor_tensor(
                out=hfb[:, 1:RO], in0=hf[:, 0:RO - 1], scalar=cw[:, 0:1],
                in1=hfb[:, 1:RO], op0=ALU.mult, op1=ALU.add,
            )
            nc.vector.scalar_tensor_tensor(
                out=hfb[:, 0:1], in0=hf[:, RO:RO + 1], scalar=cw[:, 0:1],
                in1=hfb[:, 0:1], op0=ALU.mult, op1=ALU.add,
            )
            nc.vector.scalar_tensor_tensor(
                out=dst[:, jj, 0:RO - 1], in0=hf[:, 1:RO], scalar=cw[:, 2:3],
                in1=hfb[:, 0:RO - 1], op0=ALU.mult, op1=ALU.add,
            )
            nc.vector.scalar_tensor_tensor(
                out=dst[:, jj, RO - 1:RO], in0=hf[:, RO + 1:RO + 2], scalar=cw[:, 2:3],
                in1=hfb[:, RO - 1:RO], op0=ALU.mult, op1=ALU.add,
            )
    emit_wdown_pair(15)
    gu_stack.close()

    # ---------------- Phase F4: Wdown second half + residual + output ----------
    pp_d1 = f2_stack.enter_context(tc.tile_pool(name="pp_d1", bufs=1, space="PSUM"))
    psd1 = [pp_d1.tile([128, 512], F32, name=f"psd1_{i}") for i in range(4)]
    for t in range(16):
        for i in range(4):
            nc.tensor.matmul(
                psd1[i][:], hfc[t][:, :, i * 128:(i + 1) * 128],
                wd1_sb[t][:, :, :], start=(t == 0), stop=(t == 15), perf_mode=DR,
            )
    ffn_stack.close()
    out_pool = f2_stack.enter_context(tc.tile_pool(name="outp", bufs=4))
    for i in range(4):
        o_t = out_pool.tile([128, D], F32, tag="osb")
        nc.vector.scalar_tensor_tensor(
            out=o_t[:, 0:512], in0=psd0[i][:], scalar=WDINV,
            in1=qint[i][:, 0:512], op0=ALU.mult, op1=ALU.add,
        )
        nc.vector.scalar_tensor_tensor(
            out=o_t[:, 512:1024], in0=psd1[i][:], scalar=WDINV,
            in1=qint[i][:, 512:1024], op0=ALU.mult, op1=ALU.add,
        )
        out_q = (nc.sync, nc.scalar, nc.gpsimd, nc.sync)[i]
        out_q.dma_start(out=p_out[i * 128:(i + 1) * 128, :], in_=o_t[:])
    f2_stack.close()
    f34_stack.close()


def kernel(**inputs) -> np.ndarray:
    Q_in = np.ascontiguousarray(np.asarray(inputs["Q_in"], dtype=np.float32))
    X = np.ascontiguousarray(np.asarray(inputs["X"], dtype=np.float32))
    Wq = np.asarray(inputs["Wq"], dtype=np.float32)
    Wk = np.asarray(inputs["Wk"], dtype=np.float32)
    Wv = np.asarray(inputs["Wv"], dtype=np.float32)
    Wo = np.asarray(inputs["Wo"], dtype=np.float32)
    Wup = np.asarray(inputs["Wup"], dtype=np.float32)
    conv_w = np.asarray(inputs["conv_w"], dtype=np.float32)
    Wdown = np.asarray(inputs["Wdown"], dtype=np.float32)
    g1 = np.asarray(inputs["g1"], dtype=np.float32)
    b1 = np.asarray(inputs["b1"], dtype=np.float32)
    g2 = np.asarray(inputs["g2"], dtype=np.float32)
    b2 = np.asarray(inputs["b2"], dtype=np.float32)
    dt = float(np.asarray(inputs["dt"], dtype=np.float32))

    # softplus(dt) on host; baked into the NEFF as an immediate
    dt_safe = float(np.log1p(np.exp(dt)))

    # fold g2/b2 into Wup (LN2's affine commutes into the up-projection)
    wup_f = g2[:, None] * Wup
    bias_gu = np.ascontiguousarray(b2 @ Wup)
    cw3 = np.ascontiguousarray(conv_w[:, 0, :])

    key = round(dt_safe, 9)
    if key not in _cache:
        _cache[key] = _build(dt_safe)
    nc = _cache[key]

    bf = ml_dtypes.bfloat16
    f8 = ml_dtypes.float8_e4m3
    wq_b = np.ascontiguousarray(Wq.astype(bf))
    wk_b = np.ascontiguousarray(Wk.astype(bf))
    wv_b = np.ascontiguousarray(Wv.astype(bf))
    wo_b = np.ascontiguousarray(Wo.astype(bf))
    wup_b = np.ascontiguousarray(wup_f.astype(bf))

    # Wdown DoubleRow pair packing: tile-row t*128+r holds inner element
    # i = 128*(2t+j)+r in slot j; cols are (j, c).
    def pack_down(w):  # [INNER, 512] -> [INNER//2, D] fp8 pair layout
        t = w.reshape(16, 2, 128, 512)
        return np.ascontiguousarray(
            t.transpose(0, 2, 1, 3).reshape(INNER // 2, D).astype(f8))

    wd08 = pack_down(WDS * Wdown[:, 0:512])
    wd18 = pack_down(WDS * Wdown[:, 512:1024])

    in_maps = []
    for core in range(8):
        b, q = divmod(core, 4)
        qin = np.ascontiguousarray(Q_in[b, q * RO:(q + 1) * RO])
        xb1 = np.ascontiguousarray(X[b, q * RO:(q + 1) * RO] + b1[None, :])
        masks = np.array(
            [1.0 if q > 0 else 0.0, 1.0 if q < NQ - 1 else 0.0], dtype=np.float32
        )
        # maskmat.T @ gathered_boundaries = [prev-halo row; next-halo row]
        mm = np.zeros((2 * NQ, 2), dtype=np.float32)
        if q > 0:
            mm[2 * (q - 1) + 1, 0] = 1.0
        if q < NQ - 1:
            mm[2 * (q + 1), 1] = 1.0
        in_maps.append({
            "q_in": qin, "x_b1": xb1, "wq": wq_b, "wk": wk_b, "wv": wv_b,
            "wo": wo_b, "wup": wup_b, "bias_gu": bias_gu,
            "wd08": wd08, "wd18": wd18, "cw3": cw3,
            "g1": np.ascontiguousarray(g1), "masks": masks, "maskmat": mm,
        })

    global _last_in_maps
    _last_in_maps = in_maps
    res = run_bass_kernel_spmd(nc, in_maps, core_ids=list(range(8)))

    out = np.empty((B, N, D), dtype=np.float32)
    for core in range(8):
        b, q = divmod(core, 4)
        out[b, q * RO:(q + 1) * RO] = res.results[core]["out"]
    return out



# revision 72
# speedup vs baseline: 1.3601x; 1.0041x over previous
"""Distributed Trainium2 Bass kernel for nn_AMK_Block (kernelized-attention + ConvSwiGLU).

Sharding: sequence-parallel. Each of the 8 cores owns (batch b, query-row block q):
core = b*4 + q, rows q*512..q*512+511 of batch b, ALL heads. Each core computes
Q/K/V projections for its rows, AllGathers PhiK^T and V(+ones) across the 4 cores
of its batch group (fp8), then computes its 512 rows of attention, Wo, LN2 and
the full FFN locally. The depthwise-conv halo rows of Q_interact come from a tiny
third AllGather of boundary rows, extracted rank-agnostically with a mask-matrix
matmul. Weight matmuls run in bf16 (fp32 PSUM accumulation); the attention
kernel-matrix matmuls run in fp8 (PhiQ/PhiK/V/W^2 evicted as e4m3, Attr uses
DoubleRow packed k-pairs); norm/statistics in fp32. Wdown's first column-half is
interleaved into the FFN chunk stream so only half remains as a tail.
"""

import sys

sys.path.insert(0, "/opt/trn_rl_repo")

from contextlib import ExitStack

import ml_dtypes
import numpy as np

import concourse.bass as bass
import concourse.tile as tile
from concourse import bacc, mybir
from concourse.bass_utils import run_bass_kernel_spmd
from concourse.masks import make_identity

F32 = mybir.dt.float32
F32R = mybir.dt.float32r
BF16 = mybir.dt.bfloat16
F8 = mybir.dt.float8e4
ALU = mybir.AluOpType
AF = mybir.ActivationFunctionType
DR = mybir.MatmulPerfMode.DoubleRow

B, N, D = 2, 2048, 1024
H, DH = 16, 64
INNER = 4096
LN_EPS = 1e-5
WSC = 1.0 / 32.0          # W^2 is evicted as (W/32)^2 = W^2/1024 in fp8
WSC2 = WSC * WSC
UPS = 32.0                # Wup fp8 host scale (values ~N(0,1/32) -> ~N(0,1))
UPSC = 1.0 / UPS
WDS = 64.0                # Wdown fp8 host scale
WDINV = 1.0 / WDS

RO = 512          # owned rows per core
NQ = 4            # cores per batch group
GROUPS = [[0, 1, 2, 3], [4, 5, 6, 7]]
CHUNKS = [(0, 128), (128, 128), (256, 128), (384, 128)]
HALVES = [(0, 258), (258, 256)]  # even halves of 514; halo cols 512/513 in 2nd

_cache: dict[float, object] = {}
_last_in_maps: list | None = None


def _build(dt_safe: float, g1_one: bool):
    nc = bacc.Bacc("TRN2", target_bir_lowering=False, debug=False, num_devices=8)

    # ---------------- DRAM parameters (per-core shapes) ----------------
    p_qin = nc.declare_dram_parameter("q_in", [RO, D], F32, isOutput=False)
    p_xb1 = nc.declare_dram_parameter("x_b1", [RO, D], F32, isOutput=False)
    p_wq = nc.declare_dram_parameter("wq", [D, D], BF16, isOutput=False)
    p_wk = nc.declare_dram_parameter("wk", [D, D], BF16, isOutput=False)
    p_wv = nc.declare_dram_parameter("wv", [D, D], BF16, isOutput=False)
    p_wo = nc.declare_dram_parameter("wo", [D, D], BF16, isOutput=False)
    p_wup = nc.declare_dram_parameter("wup", [D, 2 * INNER], BF16, isOutput=False)
    p_bgu = nc.declare_dram_parameter("bias_gu", [2 * INNER], F32, isOutput=False)
    p_wd08 = nc.declare_dram_parameter("wd08", [INNER // 2, D], F8, isOutput=False)
    p_wd18 = nc.declare_dram_parameter("wd18", [INNER // 2, D], F8, isOutput=False)
    p_cw = nc.declare_dram_parameter("cw3", [INNER, 3], F32, isOutput=False)
    p_g1 = nc.declare_dram_parameter("g1", [D], F32, isOutput=False)
    p_mask = nc.declare_dram_parameter("masks", [2], F32, isOutput=False)
    p_mm = nc.declare_dram_parameter("maskmat", [2 * NQ, 2], F32R, isOutput=False)
    p_out = nc.declare_dram_parameter("out", [RO, D], F32, isOutput=True)

    with tile.TileContext(nc) as tc:
        build_ctx = ExitStack()
        with build_ctx:
            _emit(nc, tc, build_ctx, dt_safe, g1_one, p_qin, p_xb1, p_wq,
                  p_wk, p_wv, p_wo, p_wup, p_bgu, p_wd08, p_wd18, p_cw, p_g1,
                  p_mask, p_mm, p_out)
    nc.finalize()
    return nc


def _emit(nc, tc, bctx, dt_safe, g1_one, p_qin, p_xb1, p_wq, p_wk, p_wv,
          p_wo, p_wup, p_bgu, p_wd08, p_wd18, p_cw, p_g1, p_mask, p_mm, p_out):
    # ---------------- constant tiles (gpsimd queue: off the qin path) ------
    consts = bctx.enter_context(tc.tile_pool(name="consts", bufs=1))
    g1b = consts.tile([128, D], F32, name="g1b")
    nc.gpsimd.dma_start(
        out=g1b[:],
        in_=bass.AP(tensor=p_g1, offset=0, ap=[[0, 128], [1, D]]),
    )
    ident_f = consts.tile([128, 128], F32, name="ident_f")
    make_identity(nc, ident_f[:])
    ident_r = consts.tile([128, 128], F32R, name="ident_r")
    nc.gpsimd.dma_start(out=ident_r[:], in_=ident_f[:])
    mask_p = consts.tile([128, 1], F32, name="mask_p")
    nc.gpsimd.dma_start(out=mask_p[:], in_=bass.AP(tensor=p_mask, offset=0, ap=[[0, 128], [1, 1]]))
    mask_n = consts.tile([128, 1], F32, name="mask_n")
    nc.gpsimd.dma_start(out=mask_n[:], in_=bass.AP(tensor=p_mask, offset=1, ap=[[0, 128], [1, 1]]))
    ones_col = consts.tile([128, 1], F8, name="ones_col")
    nc.vector.memset(ones_col[:], 1.0)
    eps_t = consts.tile([128, 1], F32, name="eps_t")
    nc.vector.memset(eps_t[:], LN_EPS)
    maskmat = consts.tile([2 * NQ, 2], F32R, name="maskmat")
    nc.gpsimd.dma_start(out=maskmat[:], in_=p_mm[:, :])

    # DRAM scratch for the collectives (fp8 payloads)
    dram = bctx.enter_context(tc.tile_pool(name="dram", bufs=1, space="DRAM"))
    kag_in = dram.tile([H * DH, RO], F8, name="kag_in")        # PhiK^T local slice
    kag1 = dram.tile([NQ * 512, RO], F8, name="kag1")          # gathered heads 0-7
    kag2 = dram.tile([NQ * 512, RO], F8, name="kag2")          # gathered heads 8-15
    vag_in = dram.tile([RO, H * 66], F8, name="vag_in")        # V(+ones) local rows
    vag1 = dram.tile([NQ * 256, H * 66], F8, name="vag1")      # gathered rows 0-255
    vag2 = dram.tile([NQ * 256, H * 66], F8, name="vag2")      # gathered rows 256-511
    hag_in = dram.tile([2, D], F32R, name="hag_in")            # my boundary Qint rows
    hag = dram.tile([2 * NQ, D], F32R, name="hag")             # gathered boundaries

    ev_state = [0]

    def evict_copy(dst_ap, src_ap):
        ev_state[0] = (ev_state[0] + 1) % 3
        if ev_state[0] == 0:
            nc.vector.tensor_copy(dst_ap, src_ap)
        else:
            nc.scalar.activation(dst_ap, src_ap, AF.Copy)

    ln_pool = bctx.enter_context(tc.tile_pool(name="ln", bufs=3))

    def layernorm_to(x_ap, p):
        """Returns (mv, rstd) tiles: mean in mv[:,0:1], rstd [p,1], for x_ap [p, D]."""
        st = ln_pool.tile([128, 2, 6], F32, tag="bn_st")
        xr = x_ap.rearrange("p (s f) -> p s f", s=2)
        for s in range(2):
            nc.vector.bn_stats(st[:p, s, :], xr[:, s, :])
        mv = ln_pool.tile([128, 2], F32, tag="bn_mv")
        nc.vector.bn_aggr(mv[:p], st[:p])
        rstd = ln_pool.tile([128, 1], F32, tag="bn_rstd")
        nc.scalar.activation(rstd[:p], mv[:p, 1:2], AF.Sqrt, bias=eps_t[:p, 0:1])
        nc.vector.reciprocal(rstd[:p], rstd[:p])
        return mv, rstd

    # ---- lifetime stacks (must nest LIFO): f34 > av2 > av > phase stacks ----
    f34_stack = ExitStack()   # hfc + qint: from Wo until the end
    av2_stack = ExitStack()   # mTc: until end of Wo
    av_stack = ExitStack()    # vT, phiQ: until end of head loop
    hfc_pool = f34_stack.enter_context(tc.tile_pool(name="hfc", bufs=1))
    qint_pool = f34_stack.enter_context(tc.tile_pool(name="qint", bufs=1))
    wup_pool = f34_stack.enter_context(tc.tile_pool(name="wup", bufs=12))
    mTc_pool = av2_stack.enter_context(tc.tile_pool(name="mTc", bufs=1))
    vT_pool = av_stack.enter_context(tc.tile_pool(name="vT", bufs=1))
    phiQ_pool = av_stack.enter_context(tc.tile_pool(name="phiQ", bufs=1))
    mTc = [mTc_pool.tile([128, RO], BF16, name=f"mTc{j}") for j in range(8)]
    vT = [vT_pool.tile([128, RO], F32R, name=f"vT{j}") for j in range(8)]
    phiQT = [phiQ_pool.tile([128, RO], F8, name=f"phiQT{j}") for j in range(8)]

    # ---------------- Phase P: LN1 + Hc + transposes ----------------
    p_stack = ExitStack()
    hcT_pool = p_stack.enter_context(tc.tile_pool(name="hcT", bufs=1))
    hcT = [hcT_pool.tile([128, RO], BF16, name=f"hcT{j}") for j in range(8)]
    io_pool = p_stack.enter_context(tc.tile_pool(name="io", bufs=4))
    hc_pool = p_stack.enter_context(tc.tile_pool(name="hc", bufs=2))
    # the PSUM phases of P (hc transposes / projections / V transposes) are
    # disjoint in time, so each gets its own short-lived 4-6 buffer pool
    t_stack = ExitStack()
    pp_t = t_stack.enter_context(tc.tile_pool(name="pp_t", bufs=4, space="PSUM"))

    # stream all qin/xb1 chunks up front; chunk 0 is the critical path, so
    # its rows are split across both hardware DMA queues
    qin_ts, xb1_ts = [], []
    for i, (r0, p) in enumerate(CHUNKS):
        qin_t = io_pool.tile([p, D], F32, tag="qin")
        if i == 0:
            h = p // 2
            nc.sync.dma_start(out=qin_t[0:h, :], in_=p_qin[r0:r0 + h, :])
            nc.scalar.dma_start(out=qin_t[h:p, :], in_=p_qin[r0 + h:r0 + p, :])
        else:
            nc.sync.dma_start(out=qin_t[:], in_=p_qin[r0:r0 + p, :])
        qin_ts.append(qin_t)
        xb1_t = io_pool.tile([p, D], F32, tag="xb1")
        nc.scalar.dma_start(out=xb1_t[:], in_=p_xb1[r0:r0 + p, :])
        xb1_ts.append(xb1_t)

    for i, (r0, p) in enumerate(CHUNKS):
        qin_t, xb1_t = qin_ts[i], xb1_ts[i]
        mv, rstd = layernorm_to(qin_t[:p, :], p)
        hc_t = hc_pool.tile([p, D], F32, tag="hc")
        nc.vector.tensor_scalar(
            out=hc_t[:p, :], in0=qin_t[:p, :], scalar1=mv[:p, 0:1],
            scalar2=rstd[:p, 0:1], op0=ALU.subtract, op1=ALU.mult,
        )
        if not g1_one:
            nc.vector.tensor_mul(hc_t[:p, :], hc_t[:p, :], g1b[:p, :])
        nc.vector.tensor_add(hc_t[:p, :], hc_t[:p, :], xb1_t[:p, :])

        # transpose this row-chunk into the 8 hcT column tiles
        for j in range(8):
            tp = pp_t.tile([128, 128], F32, tag="tp")
            nc.tensor.transpose(tp[:128, :p], hc_t[:p, j * 128:(j + 1) * 128], ident_f[:p, :p])
            evict_copy(hcT[j][:, r0:r0 + p], tp[:128, :p])
    t_stack.close()

    # ---------------- Phase P: projections ----------------
    # Order: K -> K-AllGather (smallest latency to first collective), then Q
    # (needed with K for the W matmuls), then V -> V-AllGather. The rings
    # serialize on the collective lane, so K's goes first.
    # wk/wv/wq all live at once (wv is re-read by the late V^T pass)
    wstream = p_stack.enter_context(tc.tile_pool(name="wstream", bufs=24))
    pr_stack = ExitStack()
    pp_a = pr_stack.enter_context(tc.tile_pool(name="pp_a", bufs=6, space="PSUM"))
    elu_pool = p_stack.enter_context(tc.tile_pool(name="elu", bufs=3))

    def elu1_evict(dst_ap, src_psum_ap, p, w):
        """dst = elu(src)+1 = relu(src) + exp(min(src,0)) (fp8 out)"""
        tmin = elu_pool.tile([128, 512], F32, tag="tmin")
        nc.vector.tensor_scalar_min(tmin[:p, :w], src_psum_ap, 0.0)
        texp = elu_pool.tile([128, 512], F32, tag="texp")
        nc.scalar.activation(texp[:p, :w], tmin[:p, :w], AF.Exp)
        nc.vector.scalar_tensor_tensor(
            out=dst_ap, in0=src_psum_ap, scalar=0.0, in1=texp[:p, :w],
            op0=ALU.max, op1=ALU.add,
        )

    # K^T -> PhiK^T (fp8) -> kag_in; two chunked AllGathers (heads 0-7, 8-15)
    wk_sb = []
    for k in range(8):
        w_t = wstream.tile([128, D], BF16, tag="wproj")
        nc.sync.dma_start(out=w_t[:], in_=p_wk[k * 128:(k + 1) * 128, :])
        wk_sb.append(w_t)
    phiK_pool = p_stack.enter_context(tc.tile_pool(name="phiK", bufs=4))
    for j in range(8):
        ps = pp_a.tile([128, 512], F32, tag="proj")
        for k in range(8):
            nc.tensor.matmul(
                ps[:], wk_sb[k][:, j * 128:(j + 1) * 128],
                hcT[k][:, 0:RO], start=(k == 0), stop=(k == 7),
            )
        phiK_t = phiK_pool.tile([128, RO], F8, tag="phiK")
        elu1_evict(phiK_t[:, :], ps[:], 128, RO)
        nc.sync.dma_start(out=kag_in[j * 128:(j + 1) * 128, :], in_=phiK_t[:])
        if j == 3:
            nc.gpsimd.collective_compute(
                "AllGather", ALU.bypass, replica_groups=GROUPS,
                ins=[kag_in[0:512, :].opt()], outs=[kag1[:].opt()],
            )
    nc.gpsimd.collective_compute(
        "AllGather", ALU.bypass, replica_groups=GROUPS,
        ins=[kag_in[512:1024, :].opt()], outs=[kag2[:].opt()],
    )

    # V in ROW-major, straight into the AllGather staging tiles: stationary
    # hcT row-block, moving Wv. One strided eviction per [128, 512] psum
    # lands 8 heads' 64-wide slices; no PE transposes, and the V AllGathers
    # launch ~35us earlier than the old V^T+transpose pipeline.
    wv_sb = []
    for k in range(8):
        w_t = wstream.tile([128, D], BF16, tag="wproj")
        nc.gpsimd.dma_start(out=w_t[:], in_=p_wv[k * 128:(k + 1) * 128, :])
        wv_sb.append(w_t)
    vs_pool = p_stack.enter_context(tc.tile_pool(name="vs", bufs=3))
    for i in range(4):
        r0 = i * 128
        vstage = vs_pool.tile([128, H * 66], F8, tag="vstage")
        vsr = vstage[:].rearrange("p (h d) -> p h d", h=H)
        for half in range(2):
            ps = pp_a.tile([128, 512], F32, tag="proj")
            for k in range(8):
                nc.tensor.matmul(
                    ps[:], hcT[k][:, r0:r0 + 128],
                    wv_sb[k][:, half * 512:(half + 1) * 512],
                    start=(k == 0), stop=(k == 7),
                )
            # psum col c -> head (half*8 + c//64), dim c%64
            evict_copy(vsr[:, half * 8:half * 8 + 8, 0:64], ps[:])
        nc.vector.memset(vsr[:, :, 64:65], 1.0)
        nc.vector.memset(vsr[:, :, 65:66], 0.0)
        nc.gpsimd.dma_start(out=vag_in[r0:r0 + 128, :], in_=vstage[:])
        if i == 1:
            nc.gpsimd.collective_compute(
                "AllGather", ALU.bypass, replica_groups=GROUPS,
                ins=[vag_in[0:256, :].opt()], outs=[vag1[:].opt()],
            )
    nc.gpsimd.collective_compute(
        "AllGather", ALU.bypass, replica_groups=GROUPS,
        ins=[vag_in[256:512, :].opt()], outs=[vag2[:].opt()],
    )

    # Q^T -> PhiQ^T (fp8, kept in SBUF)
    wq_sb = []
    for k in range(8):
        w_t = wstream.tile([128, D], BF16, tag="wproj")
        nc.gpsimd.dma_start(out=w_t[:], in_=p_wq[k * 128:(k + 1) * 128, :])
        wq_sb.append(w_t)
    for j in range(8):
        ps = pp_a.tile([128, 512], F32, tag="proj")
        for k in range(8):
            nc.tensor.matmul(
                ps[:], wq_sb[k][:, j * 128:(j + 1) * 128],
                hcT[k][:, 0:RO], start=(k == 0), stop=(k == 7),
            )
        elu1_evict(phiQT[j][:, :], ps[:], 128, RO)

    # V^T (only the attention-post subtraction needs it)
    for j in range(8):
        ps = pp_a.tile([128, 512], F32, tag="proj")
        for k in range(8):
            nc.tensor.matmul(
                ps[:], wv_sb[k][:, j * 128:(j + 1) * 128],
                hcT[k][:, 0:RO], start=(k == 0), stop=(k == 7),
            )
        evict_copy(vT[j][:, :], ps[:])
    pr_stack.close()

    p_stack.close()

    # ---------------- Phase A: attention ----------------
    # Per head-group g (4 heads): W(g) = 64 fp8 matmuls (K=64) evicted as
    # (W/32)^2 fp8 into DoubleRow pair tiles; Attr(g) = per head 8 fp8-DR
    # matmuls over (m-block pair, key) tiles. Emission order W0 W1 A0 W2 A1
    # W3 A2 A3 keeps the PE busy while the V AllGathers land.
    # Wo weights: pool created first (released after attention pools), loads
    # issued now so the Wo phase starts instantly
    wo_stack = ExitStack()
    wo_pool = wo_stack.enter_context(tc.tile_pool(name="wo", bufs=8))
    wo_sb = []
    for k in range(8):
        w_t = wo_pool.tile([128, D], BF16, tag="wo")
        nc.scalar.dma_start(out=w_t[:], in_=p_wo[k * 128:(k + 1) * 128, :])
        wo_sb.append(w_t)
    # prefetch the first FFN superchunk's gate weights during attention
    wupg_pf = []
    for k in range(8):
        wg = wup_pool.tile([128, 512], BF16, tag="wupg")
        nc.gpsimd.dma_start(out=wg[:], in_=p_wup[k * 128:(k + 1) * 128, 0:512])
        wupg_pf.append(wg)

    a_stack = ExitStack()
    kq_pool = a_stack.enter_context(tc.tile_pool(name="kq", bufs=3))
    vhd_pool = a_stack.enter_context(tc.tile_pool(name="vhd", bufs=16))
    # dual-fp8 LDWEIGHTS needs stationary width % 32 == 0: vhd is 96 wide
    # (V 0-63, ones 64, pad 65-95). DMA writes cols 0-65; zero the pad cols
    # once per pool buffer (round-robin reuse keeps them zero).
    for _ in range(16):
        vz = vhd_pool.tile([128, 8, 96], F8, tag="vhd")
        nc.vector.memset(vz[:, :, 66:96], 0.0)
    wt_pool = a_stack.enter_context(tc.tile_pool(name="wt", bufs=64))
    asm_pool = a_stack.enter_context(tc.tile_pool(name="asm", bufs=3))
    pp_w = a_stack.enter_context(tc.tile_pool(name="pp_w", bufs=4, space="PSUM"))
    pp_at = a_stack.enter_context(tc.tile_pool(name="pp_at", bufs=4, space="PSUM"))

    sq_state = [0]

    def square_evict(dst_ap, src_psum_ap):
        """dst = src^2 fp8; src is already W/32 (W > 0, relu is a no-op).
        Rotated 5:2 across ACT/DVE: ACT streams ~1 col/ns single-pass; the
        DVE two-pass path costs ~2x that, so it only soaks the overflow."""
        sq_state[0] = (sq_state[0] + 1) % 7
        if sq_state[0] < 5:
            nc.scalar.activation(dst_ap, src_psum_ap, AF.Square, scale=WSC)
        else:
            tr = asm_pool.tile([128, 512], BF16, tag="r2tmpv")
            nc.vector.tensor_scalar_mul(tr[:, :], src_psum_ap, WSC)
            nc.vector.tensor_mul(dst_ap, tr[:, :], tr[:, :])

    def emit_w(hg):
        """W^T for 4 heads of group hg -> wtp fp8 DoubleRow pair tiles.
        The two heads sharing a kq/phiQT tile (PE row halves 0-63 / 64-127)
        are interleaved: consecutive matmuls hit disjoint row groups, so
        LDWEIGHTS overlaps the in-flight matmul."""
        kag_t = kag1 if hg < 2 else kag2
        kq_sb = {}
        for j2 in (2 * hg, 2 * hg + 1):
            hrow = (j2 % 4) * 128  # row offset of head-pair j2 within kag_t
            kt = kq_pool.tile([128, NQ, RO], F8, tag="kq")
            ksrc = kag_t[:, :]
            nc.sync.dma_start(
                out=kt[:, :, :],
                in_=bass.AP(tensor=ksrc.tensor, offset=ksrc.offset + hrow * RO,
                            ap=[[RO, 128], [512 * RO, NQ], [1, RO]]),
            )
            kq_sb[j2] = kt
        wtp = {}
        for hh in range(4):
            h = hg * 4 + hh
            wtp[h] = [wt_pool.tile([128, 2, RO], F8, tag="wt", name=f"wt{h}_{t}")
                      for t in range(8)]
        for j2 in (2 * hg, 2 * hg + 1):
            hA, hB = 2 * j2, 2 * j2 + 1
            for m in range(16):
                qq, lc = m // 4, m % 4
                for off, h in ((0, hA), (64, hB)):
                    psw = pp_w.tile([128, 512], F32, tag="psw")
                    nc.tensor.matmul(
                        psw[:], kq_sb[j2][off:off + 64, qq, lc * 128:(lc + 1) * 128],
                        phiQT[j2][off:off + 64, :], start=True, stop=True,
                    )
                    square_evict(wtp[h][m // 2][:, m % 2, :], psw)
        return wtp

    # pair order follows the chunked V gathers: vag1 pairs (lc 0,1) first
    T_ORDER = [qq * 2 for qq in range(NQ)] + [qq * 2 + 1 for qq in range(NQ)]

    def emit_attr(hg, wtp):
        pats = []
        for hh in range(4):
            h = hg * 4 + hh
            pat = pp_at.tile([96, 512], F32, tag="pat", name=f"pat{h}")
            vh = {}
            for half in range(2):
                vsrc = vag1 if half == 0 else vag2
                vt = vhd_pool.tile([128, 8, 96], F8, tag="vhd")
                vap = vsrc[:, :]
                W16 = H * 66
                nc.sync.dma_start(
                    out=vt[:, :, 0:66],
                    in_=bass.AP(tensor=vap.tensor, offset=vap.offset + h * 66,
                                ap=[[W16, 128], [128 * W16, 8], [1, 66]]))
                vh[half] = vt
            for ti, t in enumerate(T_ORDER):
                qq, half = t // 2, t % 2
                nc.tensor.matmul(
                    pat[:], vh[half][:, 2 * qq:2 * qq + 2, :], wtp[h][t][:, :, :],
                    start=(ti == 0), stop=(ti == 7), perf_mode=DR,
                )
            pats.append(pat)
        for hh in range(4):
            h = hg * 4 + hh
            j2, off = h // 2, (h % 2) * 64
            nrm = asm_pool.tile([1, RO], F32, tag="nrm")
            nc.vector.tensor_scalar_add(nrm[0:1, :], pats[hh][64:65, :], WSC2)
            nc.vector.reciprocal_approx_fast(out=nrm[:], in_=nrm[:])
            rcb = asm_pool.tile([64, RO], F32, tag="rcb")
            nc.gpsimd.partition_broadcast(rcb[:], nrm[:])
            tm = asm_pool.tile([128, RO], F32, tag="tm")
            nc.vector.tensor_mul(tm[off:off + 64, :], pats[hh][0:64, :], rcb[:, :])
            nc.vector.tensor_sub(
                mTc[j2][off:off + 64, :], tm[off:off + 64, :],
                vT[j2][off:off + 64, :],
            )

    wtp_q = [emit_w(0), emit_w(1)]
    for hg in range(4):
        emit_attr(hg, wtp_q[hg])
        if hg + 2 < 4:
            wtp_q.append(emit_w(hg + 2))
    a_stack.close()

    # ---------------- Phase A5: Wo + Q_interact ----------------
    # Chunk order 0,3,1,2 so the conv-halo boundary rows exist after two
    # chunks and their AllGather overlaps the rest of Wo + LN2.
    a5_stack = ExitStack()
    qi_pool = a5_stack.enter_context(tc.tile_pool(name="qi", bufs=3))
    pp_o = a5_stack.enter_context(tc.tile_pool(name="pp_o", bufs=4, space="PSUM"))
    qint = [None] * 4
    for oi, i in enumerate((0, 3, 1, 2)):
        r0, p = CHUNKS[i]
        qin_t = qi_pool.tile([p, D], F32, tag="qin2")
        nc.sync.dma_start(out=qin_t[:], in_=p_qin[r0:r0 + p, :])
        qi = qint_pool.tile([p, D], F32, name=f"qint{i}")
        for half in range(2):
            pso = pp_o.tile([128, 512], F32, tag="pso")
            for k in range(8):
                nc.tensor.matmul(
                    pso[:p, :], mTc[k][:, r0:r0 + p],
                    wo_sb[k][:, half * 512:(half + 1) * 512],
                    start=(k == 0), stop=(k == 7),
                )
            nc.vector.scalar_tensor_tensor(
                out=qi[:p, half * 512:(half + 1) * 512], in0=pso[:p, :],
                scalar=dt_safe, in1=qin_t[:p, half * 512:(half + 1) * 512],
                op0=ALU.mult, op1=ALU.add,
            )
        qint[i] = qi
        if oi == 1:
            # boundary rows ready: stage + AllGather (conv halo exchange)
            nc.gpsimd.dma_start(out=hag_in[0:1, :], in_=qint[0][0:1, :])
            nc.gpsimd.dma_start(out=hag_in[1:2, :], in_=qint[3][127:128, :])
            nc.gpsimd.collective_compute(
                "AllGather", ALU.bypass, replica_groups=GROUPS,
                ins=[hag_in[:].opt()], outs=[hag[:].opt()],
            )
    a5_stack.close()
    wo_stack.close()
    av_stack.close()   # frees vT, phiQ
    av2_stack.close()  # frees mTc

    # ---------------- Phase F: LN2 + transpose + FFN ----------------
    hfc = []

    qn2T_pool = f34_stack.enter_context(tc.tile_pool(name="qn2T", bufs=1))
    f_stack = ExitStack()
    qn2_pool = f_stack.enter_context(tc.tile_pool(name="qn2", bufs=2))
    pp_f = f_stack.enter_context(tc.tile_pool(name="pp_f", bufs=4, space="PSUM"))
    # qn2T cols: 0..511 owned rows, 512 = prev-halo row, 513 = next-halo row
    qn2T = [qn2T_pool.tile([128, RO + 2], BF16, name=f"qn2T{j}") for j in range(8)]
    for i, (r0, p) in enumerate(CHUNKS):
        mv, rstd = layernorm_to(qint[i][:p, :], p)
        qn2_t = qn2_pool.tile([p, D], F32, tag="qn2")
        nc.vector.tensor_scalar(
            out=qn2_t[:p, :], in0=qint[i][:p, :], scalar1=mv[:p, 0:1],
            scalar2=rstd[:p, 0:1], op0=ALU.subtract, op1=ALU.mult,
        )
        for j in range(8):
            tp = pp_f.tile([128, 128], F32, tag="tpf")
            nc.tensor.transpose(tp[:128, :p], qn2_t[:p, j * 128:(j + 1) * 128], ident_f[:p, :p])
            evict_copy(qn2T[j][:, r0:r0 + p], tp[:128, :p])

    # halo rows: extract prev/next boundary rows via maskmat.T @ gathered,
    # then LN2 + transpose into qn2T cols 512/513
    pp_h = f_stack.enter_context(tc.tile_pool(name="pp_h", bufs=1, space="PSUM"))
    hg_sb = qn2_pool.tile([2 * NQ, D], F32R, name="hg_sb")
    nc.sync.dma_start(out=hg_sb[:], in_=hag[:, :])
    qih = qn2_pool.tile([2, D], F32, name="qih")
    for half in range(2):
        ph = pp_h.tile([2, 512], F32, tag="psh", name=f"ph{half}")
        nc.tensor.matmul(
            ph[:], maskmat[:], hg_sb[:, half * 512:(half + 1) * 512],
            start=True, stop=True,
        )
        nc.vector.tensor_copy(qih[:, half * 512:(half + 1) * 512], ph[:])
    mv, rstd = layernorm_to(qih[:2, :], 2)
    qn2h = qn2_pool.tile([2, D], F32, name="qn2h")
    nc.vector.tensor_scalar(
        out=qn2h[:2, :], in0=qih[:2, :], scalar1=mv[:2, 0:1],
        scalar2=rstd[:2, 0:1], op0=ALU.subtract, op1=ALU.mult,
    )
    for j in range(8):
        tp = pp_f.tile([128, 128], F32, tag="tpf")
        nc.tensor.transpose(tp[:128, :2], qn2h[:2, j * 128:(j + 1) * 128], ident_f[:2, :2])
        evict_copy(qn2T[j][:, RO:RO + 2], tp[:128, :2])
    f_stack.close()

    # Wup (fp8 DoubleRow, K=256 per matmul) + SwiGLU + depthwise conv, in
    # 512-col superchunks; Wdown's first column-half rides along, one inner
    # pair behind the conv. Scales: wup carries x32, wdown x64 (host side);
    # the 1/32 descale folds into the Silu input scale / U bias / conv taps,
    # the 1/64 into the output eviction.
    f2_stack = ExitStack()
    pp_d = f2_stack.enter_context(tc.tile_pool(name="pp_d", bufs=1, space="PSUM"))
    ffn_stack = ExitStack()
    gu_stack = ExitStack()
    pp_g = gu_stack.enter_context(tc.tile_pool(name="pp_g", bufs=2, space="PSUM"))
    pp_u = gu_stack.enter_context(tc.tile_pool(name="pp_u", bufs=2, space="PSUM"))
    wupu_pool = ffn_stack.enter_context(tc.tile_pool(name="wupu", bufs=12))
    fsm_pool = ffn_stack.enter_context(tc.tile_pool(name="fsm", bufs=3))
    bias_pool = ffn_stack.enter_context(tc.tile_pool(name="bias", bufs=6))
    wd0_pool = ffn_stack.enter_context(tc.tile_pool(name="wd0", bufs=4))
    wd1_pool = ffn_stack.enter_context(tc.tile_pool(name="wd1", bufs=16))

    psd0 = [pp_d.tile([128, 512], F32, name=f"psd0_{i}") for i in range(4)]
    # hfc: fp8 DoubleRow pair tiles; pair t holds inner blocks (2t, 2t+1)
    for t in range(16):
        hfc.append(hfc_pool.tile([128, 2, RO], F8, name=f"hfc{t}"))
    wd0_sb = {}

    def emit_wdown_pair(t):
        wd_t = wd0_sb.pop(t)
        for i in range(4):
            nc.tensor.matmul(
                psd0[i][:], hfc[t][:, :, i * 128:(i + 1) * 128],
                wd_t[:, :, :], start=(t == 0), stop=(t == 15), perf_mode=DR,
            )

    wd1_sb = []
    for sc in range(8):
        if sc == 0:
            wupg_sb = wupg_pf
        else:
            wupg_sb = []
            for k in range(8):
                wg = wup_pool.tile([128, 512], BF16, tag="wupg")
                nc.sync.dma_start(
                    out=wg[:], in_=p_wup[k * 128:(k + 1) * 128, sc * 512:(sc + 1) * 512]
                )
                wupg_sb.append(wg)
        wupu_sb = []
        for k in range(8):
            wu = wupu_pool.tile([128, 512], BF16, tag="wupu")
            nc.scalar.dma_start(
                out=wu[:], in_=p_wup[k * 128:(k + 1) * 128, INNER + sc * 512:INNER + (sc + 1) * 512]
            )
            wupu_sb.append(wu)
        for c in range(4):
            cc = sc * 4 + c
            if cc % 2 == 0:
                tn = cc // 2
                wd_t = wd0_pool.tile([128, 2, 512], F8, tag="wd0")
                nc.gpsimd.dma_start(out=wd_t[:, :, :], in_=p_wd08[tn * 128:(tn + 1) * 128, :])
                wd0_sb[tn] = wd_t
                # prefetch the matching second-half tile for the tail phase
                wd1_t = wd1_pool.tile([128, 2, 512], F8, tag="wd1")
                nc.gpsimd.dma_start(out=wd1_t[:, :, :], in_=p_wd18[tn * 128:(tn + 1) * 128, :])
                wd1_sb.append(wd1_t)
            bg = bias_pool.tile([128, 1], F32, tag="bg")
            nc.gpsimd.dma_start(out=bg[:], in_=bass.AP(tensor=p_bgu, offset=cc * 128, ap=[[1, 128], [1, 1]]))
            bu = bias_pool.tile([128, 1], F32, tag="bu")
            nc.gpsimd.dma_start(out=bu[:], in_=bass.AP(tensor=p_bgu, offset=INNER + cc * 128, ap=[[1, 128], [1, 1]]))
            cw = bias_pool.tile([128, 3], F32, tag="cw")
            nc.gpsimd.dma_start(out=cw[:], in_=p_cw[cc * 128:(cc + 1) * 128, :])

            gact = fsm_pool.tile([128, RO + 2], F32, tag="gact")
            hf = fsm_pool.tile([128, RO + 2], F32, tag="hf")
            for h0, w in HALVES:
                psg = pp_g.tile([128, 258], F32, tag="psg")
                for k in range(8):
                    nc.tensor.matmul(
                        psg[:, :w], wupg_sb[k][:, c * 128:(c + 1) * 128],
                        qn2T[k][:, h0:h0 + w], start=(k == 0), stop=(k == 7),
                    )
                nc.scalar.activation(gact[:, h0:h0 + w], psg[:, :w], AF.Silu, bias=bg[:, 0:1])
                psu = pp_u.tile([128, 258], F32, tag="psu")
                for k in range(8):
                    nc.tensor.matmul(
                        psu[:, :w], wupu_sb[k][:, c * 128:(c + 1) * 128],
                        qn2T[k][:, h0:h0 + w], start=(k == 0), stop=(k == 7),
                    )
                nc.vector.scalar_tensor_tensor(
                    out=hf[:, h0:h0 + w], in0=psu[:, :w], scalar=bu[:, 0:1],
                    in1=gact[:, h0:h0 + w], op0=ALU.add, op1=ALU.mult,
                )
            if cc >= 2 and cc % 2 == 0:
                emit_wdown_pair(cc // 2 - 1)
            # mask halo cols at batch edges (conv zero-pad)
            nc.vector.tensor_scalar_mul(hf[:, RO:RO + 1], hf[:, RO:RO + 1], mask_p[:, 0:1])
            nc.vector.tensor_scalar_mul(hf[:, RO + 1:RO + 2], hf[:, RO + 1:RO + 2], mask_n[:, 0:1])
            # depthwise conv along rows: cols 0..511 owned, 512=prev, 513=next.
            # center tap on ACT (scale is per-partition), side taps DVE; the
            # final two taps write the fp8 DoubleRow pair tile directly.
            hfb = fsm_pool.tile([128, RO], BF16, tag="hfb")
            dst = hfc[cc // 2]
            jj = cc % 2
            nc.scalar.activation(hfb[:, 0:RO], hf[:, 0:RO], AF.Copy, scale=cw[:, 1:2])
            nc.vector.scalar_tensor_tensor(
                out=hfb[:, 1:RO], in0=hf[:, 0:RO - 1], scalar=cw[:, 0:1],
                in1=hfb[:, 1:RO], op0=ALU.mult, op1=ALU.add,
            )
            nc.vector.scalar_tensor_tensor(
                out=hfb[:, 0:1], in0=hf[:, RO:RO + 1], scalar=cw[:, 0:1],
                in1=hfb[:, 0:1], op0=ALU.mult, op1=ALU.add,
            )
            nc.vector.scalar_tensor_tensor(
                out=dst[:, jj, 0:RO - 1], in0=hf[:, 1:RO], scalar=cw[:, 2:3],
                in1=hfb[:, 0:RO - 1], op0=ALU.mult, op1=ALU.add,
            )
            nc.vector.scalar_tensor_tensor(
                out=dst[:, jj, RO - 1:RO], in0=hf[:, RO + 1:RO + 2], scalar=cw[:, 2:3],
                in1=hfb[:, RO - 1:RO], op0=ALU.mult, op1=ALU.add,
            )
    emit_wdown_pair(15)
    gu_stack.close()

    # ---------------- Phase F4: Wdown second half + residual + output ----------
    pp_d1 = f2_stack.enter_context(tc.tile_pool(name="pp_d1", bufs=1, space="PSUM"))
    psd1 = [pp_d1.tile([128, 512], F32, name=f"psd1_{i}") for i in range(4)]
    for t in range(16):
        for i in range(4):
            nc.tensor.matmul(
                psd1[i][:], hfc[t][:, :, i * 128:(i + 1) * 128],
                wd1_sb[t][:, :, :], start=(t == 0), stop=(t == 15), perf_mode=DR,
            )
    ffn_stack.close()
    out_pool = f2_stack.enter_context(tc.tile_pool(name="outp", bufs=4))
    for i in range(4):
        o_t = out_pool.tile([128, D], F32, tag="osb")
        nc.vector.scalar_tensor_tensor(
            out=o_t[:, 0:512], in0=psd0[i][:], scalar=WDINV,
            in1=qint[i][:, 0:512], op0=ALU.mult, op1=ALU.add,
        )
        nc.vector.scalar_tensor_tensor(
            out=o_t[:, 512:1024], in0=psd1[i][:], scalar=WDINV,
            in1=qint[i][:, 512:1024], op0=ALU.mult, op1=ALU.add,
        )
        out_q = (nc.sync, nc.scalar, nc.gpsimd, nc.sync)[i]
        out_q.dma_start(out=p_out[i * 128:(i + 1) * 128, :], in_=o_t[:])
    f2_stack.close()
    f34_stack.close()


def kernel(**inputs) -> np.ndarray:
    Q_in = np.ascontiguousarray(np.asarray(inputs["Q_in"], dtype=np.float32))
    X = np.ascontiguousarray(np.asarray(inputs["X"], dtype=np.float32))
    Wq = np.asarray(inputs["Wq"], dtype=np.float32)
    Wk = np.asarray(inputs["Wk"], dtype=np.float32)
    Wv = np.asarray(inputs["Wv"], dtype=np.float32)
    Wo = np.asarray(inputs["Wo"], dtype=np.float32)
    Wup = np.asarray(inputs["Wup"], dtype=np.float32)
    conv_w = np.asarray(inputs["conv_w"], dtype=np.float32)
    Wdown = np.asarray(inputs["Wdown"], dtype=np.float32)
    g1 = np.asarray(inputs["g1"], dtype=np.float32)
    b1 = np.asarray(inputs["b1"], dtype=np.float32)
    g2 = np.asarray(inputs["g2"], dtype=np.float32)
    b2 = np.asarray(inputs["b2"], dtype=np.float32)
    dt = float(np.asarray(inputs["dt"], dtype=np.float32))

    # softplus(dt) on host; baked into the NEFF as an immediate
    dt_safe = float(np.log1p(np.exp(dt)))

    # fold g2/b2 into Wup (LN2's affine commutes into the up-projection)
    wup_f = g2[:, None] * Wup
    bias_gu = np.ascontiguousarray(b2 @ Wup)
    cw3 = np.ascontiguousarray(conv_w[:, 0, :])

    g1_one = bool(np.all(g1 == 1.0))
    key = (round(dt_safe, 9), g1_one)
    if key not in _cache:
        _cache[key] = _build(dt_safe, g1_one)
    nc = _cache[key]

    bf = ml_dtypes.bfloat16
    f8 = ml_dtypes.float8_e4m3
    wq_b = np.ascontiguousarray(Wq.astype(bf))
    wk_b = np.ascontiguousarray(Wk.astype(bf))
    wv_b = np.ascontiguousarray(Wv.astype(bf))
    wo_b = np.ascontiguousarray(Wo.astype(bf))
    wup_b = np.ascontiguousarray(wup_f.astype(bf))

    # Wdown DoubleRow pair packing: tile-row t*128+r holds inner element
    # i = 128*(2t+j)+r in slot j; cols are (j, c).
    def pack_down(w):  # [INNER, 512] -> [INNER//2, D] fp8 pair layout
        t = w.reshape(16, 2, 128, 512)
        return np.ascontiguousarray(
            t.transpose(0, 2, 1, 3).reshape(INNER // 2, D).astype(f8))

    wd08 = pack_down(WDS * Wdown[:, 0:512])
    wd18 = pack_down(WDS * Wdown[:, 512:1024])

    in_maps = []
    for core in range(8):
        b, q = divmod(core, 4)
        qin = np.ascontiguousarray(Q_in[b, q * RO:(q + 1) * RO])
        xb1 = np.ascontiguousarray(X[b, q * RO:(q + 1) * RO] + b1[None, :])
        masks = np.array(
            [1.0 if q > 0 else 0.0, 1.0 if q < NQ - 1 else 0.0], dtype=np.float32
        )
        # maskmat.T @ gathered_boundaries = [prev-halo row; next-halo row]
        mm = np.zeros((2 * NQ, 2), dtype=np.float32)
        if q > 0:
            mm[2 * (q - 1) + 1, 0] = 1.0
        if q < NQ - 1:
            mm[2 * (q + 1), 1] = 1.0
        in_maps.append({
            "q_in": qin, "x_b1": xb1, "wq": wq_b, "wk": wk_b, "wv": wv_b,
            "wo": wo_b, "wup": wup_b, "bias_gu": bias_gu,
            "wd08": wd08, "wd18": wd18, "cw3": cw3,
            "g1": np.ascontiguousarray(g1), "masks": masks, "maskmat": mm,
        })

    global _last_in_maps
    _last_in_maps = in_maps
    res = run_bass_kernel_spmd(nc, in_maps, core_ids=list(range(8)))

    out = np.empty((B, N, D), dtype=np.float32)
    for core in range(8):
        b, q = divmod(core, 4)
        out[b, q * RO:(q + 1) * RO] = res.results[core]["out"]
    return out



# revision 73
# speedup vs baseline: 1.3714x; 1.0083x over previous
"""Distributed Trainium2 Bass kernel for nn_AMK_Block (kernelized-attention + ConvSwiGLU).

Sharding: sequence-parallel. Each of the 8 cores owns (batch b, query-row block q):
core = b*4 + q, rows q*512..q*512+511 of batch b, ALL heads. Each core computes
Q/K/V projections for its rows, AllGathers PhiK^T and V(+ones) across the 4 cores
of its batch group (fp8), then computes its 512 rows of attention, Wo, LN2 and
the full FFN locally. The depthwise-conv halo rows of Q_interact come from a tiny
third AllGather of boundary rows, extracted rank-agnostically with a mask-matrix
matmul. Weight matmuls run in bf16 (fp32 PSUM accumulation); the attention
kernel-matrix matmuls run in fp8 (PhiQ/PhiK/V/W^2 evicted as e4m3, Attr uses
DoubleRow packed k-pairs); norm/statistics in fp32. Wdown's first column-half is
interleaved into the FFN chunk stream so only half remains as a tail.
"""

import sys

sys.path.insert(0, "/opt/trn_rl_repo")

from contextlib import ExitStack

import ml_dtypes
import numpy as np

import concourse.bass as bass
import concourse.tile as tile
from concourse import bacc, mybir
from concourse.bass_utils import run_bass_kernel_spmd
from concourse.masks import make_identity

F32 = mybir.dt.float32
F32R = mybir.dt.float32r
BF16 = mybir.dt.bfloat16
F8 = mybir.dt.float8e4
ALU = mybir.AluOpType
AF = mybir.ActivationFunctionType
DR = mybir.MatmulPerfMode.DoubleRow

B, N, D = 2, 2048, 1024
H, DH = 16, 64
INNER = 4096
LN_EPS = 1e-5
WSC = 1.0 / 32.0          # W^2 is evicted as (W/32)^2 = W^2/1024 in fp8
WSC2 = WSC * WSC
UPS = 32.0                # Wup fp8 host scale (values ~N(0,1/32) -> ~N(0,1))
UPSC = 1.0 / UPS
WDS = 64.0                # Wdown fp8 host scale
WDINV = 1.0 / WDS

RO = 512          # owned rows per core
NQ = 4            # cores per batch group
GROUPS = [[0, 1, 2, 3], [4, 5, 6, 7]]
CHUNKS = [(0, 128), (128, 128), (256, 128), (384, 128)]
HALVES = [(0, 258), (258, 256)]  # even halves of 514; halo cols 512/513 in 2nd

_cache: dict[float, object] = {}
_last_in_maps: list | None = None


def _build(dt_safe: float, g1_one: bool):
    nc = bacc.Bacc("TRN2", target_bir_lowering=False, debug=False, num_devices=8)

    # ---------------- DRAM parameters (per-core shapes) ----------------
    p_qin = nc.declare_dram_parameter("q_in", [RO, D], F32, isOutput=False)
    p_xb1 = nc.declare_dram_parameter("x_b1", [RO, D], F32, isOutput=False)
    p_wq = nc.declare_dram_parameter("wq", [D, D], BF16, isOutput=False)
    p_wk = nc.declare_dram_parameter("wk", [D, D], BF16, isOutput=False)
    p_wv = nc.declare_dram_parameter("wv", [D, D], BF16, isOutput=False)
    p_wo = nc.declare_dram_parameter("wo", [D, D], BF16, isOutput=False)
    p_wup = nc.declare_dram_parameter("wup", [D, 2 * INNER], BF16, isOutput=False)
    p_bgu = nc.declare_dram_parameter("bias_gu", [2 * INNER], F32, isOutput=False)
    p_wd08 = nc.declare_dram_parameter("wd08", [INNER // 2, D], F8, isOutput=False)
    p_wd18 = nc.declare_dram_parameter("wd18", [INNER // 2, D], F8, isOutput=False)
    p_cw = nc.declare_dram_parameter("cw3", [INNER, 3], F32, isOutput=False)
    p_g1 = nc.declare_dram_parameter("g1", [D], F32, isOutput=False)
    p_mask = nc.declare_dram_parameter("masks", [2], F32, isOutput=False)
    p_mm = nc.declare_dram_parameter("maskmat", [2 * NQ, 2], F32R, isOutput=False)
    p_out = nc.declare_dram_parameter("out", [RO, D], F32, isOutput=True)

    with tile.TileContext(nc) as tc:
        build_ctx = ExitStack()
        with build_ctx:
            _emit(nc, tc, build_ctx, dt_safe, g1_one, p_qin, p_xb1, p_wq,
                  p_wk, p_wv, p_wo, p_wup, p_bgu, p_wd08, p_wd18, p_cw, p_g1,
                  p_mask, p_mm, p_out)
    nc.finalize()
    return nc


def _emit(nc, tc, bctx, dt_safe, g1_one, p_qin, p_xb1, p_wq, p_wk, p_wv,
          p_wo, p_wup, p_bgu, p_wd08, p_wd18, p_cw, p_g1, p_mask, p_mm, p_out):
    # ---------------- constant tiles (gpsimd queue: off the qin path) ------
    consts = bctx.enter_context(tc.tile_pool(name="consts", bufs=1))
    g1b = consts.tile([128, D], F32, name="g1b")
    nc.gpsimd.dma_start(
        out=g1b[:],
        in_=bass.AP(tensor=p_g1, offset=0, ap=[[0, 128], [1, D]]),
    )
    ident_f = consts.tile([128, 128], F32, name="ident_f")
    make_identity(nc, ident_f[:])
    ident_r = consts.tile([128, 128], F32R, name="ident_r")
    nc.gpsimd.dma_start(out=ident_r[:], in_=ident_f[:])
    mask_p = consts.tile([128, 1], F32, name="mask_p")
    nc.gpsimd.dma_start(out=mask_p[:], in_=bass.AP(tensor=p_mask, offset=0, ap=[[0, 128], [1, 1]]))
    mask_n = consts.tile([128, 1], F32, name="mask_n")
    nc.gpsimd.dma_start(out=mask_n[:], in_=bass.AP(tensor=p_mask, offset=1, ap=[[0, 128], [1, 1]]))
    ones_col = consts.tile([128, 1], F8, name="ones_col")
    nc.vector.memset(ones_col[:], 1.0)
    eps_t = consts.tile([128, 1], F32, name="eps_t")
    nc.vector.memset(eps_t[:], LN_EPS)
    maskmat = consts.tile([2 * NQ, 2], F32R, name="maskmat")
    nc.gpsimd.dma_start(out=maskmat[:], in_=p_mm[:, :])

    # DRAM scratch for the collectives (fp8 payloads)
    dram = bctx.enter_context(tc.tile_pool(name="dram", bufs=1, space="DRAM"))
    kag_in = dram.tile([H * DH, RO], F8, name="kag_in")        # PhiK^T local slice
    kag1 = dram.tile([NQ * 512, RO], F8, name="kag1")          # gathered heads 0-7
    kag2 = dram.tile([NQ * 512, RO], F8, name="kag2")          # gathered heads 8-15
    vag_in = dram.tile([RO, H * 66], F8, name="vag_in")        # V(+ones) local rows
    vag1 = dram.tile([NQ * 256, H * 66], F8, name="vag1")      # gathered rows 0-255
    vag2 = dram.tile([NQ * 256, H * 66], F8, name="vag2")      # gathered rows 256-511
    hag_in = dram.tile([2, D], F32R, name="hag_in")            # my boundary Qint rows
    hag = dram.tile([2 * NQ, D], F32R, name="hag")             # gathered boundaries

    ev_state = [0]

    def evict_copy(dst_ap, src_ap):
        ev_state[0] = (ev_state[0] + 1) % 3
        if ev_state[0] == 0:
            nc.vector.tensor_copy(dst_ap, src_ap)
        else:
            nc.scalar.activation(dst_ap, src_ap, AF.Copy)

    ln_pool = bctx.enter_context(tc.tile_pool(name="ln", bufs=3))

    def layernorm_to(x_ap, p):
        """Returns (mv, rstd) tiles: mean in mv[:,0:1], rstd [p,1], for x_ap [p, D]."""
        st = ln_pool.tile([128, 2, 6], F32, tag="bn_st")
        xr = x_ap.rearrange("p (s f) -> p s f", s=2)
        for s in range(2):
            nc.vector.bn_stats(st[:p, s, :], xr[:, s, :])
        mv = ln_pool.tile([128, 2], F32, tag="bn_mv")
        nc.vector.bn_aggr(mv[:p], st[:p])
        rstd = ln_pool.tile([128, 1], F32, tag="bn_rstd")
        nc.scalar.activation(rstd[:p], mv[:p, 1:2], AF.Sqrt, bias=eps_t[:p, 0:1])
        nc.vector.reciprocal(rstd[:p], rstd[:p])
        return mv, rstd

    # ---- lifetime stacks (must nest LIFO): f34 > av2 > av > phase stacks ----
    f34_stack = ExitStack()   # hfc + qint: from Wo until the end
    av2_stack = ExitStack()   # mTc: until end of Wo
    av_stack = ExitStack()    # vT, phiQ: until end of head loop
    hfc_pool = f34_stack.enter_context(tc.tile_pool(name="hfc", bufs=1))
    qint_pool = f34_stack.enter_context(tc.tile_pool(name="qint", bufs=1))
    wup_pool = f34_stack.enter_context(tc.tile_pool(name="wup", bufs=12))
    mTc_pool = av2_stack.enter_context(tc.tile_pool(name="mTc", bufs=1))
    vT_pool = av_stack.enter_context(tc.tile_pool(name="vT", bufs=1))
    phiQ_pool = av_stack.enter_context(tc.tile_pool(name="phiQ", bufs=1))
    mTc = [mTc_pool.tile([128, RO], BF16, name=f"mTc{j}") for j in range(8)]
    vT = [vT_pool.tile([128, RO], F32R, name=f"vT{j}") for j in range(8)]
    phiQT = [phiQ_pool.tile([128, RO], F8, name=f"phiQT{j}") for j in range(8)]

    # ---------------- Phase P: LN1 + Hc + transposes ----------------
    p_stack = ExitStack()
    hcT_pool = p_stack.enter_context(tc.tile_pool(name="hcT", bufs=1))
    hcT = [hcT_pool.tile([128, RO], BF16, name=f"hcT{j}") for j in range(8)]
    io_pool = p_stack.enter_context(tc.tile_pool(name="io", bufs=4))
    hc_pool = p_stack.enter_context(tc.tile_pool(name="hc", bufs=2))
    # the PSUM phases of P (hc transposes / projections / V transposes) are
    # disjoint in time, so each gets its own short-lived 4-6 buffer pool
    t_stack = ExitStack()
    pp_t = t_stack.enter_context(tc.tile_pool(name="pp_t", bufs=4, space="PSUM"))

    # stream all qin/xb1 chunks up front; chunk 0 is the critical path, so
    # its rows are split across both hardware DMA queues
    qin_ts, xb1_ts = [], []
    for i, (r0, p) in enumerate(CHUNKS):
        qin_t = io_pool.tile([p, D], F32, tag="qin")
        if i == 0:
            h = p // 2
            nc.sync.dma_start(out=qin_t[0:h, :], in_=p_qin[r0:r0 + h, :])
            nc.scalar.dma_start(out=qin_t[h:p, :], in_=p_qin[r0 + h:r0 + p, :])
        else:
            nc.sync.dma_start(out=qin_t[:], in_=p_qin[r0:r0 + p, :])
        qin_ts.append(qin_t)
        xb1_t = io_pool.tile([p, D], F32, tag="xb1")
        nc.scalar.dma_start(out=xb1_t[:], in_=p_xb1[r0:r0 + p, :])
        xb1_ts.append(xb1_t)

    for i, (r0, p) in enumerate(CHUNKS):
        qin_t, xb1_t = qin_ts[i], xb1_ts[i]
        mv, rstd = layernorm_to(qin_t[:p, :], p)
        hc_t = hc_pool.tile([p, D], F32, tag="hc")
        nc.vector.tensor_scalar(
            out=hc_t[:p, :], in0=qin_t[:p, :], scalar1=mv[:p, 0:1],
            scalar2=rstd[:p, 0:1], op0=ALU.subtract, op1=ALU.mult,
        )
        if not g1_one:
            nc.vector.tensor_mul(hc_t[:p, :], hc_t[:p, :], g1b[:p, :])
        nc.vector.tensor_add(hc_t[:p, :], hc_t[:p, :], xb1_t[:p, :])

        # transpose this row-chunk into the 8 hcT column tiles
        for j in range(8):
            tp = pp_t.tile([128, 128], F32, tag="tp")
            nc.tensor.transpose(tp[:128, :p], hc_t[:p, j * 128:(j + 1) * 128], ident_f[:p, :p])
            evict_copy(hcT[j][:, r0:r0 + p], tp[:128, :p])
    t_stack.close()

    # ---------------- Phase P: projections ----------------
    # Order: K -> K-AllGather (smallest latency to first collective), then Q
    # (needed with K for the W matmuls), then V -> V-AllGather. The rings
    # serialize on the collective lane, so K's goes first.
    # wk/wv/wq all live at once (wv is re-read by the late V^T pass)
    wstream = p_stack.enter_context(tc.tile_pool(name="wstream", bufs=24))
    pr_stack = ExitStack()
    pp_a = pr_stack.enter_context(tc.tile_pool(name="pp_a", bufs=6, space="PSUM"))
    elu_pool = p_stack.enter_context(tc.tile_pool(name="elu", bufs=3))

    def elu1_evict(dst_ap, src_psum_ap, p, w):
        """dst = elu(src)+1 = relu(src) + exp(min(src,0)) (fp8 out)"""
        tmin = elu_pool.tile([128, 512], F32, tag="tmin")
        nc.vector.tensor_scalar_min(tmin[:p, :w], src_psum_ap, 0.0)
        texp = elu_pool.tile([128, 512], F32, tag="texp")
        nc.scalar.activation(texp[:p, :w], tmin[:p, :w], AF.Exp)
        nc.vector.scalar_tensor_tensor(
            out=dst_ap, in0=src_psum_ap, scalar=0.0, in1=texp[:p, :w],
            op0=ALU.max, op1=ALU.add,
        )

    # K^T -> PhiK^T (fp8) -> kag_in; two chunked AllGathers (heads 0-7, 8-15)
    wk_sb = []
    for k in range(8):
        w_t = wstream.tile([128, D], BF16, tag="wproj")
        nc.sync.dma_start(out=w_t[:], in_=p_wk[k * 128:(k + 1) * 128, :])
        wk_sb.append(w_t)
    phiK_pool = p_stack.enter_context(tc.tile_pool(name="phiK", bufs=4))
    for j in range(8):
        ps = pp_a.tile([128, 512], F32, tag="proj")
        for k in range(8):
            nc.tensor.matmul(
                ps[:], wk_sb[k][:, j * 128:(j + 1) * 128],
                hcT[k][:, 0:RO], start=(k == 0), stop=(k == 7),
            )
        phiK_t = phiK_pool.tile([128, RO], F8, tag="phiK")
        elu1_evict(phiK_t[:, :], ps[:], 128, RO)
        nc.sync.dma_start(out=kag_in[j * 128:(j + 1) * 128, :], in_=phiK_t[:])
        if j == 3:
            nc.gpsimd.collective_compute(
                "AllGather", ALU.bypass, replica_groups=GROUPS,
                ins=[kag_in[0:512, :].opt()], outs=[kag1[:].opt()],
            )
    nc.gpsimd.collective_compute(
        "AllGather", ALU.bypass, replica_groups=GROUPS,
        ins=[kag_in[512:1024, :].opt()], outs=[kag2[:].opt()],
    )

    # V in ROW-major, straight into the AllGather staging tiles: stationary
    # hcT row-block, moving Wv. One strided eviction per [128, 512] psum
    # lands 8 heads' 64-wide slices; no PE transposes, and the V AllGathers
    # launch ~35us earlier than the old V^T+transpose pipeline.
    wv_sb = []
    for k in range(8):
        w_t = wstream.tile([128, D], BF16, tag="wproj")
        nc.gpsimd.dma_start(out=w_t[:], in_=p_wv[k * 128:(k + 1) * 128, :])
        wv_sb.append(w_t)
    vs_pool = p_stack.enter_context(tc.tile_pool(name="vs", bufs=3))
    for i in range(4):
        r0 = i * 128
        vstage = vs_pool.tile([128, H * 66], F8, tag="vstage")
        vsr = vstage[:].rearrange("p (h d) -> p h d", h=H)
        for half in range(2):
            ps = pp_a.tile([128, 512], F32, tag="proj")
            for k in range(8):
                nc.tensor.matmul(
                    ps[:], hcT[k][:, r0:r0 + 128],
                    wv_sb[k][:, half * 512:(half + 1) * 512],
                    start=(k == 0), stop=(k == 7),
                )
            # psum col c -> head (half*8 + c//64), dim c%64
            evict_copy(vsr[:, half * 8:half * 8 + 8, 0:64], ps[:])
        nc.vector.memset(vsr[:, :, 64:65], 1.0)
        nc.vector.memset(vsr[:, :, 65:66], 0.0)
        nc.gpsimd.dma_start(out=vag_in[r0:r0 + 128, :], in_=vstage[:])
        if i == 1:
            nc.gpsimd.collective_compute(
                "AllGather", ALU.bypass, replica_groups=GROUPS,
                ins=[vag_in[0:256, :].opt()], outs=[vag1[:].opt()],
            )
    nc.gpsimd.collective_compute(
        "AllGather", ALU.bypass, replica_groups=GROUPS,
        ins=[vag_in[256:512, :].opt()], outs=[vag2[:].opt()],
    )

    # Q^T -> PhiQ^T (fp8, kept in SBUF)
    wq_sb = []
    for k in range(8):
        w_t = wstream.tile([128, D], BF16, tag="wproj")
        nc.gpsimd.dma_start(out=w_t[:], in_=p_wq[k * 128:(k + 1) * 128, :])
        wq_sb.append(w_t)
    for j in range(8):
        ps = pp_a.tile([128, 512], F32, tag="proj")
        for k in range(8):
            nc.tensor.matmul(
                ps[:], wq_sb[k][:, j * 128:(j + 1) * 128],
                hcT[k][:, 0:RO], start=(k == 0), stop=(k == 7),
            )
        elu1_evict(phiQT[j][:, :], ps[:], 128, RO)

    # V^T (only the attention-post subtraction needs it)
    for j in range(8):
        ps = pp_a.tile([128, 512], F32, tag="proj")
        for k in range(8):
            nc.tensor.matmul(
                ps[:], wv_sb[k][:, j * 128:(j + 1) * 128],
                hcT[k][:, 0:RO], start=(k == 0), stop=(k == 7),
            )
        evict_copy(vT[j][:, :], ps[:])
    pr_stack.close()

    p_stack.close()

    # ---------------- Phase A: attention ----------------
    # Per head-group g (4 heads): W(g) = 64 fp8 matmuls (K=64) evicted as
    # (W/32)^2 fp8 into DoubleRow pair tiles; Attr(g) = per head 8 fp8-DR
    # matmuls over (m-block pair, key) tiles. Emission order W0 W1 A0 W2 A1
    # W3 A2 A3 keeps the PE busy while the V AllGathers land.
    # Wo weights: pool created first (released after attention pools), loads
    # issued now so the Wo phase starts instantly
    wo_stack = ExitStack()
    wo_pool = wo_stack.enter_context(tc.tile_pool(name="wo", bufs=8))
    wo_sb = []
    for k in range(8):
        w_t = wo_pool.tile([128, D], BF16, tag="wo")
        nc.scalar.dma_start(out=w_t[:], in_=p_wo[k * 128:(k + 1) * 128, :])
        wo_sb.append(w_t)
    # prefetch the first FFN superchunk's gate weights during attention
    wupg_pf = []
    for k in range(8):
        wg = wup_pool.tile([128, 512], BF16, tag="wupg")
        nc.gpsimd.dma_start(out=wg[:], in_=p_wup[k * 128:(k + 1) * 128, 0:512])
        wupg_pf.append(wg)

    a_stack = ExitStack()
    kq_pool = a_stack.enter_context(tc.tile_pool(name="kq", bufs=3))
    vhd_pool = a_stack.enter_context(tc.tile_pool(name="vhd", bufs=16))
    # dual-fp8 LDWEIGHTS needs stationary width % 32 == 0: vhd is 96 wide
    # (V 0-63, ones 64, pad 65-95). DMA writes cols 0-65; zero the pad cols
    # once per pool buffer (round-robin reuse keeps them zero).
    for _ in range(16):
        vz = vhd_pool.tile([128, 8, 96], F8, tag="vhd")
        nc.vector.memset(vz[:, :, 66:96], 0.0)
    wt_pool = a_stack.enter_context(tc.tile_pool(name="wt", bufs=64))
    asm_pool = a_stack.enter_context(tc.tile_pool(name="asm", bufs=4))
    pp_w = a_stack.enter_context(tc.tile_pool(name="pp_w", bufs=5, space="PSUM"))
    pp_at = a_stack.enter_context(tc.tile_pool(name="pp_at", bufs=3, space="PSUM"))

    sq_state = [0]

    def square_evict(dst_ap, src_psum_ap):
        """dst = src^2 fp8; src is already W/32 (W > 0, relu is a no-op).
        Rotated 5:2 across ACT/DVE: ACT streams ~1 col/ns single-pass; the
        DVE two-pass path costs ~2x that, so it only soaks the overflow."""
        sq_state[0] = (sq_state[0] + 1) % 7
        if sq_state[0] < 5:
            nc.scalar.activation(dst_ap, src_psum_ap, AF.Square, scale=WSC)
        else:
            tr = asm_pool.tile([128, 512], BF16, tag="r2tmpv")
            nc.vector.tensor_scalar_mul(tr[:, :], src_psum_ap, WSC)
            nc.vector.tensor_mul(dst_ap, tr[:, :], tr[:, :])

    def emit_w(hg):
        """W^T for 4 heads of group hg -> wtp fp8 DoubleRow pair tiles.
        The two heads sharing a kq/phiQT tile (PE row halves 0-63 / 64-127)
        are interleaved: consecutive matmuls hit disjoint row groups, so
        LDWEIGHTS overlaps the in-flight matmul."""
        kag_t = kag1 if hg < 2 else kag2
        kq_sb = {}
        for j2 in (2 * hg, 2 * hg + 1):
            hrow = (j2 % 4) * 128  # row offset of head-pair j2 within kag_t
            kt = kq_pool.tile([128, NQ, RO], F8, tag="kq")
            ksrc = kag_t[:, :]
            nc.sync.dma_start(
                out=kt[:, :, :],
                in_=bass.AP(tensor=ksrc.tensor, offset=ksrc.offset + hrow * RO,
                            ap=[[RO, 128], [512 * RO, NQ], [1, RO]]),
            )
            kq_sb[j2] = kt
        wtp = {}
        for hh in range(4):
            h = hg * 4 + hh
            wtp[h] = [wt_pool.tile([128, 2, RO], F8, tag="wt", name=f"wt{h}_{t}")
                      for t in range(8)]
        for j2 in (2 * hg, 2 * hg + 1):
            hA, hB = 2 * j2, 2 * j2 + 1
            for m in range(16):
                qq, lc = m // 4, m % 4
                for off, h in ((0, hA), (64, hB)):
                    psw = pp_w.tile([128, 512], F32, tag="psw")
                    nc.tensor.matmul(
                        psw[:], kq_sb[j2][off:off + 64, qq, lc * 128:(lc + 1) * 128],
                        phiQT[j2][off:off + 64, :], start=True, stop=True,
                    )
                    square_evict(wtp[h][m // 2][:, m % 2, :], psw)
        return wtp

    # pair order follows the chunked V gathers: vag1 pairs (lc 0,1) first
    T_ORDER = [qq * 2 for qq in range(NQ)] + [qq * 2 + 1 for qq in range(NQ)]

    def emit_attr(hg, wtp):
        pats = []
        for hh in range(4):
            h = hg * 4 + hh
            pat = pp_at.tile([96, 512], F32, tag="pat", name=f"pat{h}")
            vh = {}
            for half in range(2):
                vsrc = vag1 if half == 0 else vag2
                vt = vhd_pool.tile([128, 8, 96], F8, tag="vhd")
                vap = vsrc[:, :]
                W16 = H * 66
                nc.sync.dma_start(
                    out=vt[:, :, 0:66],
                    in_=bass.AP(tensor=vap.tensor, offset=vap.offset + h * 66,
                                ap=[[W16, 128], [128 * W16, 8], [1, 66]]))
                vh[half] = vt
            for ti, t in enumerate(T_ORDER):
                qq, half = t // 2, t % 2
                nc.tensor.matmul(
                    pat[:], vh[half][:, 2 * qq:2 * qq + 2, :], wtp[h][t][:, :, :],
                    start=(ti == 0), stop=(ti == 7), perf_mode=DR,
                )
            pats.append(pat)
        for hh in range(4):
            h = hg * 4 + hh
            j2, off = h // 2, (h % 2) * 64
            nrm = asm_pool.tile([1, RO], F32, tag="nrm")
            nc.vector.tensor_scalar_add(nrm[0:1, :], pats[hh][64:65, :], WSC2)
            nc.vector.reciprocal_approx_fast(out=nrm[:], in_=nrm[:])
            rcb = asm_pool.tile([64, RO], F32, tag="rcb")
            nc.gpsimd.partition_broadcast(rcb[:], nrm[:])
            tm = asm_pool.tile([128, RO], F32, tag="tm")
            nc.vector.tensor_mul(tm[off:off + 64, :], pats[hh][0:64, :], rcb[:, :])
            nc.vector.tensor_sub(
                mTc[j2][off:off + 64, :], tm[off:off + 64, :],
                vT[j2][off:off + 64, :],
            )

    wtp_q = [emit_w(0), emit_w(1)]
    for hg in range(4):
        emit_attr(hg, wtp_q[hg])
        if hg + 2 < 4:
            wtp_q.append(emit_w(hg + 2))
    a_stack.close()

    # ---------------- Phase A5: Wo + Q_interact ----------------
    # Chunk order 0,3,1,2 so the conv-halo boundary rows exist after two
    # chunks and their AllGather overlaps the rest of Wo + LN2.
    a5_stack = ExitStack()
    qi_pool = a5_stack.enter_context(tc.tile_pool(name="qi", bufs=3))
    pp_o = a5_stack.enter_context(tc.tile_pool(name="pp_o", bufs=4, space="PSUM"))
    qint = [None] * 4
    for oi, i in enumerate((0, 3, 1, 2)):
        r0, p = CHUNKS[i]
        qin_t = qi_pool.tile([p, D], F32, tag="qin2")
        nc.sync.dma_start(out=qin_t[:], in_=p_qin[r0:r0 + p, :])
        qi = qint_pool.tile([p, D], F32, name=f"qint{i}")
        for half in range(2):
            pso = pp_o.tile([128, 512], F32, tag="pso")
            for k in range(8):
                nc.tensor.matmul(
                    pso[:p, :], mTc[k][:, r0:r0 + p],
                    wo_sb[k][:, half * 512:(half + 1) * 512],
                    start=(k == 0), stop=(k == 7),
                )
            nc.vector.scalar_tensor_tensor(
                out=qi[:p, half * 512:(half + 1) * 512], in0=pso[:p, :],
                scalar=dt_safe, in1=qin_t[:p, half * 512:(half + 1) * 512],
                op0=ALU.mult, op1=ALU.add,
            )
        qint[i] = qi
        if oi == 1:
            # boundary rows ready: stage + AllGather (conv halo exchange)
            nc.gpsimd.dma_start(out=hag_in[0:1, :], in_=qint[0][0:1, :])
            nc.gpsimd.dma_start(out=hag_in[1:2, :], in_=qint[3][127:128, :])
            nc.gpsimd.collective_compute(
                "AllGather", ALU.bypass, replica_groups=GROUPS,
                ins=[hag_in[:].opt()], outs=[hag[:].opt()],
            )
    a5_stack.close()
    wo_stack.close()
    av_stack.close()   # frees vT, phiQ
    av2_stack.close()  # frees mTc

    # ---------------- Phase F: LN2 + transpose + FFN ----------------
    hfc = []

    qn2T_pool = f34_stack.enter_context(tc.tile_pool(name="qn2T", bufs=1))
    f_stack = ExitStack()
    qn2_pool = f_stack.enter_context(tc.tile_pool(name="qn2", bufs=2))
    pp_f = f_stack.enter_context(tc.tile_pool(name="pp_f", bufs=4, space="PSUM"))
    # qn2T cols: 0..511 owned rows, 512 = prev-halo row, 513 = next-halo row
    qn2T = [qn2T_pool.tile([128, RO + 2], BF16, name=f"qn2T{j}") for j in range(8)]
    for i, (r0, p) in enumerate(CHUNKS):
        mv, rstd = layernorm_to(qint[i][:p, :], p)
        qn2_t = qn2_pool.tile([p, D], F32, tag="qn2")
        nc.vector.tensor_scalar(
            out=qn2_t[:p, :], in0=qint[i][:p, :], scalar1=mv[:p, 0:1],
            scalar2=rstd[:p, 0:1], op0=ALU.subtract, op1=ALU.mult,
        )
        for j in range(8):
            tp = pp_f.tile([128, 128], F32, tag="tpf")
            nc.tensor.transpose(tp[:128, :p], qn2_t[:p, j * 128:(j + 1) * 128], ident_f[:p, :p])
            evict_copy(qn2T[j][:, r0:r0 + p], tp[:128, :p])

    # halo rows: extract prev/next boundary rows via maskmat.T @ gathered,
    # then LN2 + transpose into qn2T cols 512/513
    pp_h = f_stack.enter_context(tc.tile_pool(name="pp_h", bufs=1, space="PSUM"))
    hg_sb = qn2_pool.tile([2 * NQ, D], F32R, name="hg_sb")
    nc.sync.dma_start(out=hg_sb[:], in_=hag[:, :])
    qih = qn2_pool.tile([2, D], F32, name="qih")
    for half in range(2):
        ph = pp_h.tile([2, 512], F32, tag="psh", name=f"ph{half}")
        nc.tensor.matmul(
            ph[:], maskmat[:], hg_sb[:, half * 512:(half + 1) * 512],
            start=True, stop=True,
        )
        nc.vector.tensor_copy(qih[:, half * 512:(half + 1) * 512], ph[:])
    mv, rstd = layernorm_to(qih[:2, :], 2)
    qn2h = qn2_pool.tile([2, D], F32, name="qn2h")
    nc.vector.tensor_scalar(
        out=qn2h[:2, :], in0=qih[:2, :], scalar1=mv[:2, 0:1],
        scalar2=rstd[:2, 0:1], op0=ALU.subtract, op1=ALU.mult,
    )
    for j in range(8):
        tp = pp_f.tile([128, 128], F32, tag="tpf")
        nc.tensor.transpose(tp[:128, :2], qn2h[:2, j * 128:(j + 1) * 128], ident_f[:2, :2])
        evict_copy(qn2T[j][:, RO:RO + 2], tp[:128, :2])
    f_stack.close()

    # Wup (fp8 DoubleRow, K=256 per matmul) + SwiGLU + depthwise conv, in
    # 512-col superchunks; Wdown's first column-half rides along, one inner
    # pair behind the conv. Scales: wup carries x32, wdown x64 (host side);
    # the 1/32 descale folds into the Silu input scale / U bias / conv taps,
    # the 1/64 into the output eviction.
    f2_stack = ExitStack()
    pp_d = f2_stack.enter_context(tc.tile_pool(name="pp_d", bufs=1, space="PSUM"))
    ffn_stack = ExitStack()
    gu_stack = ExitStack()
    pp_g = gu_stack.enter_context(tc.tile_pool(name="pp_g", bufs=2, space="PSUM"))
    pp_u = gu_stack.enter_context(tc.tile_pool(name="pp_u", bufs=2, space="PSUM"))
    wupu_pool = ffn_stack.enter_context(tc.tile_pool(name="wupu", bufs=12))
    fsm_pool = ffn_stack.enter_context(tc.tile_pool(name="fsm", bufs=3))
    bias_pool = ffn_stack.enter_context(tc.tile_pool(name="bias", bufs=6))
    wd0_pool = ffn_stack.enter_context(tc.tile_pool(name="wd0", bufs=4))
    wd1_pool = ffn_stack.enter_context(tc.tile_pool(name="wd1", bufs=16))

    psd0 = [pp_d.tile([128, 512], F32, name=f"psd0_{i}") for i in range(4)]
    # hfc: fp8 DoubleRow pair tiles; pair t holds inner blocks (2t, 2t+1)
    for t in range(16):
        hfc.append(hfc_pool.tile([128, 2, RO], F8, name=f"hfc{t}"))
    wd0_sb = {}

    def emit_wdown_pair(t):
        wd_t = wd0_sb.pop(t)
        for i in range(4):
            nc.tensor.matmul(
                psd0[i][:], hfc[t][:, :, i * 128:(i + 1) * 128],
                wd_t[:, :, :], start=(t == 0), stop=(t == 15), perf_mode=DR,
            )

    wd1_sb = []
    for sc in range(8):
        if sc == 0:
            wupg_sb = wupg_pf
        else:
            wupg_sb = []
            for k in range(8):
                wg = wup_pool.tile([128, 512], BF16, tag="wupg")
                nc.sync.dma_start(
                    out=wg[:], in_=p_wup[k * 128:(k + 1) * 128, sc * 512:(sc + 1) * 512]
                )
                wupg_sb.append(wg)
        wupu_sb = []
        for k in range(8):
            wu = wupu_pool.tile([128, 512], BF16, tag="wupu")
            nc.scalar.dma_start(
                out=wu[:], in_=p_wup[k * 128:(k + 1) * 128, INNER + sc * 512:INNER + (sc + 1) * 512]
            )
            wupu_sb.append(wu)
        for c in range(4):
            cc = sc * 4 + c
            if cc % 2 == 0:
                tn = cc // 2
                wd_t = wd0_pool.tile([128, 2, 512], F8, tag="wd0")
                nc.gpsimd.dma_start(out=wd_t[:, :, :], in_=p_wd08[tn * 128:(tn + 1) * 128, :])
                wd0_sb[tn] = wd_t
                # prefetch the matching second-half tile for the tail phase
                wd1_t = wd1_pool.tile([128, 2, 512], F8, tag="wd1")
                nc.gpsimd.dma_start(out=wd1_t[:, :, :], in_=p_wd18[tn * 128:(tn + 1) * 128, :])
                wd1_sb.append(wd1_t)
            bg = bias_pool.tile([128, 1], F32, tag="bg")
            nc.gpsimd.dma_start(out=bg[:], in_=bass.AP(tensor=p_bgu, offset=cc * 128, ap=[[1, 128], [1, 1]]))
            bu = bias_pool.tile([128, 1], F32, tag="bu")
            nc.gpsimd.dma_start(out=bu[:], in_=bass.AP(tensor=p_bgu, offset=INNER + cc * 128, ap=[[1, 128], [1, 1]]))
            cw = bias_pool.tile([128, 3], F32, tag="cw")
            nc.gpsimd.dma_start(out=cw[:], in_=p_cw[cc * 128:(cc + 1) * 128, :])

            gact = fsm_pool.tile([128, RO + 2], F32, tag="gact")
            hf = fsm_pool.tile([128, RO + 2], F32, tag="hf")
            for h0, w in HALVES:
                psg = pp_g.tile([128, 258], F32, tag="psg")
                for k in range(8):
                    nc.tensor.matmul(
                        psg[:, :w], wupg_sb[k][:, c * 128:(c + 1) * 128],
                        qn2T[k][:, h0:h0 + w], start=(k == 0), stop=(k == 7),
                    )
                nc.scalar.activation(gact[:, h0:h0 + w], psg[:, :w], AF.Silu, bias=bg[:, 0:1])
                psu = pp_u.tile([128, 258], F32, tag="psu")
                for k in range(8):
                    nc.tensor.matmul(
                        psu[:, :w], wupu_sb[k][:, c * 128:(c + 1) * 128],
                        qn2T[k][:, h0:h0 + w], start=(k == 0), stop=(k == 7),
                    )
                nc.vector.scalar_tensor_tensor(
                    out=hf[:, h0:h0 + w], in0=psu[:, :w], scalar=bu[:, 0:1],
                    in1=gact[:, h0:h0 + w], op0=ALU.add, op1=ALU.mult,
                )
            if cc >= 2 and cc % 2 == 0:
                emit_wdown_pair(cc // 2 - 1)
            # mask halo cols at batch edges (conv zero-pad)
            nc.vector.tensor_scalar_mul(hf[:, RO:RO + 1], hf[:, RO:RO + 1], mask_p[:, 0:1])
            nc.vector.tensor_scalar_mul(hf[:, RO + 1:RO + 2], hf[:, RO + 1:RO + 2], mask_n[:, 0:1])
            # depthwise conv along rows: cols 0..511 owned, 512=prev, 513=next.
            # center tap on ACT (scale is per-partition), side taps DVE; the
            # final two taps write the fp8 DoubleRow pair tile directly.
            hfb = fsm_pool.tile([128, RO], BF16, tag="hfb")
            dst = hfc[cc // 2]
            jj = cc % 2
            nc.scalar.activation(hfb[:, 0:RO], hf[:, 0:RO], AF.Copy, scale=cw[:, 1:2])
            nc.vector.scalar_tensor_tensor(
                out=hfb[:, 1:RO], in0=hf[:, 0:RO - 1], scalar=cw[:, 0:1],
                in1=hfb[:, 1:RO], op0=ALU.mult, op1=ALU.add,
            )
            nc.vector.scalar_tensor_tensor(
                out=hfb[:, 0:1], in0=hf[:, RO:RO + 1], scalar=cw[:, 0:1],
                in1=hfb[:, 0:1], op0=ALU.mult, op1=ALU.add,
            )
            nc.vector.scalar_tensor_tensor(
                out=dst[:, jj, 0:RO - 1], in0=hf[:, 1:RO], scalar=cw[:, 2:3],
                in1=hfb[:, 0:RO - 1], op0=ALU.mult, op1=ALU.add,
            )
            nc.vector.scalar_tensor_tensor(
                out=dst[:, jj, RO - 1:RO], in0=hf[:, RO + 1:RO + 2], scalar=cw[:, 2:3],
                in1=hfb[:, RO - 1:RO], op0=ALU.mult, op1=ALU.add,
            )
    emit_wdown_pair(15)
    gu_stack.close()

    # ---------------- Phase F4: Wdown second half + residual + output ----------
    pp_d1 = f2_stack.enter_context(tc.tile_pool(name="pp_d1", bufs=1, space="PSUM"))
    psd1 = [pp_d1.tile([128, 512], F32, name=f"psd1_{i}") for i in range(4)]
    for t in range(16):
        for i in range(4):
            nc.tensor.matmul(
                psd1[i][:], hfc[t][:, :, i * 128:(i + 1) * 128],
                wd1_sb[t][:, :, :], start=(t == 0), stop=(t == 15), perf_mode=DR,
            )
    ffn_stack.close()
    out_pool = f2_stack.enter_context(tc.tile_pool(name="outp", bufs=4))
    for i in range(4):
        o_t = out_pool.tile([128, D], F32, tag="osb")
        nc.vector.scalar_tensor_tensor(
            out=o_t[:, 0:512], in0=psd0[i][:], scalar=WDINV,
            in1=qint[i][:, 0:512], op0=ALU.mult, op1=ALU.add,
        )
        nc.vector.scalar_tensor_tensor(
            out=o_t[:, 512:1024], in0=psd1[i][:], scalar=WDINV,
            in1=qint[i][:, 512:1024], op0=ALU.mult, op1=ALU.add,
        )
        out_q = (nc.sync, nc.scalar, nc.gpsimd, nc.sync)[i]
        out_q.dma_start(out=p_out[i * 128:(i + 1) * 128, :], in_=o_t[:])
    f2_stack.close()
    f34_stack.close()


def kernel(**inputs) -> np.ndarray:
    Q_in = np.ascontiguousarray(np.asarray(inputs["Q_in"], dtype=np.float32))
    X = np.ascontiguousarray(np.asarray(inputs["X"], dtype=np.float32))
    Wq = np.asarray(inputs["Wq"], dtype=np.float32)
    Wk = np.asarray(inputs["Wk"], dtype=np.float32)
    Wv = np.asarray(inputs["Wv"], dtype=np.float32)
    Wo = np.asarray(inputs["Wo"], dtype=np.float32)
    Wup = np.asarray(inputs["Wup"], dtype=np.float32)
    conv_w = np.asarray(inputs["conv_w"], dtype=np.float32)
    Wdown = np.asarray(inputs["Wdown"], dtype=np.float32)
    g1 = np.asarray(inputs["g1"], dtype=np.float32)
    b1 = np.asarray(inputs["b1"], dtype=np.float32)
    g2 = np.asarray(inputs["g2"], dtype=np.float32)
    b2 = np.asarray(inputs["b2"], dtype=np.float32)
    dt = float(np.asarray(inputs["dt"], dtype=np.float32))

    # softplus(dt) on host; baked into the NEFF as an immediate
    dt_safe = float(np.log1p(np.exp(dt)))

    # fold g2/b2 into Wup (LN2's affine commutes into the up-projection)
    wup_f = g2[:, None] * Wup
    bias_gu = np.ascontiguousarray(b2 @ Wup)
    cw3 = np.ascontiguousarray(conv_w[:, 0, :])

    g1_one = bool(np.all(g1 == 1.0))
    key = (round(dt_safe, 9), g1_one)
    if key not in _cache:
        _cache[key] = _build(dt_safe, g1_one)
    nc = _cache[key]

    bf = ml_dtypes.bfloat16
    f8 = ml_dtypes.float8_e4m3
    wq_b = np.ascontiguousarray(Wq.astype(bf))
    wk_b = np.ascontiguousarray(Wk.astype(bf))
    wv_b = np.ascontiguousarray(Wv.astype(bf))
    wo_b = np.ascontiguousarray(Wo.astype(bf))
    wup_b = np.ascontiguousarray(wup_f.astype(bf))

    # Wdown DoubleRow pair packing: tile-row t*128+r holds inner element
    # i = 128*(2t+j)+r in slot j; cols are (j, c).
    def pack_down(w):  # [INNER, 512] -> [INNER//2, D] fp8 pair layout
        t = w.reshape(16, 2, 128, 512)
        return np.ascontiguousarray(
            t.transpose(0, 2, 1, 3).reshape(INNER // 2, D).astype(f8))

    wd08 = pack_down(WDS * Wdown[:, 0:512])
    wd18 = pack_down(WDS * Wdown[:, 512:1024])

    in_maps = []
    for core in range(8):
        b, q = divmod(core, 4)
        qin = np.ascontiguousarray(Q_in[b, q * RO:(q + 1) * RO])
        xb1 = np.ascontiguousarray(X[b, q * RO:(q + 1) * RO] + b1[None, :])
        masks = np.array(
            [1.0 if q > 0 else 0.0, 1.0 if q < NQ - 1 else 0.0], dtype=np.float32
        )
        # maskmat.T @ gathered_boundaries = [prev-halo row; next-halo row]
        mm = np.zeros((2 * NQ, 2), dtype=np.float32)
        if q > 0:
            mm[2 * (q - 1) + 1, 0] = 1.0
        if q < NQ - 1:
            mm[2 * (q + 1), 1] = 1.0
        in_maps.append({
            "q_in": qin, "x_b1": xb1, "wq": wq_b, "wk": wk_b, "wv": wv_b,
            "wo": wo_b, "wup": wup_b, "bias_gu": bias_gu,
            "wd08": wd08, "wd18": wd18, "cw3": cw3,
            "g1": np.ascontiguousarray(g1), "masks": masks, "maskmat": mm,
        })

    global _last_in_maps
    _last_in_maps = in_maps
    res = run_bass_kernel_spmd(nc, in_maps, core_ids=list(range(8)))

    out = np.empty((B, N, D), dtype=np.float32)
    for core in range(8):
        b, q = divmod(core, 4)
        out[b, q * RO:(q + 1) * RO] = res.results[core]["out"]
    return out

